# revision 1
# baseline (speedup 1.0000x reference)
"""2-layer GAT kernel for Trainium2 (8 NeuronCores), Bass/Tile.

Sharding: nodes by dst across 8 cores; edges routed to the dst owner.
Per core, edges split into two passes by src half (dma_gather idx is int16
-> gather tables limited to <=32768 rows). Per pass, dst nodes are sorted
by per-pass degree and packed into 128-partition tiles with compile-time
slot budgets D[t]; edge j of dst node d sits at (partition d, slot j).
Pad slots point at a sentinel table row whose a_src = -1e4 => p = 0.

Per slot-grid tile:
    gather rows [h | a_src] of table[src] (gpsimd.dma_gather from HBM)
    alpha = a_src + a_dst[d]  (a_dst per-partition, free-axis broadcast)
    alpha = max(alpha, 0.2*alpha);  p = exp(alpha)   (no max-subtraction:
        logits are O(1) for this model, exp is safe in f32)
    out[d,:] = sum_j p * h_j   (DVE strided reduce, f32 accumulation)
    den[d]   = sum_j p
Partials (out|den) per pass go to HBM scratch in pass order; a combine pass
gathers both passes' rows by permutation index, normalizes by 1/(den+eps),
adds bias (+elu between layers).

Layer tables: t1 = x @ [W1 | W1@Asrc | W1@Adst] (f32, 512B rows), built
replicated on every core from host-pre-transposed x. Between layers,
h^T = elu(out1)^T (bf16) is AllGathered and every core builds the full
t2 = h @ [W2 | W2@att_src2^T | W2@att_dst2^T] (bf16, 512B rows).
a_dst1 comes pre-permuted from the host (it equals x @ W1@Adst, which the
host can compute); a_dst2 is computed on-device per natural tile via a tiny
matmul, staged to HBM scratch, and permutation-gathered per pass.
"""

import numpy as np


class _StopBuild(Exception):
    pass


import concourse.bacc as bacc
import concourse.bass as bass
import concourse.mybir as mybir
import concourse.tile as tile
from concourse._compat import cdiv
from concourse.bass_utils import run_bass_kernel_spmd

AF = mybir.ActivationFunctionType
ALU = mybir.AluOpType
AX = mybir.AxisListType
DT = mybir.dt

NEG_SLOPE = 0.2
EPS = 1e-16
SENT_VAL = -1e4


# ----------------------------------------------------------------------------
# Configuration (all compile-time, data-independent)
# ----------------------------------------------------------------------------
class Cfg:
    def __init__(self, N=50000, F=128, H=8, C1=8, C2=128, E=1600000, ncores=8,
                 group_cols=48, margin=2):
        assert N % (2 * ncores) == 0
        self.N, self.F, self.H, self.C1, self.C2, self.E = N, F, H, C1, C2, E
        self.ncores = ncores
        self.nloc = N // ncores
        self.half = N // 2
        self.ntiles = cdiv(self.nloc, 128)
        self.nrows_pad = self.ntiles * 128
        self.d1 = H * C1                     # layer-1 width (64)
        self.t1_cols = 128                   # f32 -> 512B rows
        self.t2_cols = 256                   # bf16 -> 512B rows
        self.p1_cols = 128                   # partial rows l1: [agg 64|den 8] pad
        self.p2_cols = 192                   # partial rows l2: [agg 128|den 1] pad
        self.group_cols = group_cols
        lam = (E + N) / N / 2.0
        self.D = _budgets(self.nloc, self.ntiles, lam, margin)
        self.total_cols = int(sum(self.D))
        self.col_off = np.concatenate([[0], np.cumsum(self.D)]).astype(int)
        self.groups = []
        t = 0
        while t < self.ntiles:
            t0, c0 = t, int(self.col_off[t])
            cols = 0
            while t < self.ntiles and (cols == 0 or cols + self.D[t] <= group_cols):
                cols += self.D[t]
                t += 1
            self.groups.append((t0, t, c0, cols))
        self.max_group_cols = max(g[3] for g in self.groups)


def _budgets(nloc, ntiles, lam, margin):
    rng = np.random.default_rng(20260805)
    mx = np.zeros(ntiles, dtype=np.int64)
    for _ in range(24):
        s = np.sort(rng.poisson(lam, nloc) + 1)[::-1]
        pad = np.zeros(ntiles * 128, dtype=np.int64)
        pad[:min(nloc, ntiles * 128)] = s[:ntiles * 128]
        mx = np.maximum(mx, pad.reshape(ntiles, 128).max(axis=1))
    return (mx + margin).astype(int)


# ----------------------------------------------------------------------------
# Host-side routing
# ----------------------------------------------------------------------------
def _wrap_idx(idx):
    """[n] -> [128, n/16] int16: position j -> (partition j%16, col j//16),
    replicated across the 8 groups of 16 partitions."""
    idx = np.asarray(idx, dtype=np.int16)
    assert len(idx) % 16 == 0
    return np.tile(idx.reshape(-1, 16).T, (8, 1))


def _route_core(cfg, src, dst, core, adst1_full):
    nloc, half = cfg.nloc, cfg.half
    base = core * nloc
    m = (dst >= base) & (dst < base + nloc)
    s_c = src[m]
    d_c = (dst[m] - base).astype(np.int64)
    gidx, aidx, cidx, adst1p = [], [], [], []
    for s in (0, 1):
        m2 = (s_c // half) == s
        ss = (s_c[m2] % half).astype(np.int64)
        dd = d_c[m2]
        degs = np.bincount(dd, minlength=nloc)
        order = np.argsort(-degs, kind="stable")
        rank = np.empty(nloc, dtype=np.int64)
        rank[order] = np.arange(nloc)
        eo = np.lexsort((ss, dd))
        ss_o, dd_o = ss[eo], dd[eo]
        starts = np.concatenate([[0], np.cumsum(degs)])
        j = np.arange(len(dd_o)) - starts[dd_o]
        r = rank[dd_o]
        tile_e, row_e = r // 128, r % 128
        Dv = np.asarray(cfg.D)
        keep = j < Dv[tile_e]
        if (~keep).any():
            print(f"WARNING core {core} pass {s}: dropping {int((~keep).sum())} "
                  f"edges over slot budget")
            ss_o, j, tile_e, row_e = ss_o[keep], j[keep], tile_e[keep], row_e[keep]
        flat = np.full(cfg.total_cols * 128, half, dtype=np.int64)   # sentinel
        flat[(cfg.col_off[tile_e] + j) * 128 + row_e] = ss_o
        gidx.append(flat)
        # local node ids in pass order (for the on-device a_dst2 perm gather)
        ap = np.full(cfg.nrows_pad, cfg.nrows_pad - 1, dtype=np.int64)
        ap[:nloc] = order
        aidx.append(ap)
        # combine perm: natural node n -> its partial row (= rank)
        cb = np.zeros(cfg.nrows_pad, dtype=np.int64)
        cb[:nloc] = rank
        cidx.append(cb)
        # host-computed a_dst1, permuted to pass order [nrows_pad, H]
        a = np.full((cfg.nrows_pad, cfg.H), SENT_VAL, dtype=np.float32)
        a[:nloc] = adst1_full[base + order]
        adst1p.append(a)
    return {
        "gidx": _wrap_idx(np.concatenate(gidx)),
        "aidx": _wrap_idx(np.concatenate(aidx)),
        "cidx": _wrap_idx(np.concatenate(cidx)),
        "adst1p": np.concatenate(adst1p, axis=0),
    }


# ----------------------------------------------------------------------------
# Device program
# ----------------------------------------------------------------------------
def build_program(cfg, stop_after=99):
    from concourse.masks import make_identity

    nc = bacc.Bacc(None, target_bir_lowering=False, debug=True)
    H, d1, C2, F = cfg.H, cfg.d1, cfg.C2, cfg.F
    nloc, half, ntiles = cfg.nloc, cfg.half, cfg.ntiles
    nfull_tiles = cdiv(cfg.N, 128)
    P1C, P2C = cfg.p1_cols, cfg.p2_cols
    tail = nloc - (ntiles - 1) * 128
    NRP = cfg.nrows_pad
    GC = cfg.max_group_cols

    # ---- external IO ----
    xT = nc.dram_tensor("xT", [F, cfg.N], DT.float32, kind="ExternalInput")
    w1aug_h = nc.dram_tensor("w1aug", [F, d1 + 2 * H], DT.float32, kind="ExternalInput")
    w2aug_h = nc.dram_tensor("w2aug", [d1, C2 + 2], DT.bfloat16, kind="ExternalInput")
    b1_h = nc.dram_tensor("bias1r", [128, d1], DT.float32, kind="ExternalInput")
    b2_h = nc.dram_tensor("bias2r", [128, C2], DT.float32, kind="ExternalInput")
    sent1_h = nc.dram_tensor("sent1", [1, cfg.t1_cols], DT.float32, kind="ExternalInput")
    sent2_h = nc.dram_tensor("sent2", [1, cfg.t2_cols], DT.bfloat16, kind="ExternalInput")
    gidx_h = nc.dram_tensor("gidx", [128, 2 * cfg.total_cols * 8], DT.int16, kind="ExternalInput")
    aidx_h = nc.dram_tensor("aidx", [128, 2 * NRP // 16], DT.int16, kind="ExternalInput")
    cidx_h = nc.dram_tensor("cidx", [128, 2 * NRP // 16], DT.int16, kind="ExternalInput")
    adst1p_h = nc.dram_tensor("adst1p", [2 * NRP, H], DT.float32, kind="ExternalInput")
    out_h = nc.dram_tensor("out", [nloc, C2], DT.float32, kind="ExternalOutput")

    # ---- internal DRAM ----
    t1 = [nc.dram_tensor(f"t1_{s}", [half + 1, cfg.t1_cols], DT.float32) for s in range(2)]
    t2 = [nc.dram_tensor(f"t2_{s}", [half + 1, cfg.t2_cols], DT.bfloat16) for s in range(2)]
    part1 = [nc.dram_tensor(f"part1_{s}", [NRP, P1C], DT.float32) for s in range(2)]
    part2 = [nc.dram_tensor(f"part2_{s}", [NRP, P2C], DT.float32) for s in range(2)]
    adst2sc = nc.dram_tensor("adst2sc", [NRP, 64], DT.float32)
    hT_shard = nc.dram_tensor("hT_shard", [d1, nloc], DT.bfloat16)
    hT_full = nc.dram_tensor("hT_full", [cfg.ncores * d1, nloc], DT.bfloat16)

    try:
      with tile.TileContext(nc) as tc:
        with tc.tile_pool(name="const", bufs=1) as cpool:
            w1s = cpool.tile([F, d1 + 2 * H], DT.float32)
            nc.sync.dma_start(w1s[:], w1aug_h[:])
            w2s = cpool.tile([d1, C2 + 2], DT.bfloat16)
            nc.sync.dma_start(w2s[:], w2aug_h[:])
            b1s = cpool.tile([128, d1], DT.float32)
            nc.sync.dma_start(b1s[:], b1_h[:])
            b2s = cpool.tile([128, C2], DT.float32)
            nc.sync.dma_start(b2s[:], b2_h[:])
            ident = cpool.tile([128, 128], DT.float32)
            make_identity(nc, ident[:])
            adst2nat = cpool.tile([128, ntiles], DT.float32)

            # ================= P0: layer-1 table =================
            with tc.tile_pool(name="p0", bufs=3) as p0, \
                 tc.tile_pool(name="p0ps", bufs=4, space="PSUM") as p0ps:
                sc1 = p0.tile([1, cfg.t1_cols], DT.float32, tag="sent")
                nc.sync.dma_start(sc1[:], sent1_h[:])
                for s in range(2):
                    nc.sync.dma_start(t1[s][half:half + 1, :], sc1[:])
                for k in range(nfull_tiles):
                    n0 = k * 128
                    cnt = min(128, cfg.N - n0)
                    xt_t = p0.tile([F, 128], DT.float32, tag="xt")
                    nc.sync.dma_start(xt_t[:, :cnt], xT[:, n0:n0 + cnt])
                    ps = p0ps.tile([128, d1 + 2 * H], DT.float32, tag="ps", space="PSUM")
                    nc.tensor.matmul(ps[:cnt, :], lhsT=xt_t[:, :cnt], rhs=w1s[:],
                                     start=True, stop=True)
                    row = p0.tile([128, d1 + 2 * H], DT.float32, tag="row")
                    nc.any.tensor_copy(out=row[:cnt, :], in_=ps[:cnt, :])
                    for s in range(2):
                        lo, hi = max(n0, s * half), min(n0 + cnt, (s + 1) * half)
                        if lo < hi:
                            nc.sync.dma_start(
                                t1[s][lo - s * half:hi - s * half, 0:d1 + 2 * H],
                                row[lo - n0:hi - n0, :])

            # ================= pass machinery =================
            def run_pass(layer, s, tbl, elem, tdt, part, dfeat, nheads, adst_src):
                base_cols = s * cfg.total_cols
                with tc.tile_pool(name=f"ap{layer}{s}", bufs=1) as apl, \
                     tc.tile_pool(name=f"pass{layer}{s}", bufs=2) as pp:
                    adst_all = adst_src(apl, s)   # [128, ntiles, nheads] f32
                    for (t0, t1_, c0, ncols) in cfg.groups:
                        gi = pp.tile([128, GC * 8], DT.int16, tag="gi")
                        nc.sync.dma_start(
                            gi[:, :ncols * 8],
                            gidx_h[:, (base_cols + c0) * 8:(base_cols + c0 + ncols) * 8])
                        G = pp.tile([128, GC, elem], tdt, tag="G")
                        nc.gpsimd.dma_gather(G[:, :ncols, :], tbl[s][:],
                                             gi[:, :ncols * 8], ncols * 128,
                                             ncols * 128, elem, single_packet=False)
                        pex = pp.tile([128, GC, dfeat], tdt, tag="pex")
                        for t in range(t0, t1_):
                            D = int(cfg.D[t])
                            o = int(cfg.col_off[t]) - c0
                            Gt = G[:, o:o + D, :]
                            if layer == 1:
                                asrc = Gt[:, :, d1:d1 + H]
                            else:
                                asrc = Gt[:, :, C2:C2 + 1]
                            al = pp.tile([128, GC, nheads], DT.float32, tag="al")
                            alt = al[:, :D, :]
                            nc.vector.tensor_tensor(
                                out=alt, in0=asrc,
                                in1=adst_all[:, t:t + 1, :].to_broadcast([128, D, nheads]),
                                op=ALU.add)
                            nc.vector.scalar_tensor_tensor(
                                out=alt, in0=alt, scalar=NEG_SLOPE, in1=alt,
                                op0=ALU.mult, op1=ALU.max)
                            pext = pex[:, o:o + D, :]
                            nc.scalar.activation(
                                out=pext,
                                in_=alt.rearrange("p j (h c) -> p j h c", c=1)
                                       .to_broadcast([128, D, nheads, dfeat // nheads]),
                                func=AF.Exp)
                            res = pp.tile([128, dfeat + nheads], DT.float32, tag="res")
                            nc.vector.tensor_reduce(
                                out=res[:, dfeat:dfeat + nheads],
                                in_=pext.rearrange("p j (h c) -> p h c j",
                                                   h=nheads)[:, :, 0, :],
                                axis=AX.X, op=ALU.add)
                            nc.vector.tensor_tensor(out=pext, in0=Gt[:, :, 0:dfeat],
                                                    in1=pext, op=ALU.mult)
                            nc.vector.tensor_reduce(
                                out=res[:, 0:dfeat],
                                in_=pext.rearrange("p j f -> p f j"),
                                axis=AX.X, op=ALU.add)
                            nc.sync.dma_start(
                                part[s][t * 128:(t + 1) * 128, 0:dfeat + nheads],
                                res[:, 0:dfeat + nheads])

            def combine(layer, part, pcols, dfeat, nheads, store):
                with tc.tile_pool(name=f"cba{layer}", bufs=1) as cba, \
                     tc.tile_pool(name=f"comb{layer}", bufs=2) as cb:
                    pg = []
                    for s in range(2):
                        ci = cba.tile([128, NRP // 16], DT.int16, tag=f"ci{s}")
                        nc.sync.dma_start(
                            ci[:], cidx_h[:, s * NRP // 16:(s + 1) * NRP // 16])
                        g = cba.tile([128, ntiles, pcols], DT.float32, tag=f"g{s}")
                        nc.gpsimd.dma_gather(g[:], part[s][:], ci[:], NRP, NRP,
                                             pcols, single_packet=False)
                        pg.append(g)
                    for t in range(ntiles):
                        rows = 128 if t < ntiles - 1 else tail
                        comb = cb.tile([128, dfeat + nheads], DT.float32, tag="comb")
                        nc.vector.tensor_tensor(
                            out=comb[:], in0=pg[0][:, t, 0:dfeat + nheads],
                            in1=pg[1][:, t, 0:dfeat + nheads], op=ALU.add)
                        rec = cb.tile([128, nheads], DT.float32, tag="rec")
                        nc.vector.tensor_scalar_add(rec[:], comb[:, dfeat:], EPS)
                        nc.vector.reciprocal(rec[:], rec[:])
                        o1 = cb.tile([128, dfeat], DT.float32, tag="o1")
                        nc.vector.tensor_tensor(
                            out=o1[:].rearrange("p (h c) -> p h c", h=nheads),
                            in0=comb[:, 0:dfeat].rearrange("p (h c) -> p h c",
                                                           h=nheads),
                            in1=rec[:].rearrange("p (h c) -> p h c", c=1)
                                      .to_broadcast([128, nheads, dfeat // nheads]),
                            op=ALU.mult)
                        store(t, rows, o1, cb)

            # ================= layer 1 =================
            if stop_after < 1:
                raise _StopBuild()
            def adst1_src(apl, s):
                a = apl.tile([128, ntiles, H], DT.float32)
                nc.sync.dma_start(
                    a[:],
                    adst1p_h[s * NRP:(s + 1) * NRP, :]
                    .rearrange("(t p) h -> p t h", p=128))
                return a

            for s in range(2):
                run_pass(1, s, t1, cfg.t1_cols, DT.float32, part1, d1, H, adst1_src)

            if stop_after < 2:
                raise _StopBuild()
            with tc.tile_pool(name="hps", bufs=4, space="PSUM") as hps:
                def store1(t, rows, o1, cb):
                    hf = cb.tile([128, d1], DT.float32, tag="hf")
                    nc.vector.tensor_tensor(out=hf[:], in0=o1[:], in1=b1s[:], op=ALU.add)
                    # elu(h) = max(h,0) + exp(min(h,0)) - 1
                    r = cb.tile([128, d1], DT.float32, tag="r")
                    nc.vector.tensor_scalar_max(r[:], hf[:], 0.0)
                    nc.vector.tensor_scalar_min(hf[:], hf[:], 0.0)
                    e = cb.tile([128, d1], DT.float32, tag="e")
                    nc.scalar.activation(out=e[:], in_=hf[:], func=AF.Exp)
                    nc.vector.tensor_tensor(out=r[:], in0=r[:], in1=e[:], op=ALU.add)
                    nc.vector.tensor_scalar_add(r[:], r[:], -1.0)
                    ps = hps.tile([d1, 128], DT.float32, tag="tp", space="PSUM")
                    nc.tensor.transpose(out=ps[:, :], in_=r[:, :], identity=ident[:])
                    htb = cb.tile([d1, 128], DT.bfloat16, tag="htb")
                    nc.any.tensor_copy(out=htb[:], in_=ps[:])
                    nc.sync.dma_start(hT_shard[:, t * 128:t * 128 + rows], htb[:, :rows])
                    # a_dst2 for own nodes: h_tile @ w2aug[:, C2+1]
                    ps2 = hps.tile([128, 1], DT.float32, tag="a2p", space="PSUM")
                    nc.tensor.matmul(ps2[:], lhsT=htb[:], rhs=w2s[:, C2 + 1:C2 + 2],
                                     start=True, stop=True)
                    nc.any.tensor_copy(out=adst2nat[:, t:t + 1], in_=ps2[:])

                combine(1, part1, P1C, d1, H, store1)

            # stage a_dst2 to HBM scratch (natural order: row t*128+d <- [d, t])
            nc.sync.dma_start(
                adst2sc[:, 0:1].rearrange("(t p) c -> p (t c)", p=128),
                adst2nat[:])

            if stop_after < 3:
                raise _StopBuild()
            # ---- AllGather h^T ----
            nc.gpsimd.collective_compute(
                "AllGather", ALU.bypass, ins=[hT_shard[:]], outs=[hT_full[:]],
                replica_groups=[list(range(cfg.ncores))])

            if stop_after < 4:
                raise _StopBuild()
            # ================= P3: layer-2 table =================
            with tc.tile_pool(name="p3", bufs=2) as p3, \
                 tc.tile_pool(name="p3ps", bufs=4, space="PSUM") as p3ps:
                sc2 = p3.tile([1, cfg.t2_cols], DT.bfloat16, tag="sent2")
                nc.sync.dma_start(sc2[:], sent2_h[:])
                for s in range(2):
                    nc.sync.dma_start(t2[s][half:half + 1, :], sc2[:])
                for sh in range(cfg.ncores):
                    hts = p3.tile([d1, nloc], DT.bfloat16, tag="hts")
                    nc.sync.dma_start(hts[:], hT_full[sh * d1:(sh + 1) * d1, :])
                    for k in range(ntiles):
                        n0 = k * 128
                        cnt = min(128, nloc - n0)
                        gbase = sh * nloc + n0
                        s = gbase // half
                        ps = p3ps.tile([128, C2 + 2], DT.float32, tag="ps2", space="PSUM")
                        nc.tensor.matmul(ps[:cnt, :], lhsT=hts[:, n0:n0 + cnt],
                                         rhs=w2s[:], start=True, stop=True)
                        row = p3.tile([128, C2 + 2], DT.bfloat16, tag="row2")
                        nc.any.tensor_copy(out=row[:cnt, :], in_=ps[:cnt, :])
                        nc.sync.dma_start(
                            t2[s][gbase - s * half:gbase - s * half + cnt, 0:C2 + 2],
                            row[:cnt, :])

            if stop_after < 5:
                raise _StopBuild()
            # ================= layer 2 =================
            def adst2_src(apl, s):
                ai = apl.tile([128, NRP // 16], DT.int16)
                nc.sync.dma_start(ai[:], aidx_h[:, s * NRP // 16:(s + 1) * NRP // 16])
                g = apl.tile([128, ntiles, 64], DT.float32)
                nc.gpsimd.dma_gather(g[:], adst2sc[:], ai[:], NRP, NRP, 64,
                                     single_packet=False)
                gb = apl.tile([128, ntiles, 1], DT.bfloat16)
                nc.vector.tensor_copy(out=gb[:], in_=g[:, :, 0:1])
                return gb

            for s in range(2):
                run_pass(2, s, t2, cfg.t2_cols, DT.bfloat16, part2, C2, 1, adst2_src)

            if stop_after < 6:
                raise _StopBuild()
            def store2(t, rows, o1, cb):
                o2 = cb.tile([128, C2], DT.float32, tag="o2")
                nc.vector.tensor_tensor(out=o2[:], in0=o1[:], in1=b2s[:], op=ALU.add)
                nc.sync.dma_start(out_h[t * 128:t * 128 + rows, :], o2[:rows, :])

            combine(2, part2, P2C, C2, 1, store2)

    except _StopBuild:
        pass
    nc.compile()
    return nc


# ----------------------------------------------------------------------------
# Host entry
# ----------------------------------------------------------------------------
def host_inputs(cfg, x, edge_index, W1, att_src1, att_dst1, bias1, W2,
                att_src2, att_dst2, bias2):
    import ml_dtypes
    H, C1, C2, d1 = cfg.H, cfg.C1, cfg.C2, cfg.d1
    x = np.asarray(x, np.float32)
    ei = np.asarray(edge_index).astype(np.int64)
    loops = np.arange(cfg.N, dtype=np.int64)
    src = np.concatenate([ei[0], loops])
    dst = np.concatenate([ei[1], loops])

    W1 = np.asarray(W1, np.float32)
    A_src = np.zeros((d1, H), np.float32)
    A_dst = np.zeros((d1, H), np.float32)
    for h in range(H):
        A_src[h * C1:(h + 1) * C1, h] = np.asarray(att_src1, np.float32)[h]
        A_dst[h * C1:(h + 1) * C1, h] = np.asarray(att_dst1, np.float32)[h]
    w1aug = np.concatenate([W1, W1 @ A_src, W1 @ A_dst], axis=1)
    W2 = np.asarray(W2, np.float32)
    w2aug = np.concatenate(
        [W2, W2 @ np.asarray(att_src2, np.float32).T,
         W2 @ np.asarray(att_dst2, np.float32).T], axis=1).astype(ml_dtypes.bfloat16)

    adst1_full = x @ (W1 @ A_dst)            # [N, H] exact same math as device

    sent1 = np.zeros((1, cfg.t1_cols), np.float32)
    sent1[0, d1:d1 + 2 * H] = SENT_VAL
    sent2 = np.zeros((1, cfg.t2_cols), np.float32)
    sent2[0, C2:C2 + 2] = SENT_VAL
    sent2 = sent2.astype(ml_dtypes.bfloat16)

    common = {
        "xT": np.ascontiguousarray(x.T),
        "w1aug": w1aug,
        "w2aug": w2aug,
        "bias1r": np.tile(np.asarray(bias1, np.float32)[None, :], (128, 1)),
        "bias2r": np.tile(np.asarray(bias2, np.float32)[None, :], (128, 1)),
        "sent1": sent1, "sent2": sent2,
    }
    in_maps = []
    for c in range(cfg.ncores):
        r = _route_core(cfg, src, dst, c, adst1_full)
        in_maps.append({**common, "gidx": r["gidx"], "aidx": r["aidx"],
                        "cidx": r["cidx"], "adst1p": r["adst1p"]})
    return in_maps


_CACHE = {}


def kernel(x, edge_index, W1, att_src1, att_dst1, bias1, W2, att_src2,
           att_dst2, bias2):
    x = np.asarray(x, dtype=np.float32)
    N, F = x.shape
    cfg = Cfg(N=N, F=F, E=edge_index.shape[1])
    key = (N, F, cfg.E)
    if key not in _CACHE:
        _CACHE[key] = build_program(cfg)
    nc = _CACHE[key]
    in_maps = host_inputs(cfg, x, edge_index, W1, att_src1, att_dst1, bias1,
                          W2, att_src2, att_dst2, bias2)
    res = run_bass_kernel_spmd(nc, in_maps, list(range(cfg.ncores)))
    return np.concatenate(
        [res.results[c]["out"] for c in range(cfg.ncores)], axis=0
    ).astype(np.float32)



# revision 8
# speedup vs baseline: 1.2128x; 1.2128x over previous
"""2-layer GAT kernel for Trainium2 (8 NeuronCores), Bass/Tile.  v3.

Sharding: nodes by dst across 8 cores; edges routed to the dst owner.
Per core, edges split into two passes by src half (dma_gather idx is int16
-> gather tables limited to <=32768 rows).  Per pass, dst nodes are sorted
by per-pass degree and packed into 128-partition tiles with DATA-DEPENDENT
exact slot budgets D[s][t] (max over cores; program compiled per budget
vector).  Edge j of dst node d sits at (partition rank%128, tile rank//128,
slot j).  Pad slots point at a sentinel table row (a_src = -1e4 => p = 0).

Both layers share the SAME slot grids (same edges, same orders):
  gidx1[slot] = src id within its half (layer-1 table row)
  gidx2[slot] = global pass-order position of src (layer-2 table row)

Layer tables:
  t1[s] = [x @ W1 | x @ W1 @ Asrc] rows (f32, 512B), built on-device from
    host-pre-transposed x; a_dst1 comes host-computed+permuted (adst1p).
  t2[s] = [g | g@(W2 a_src2^T) | g@(W2 a_dst2^T)] rows (f32, 512B) where
    g = elu(out1 + b1).  h^T (bf16, pass-1-order) is AllGathered in two
    column chunks; t2 rows are one matmul per tile vs [I | w2a] rhs.
    Layer-2 aggregates 64-wide g; W2 is applied after normalization
    (out2 = (gagg/den) @ W2 + b2), valid because W2 is linear.

Per pass: pass 0 writes per-tile partial rows [num|den] to HBM; pass 1
reduces directly into an SBUF accumulator and immediately combines each
tile: gather the pass-0 partial rows for the tile's nodes (cross-rank
permutation), add, normalize.  Layer-1 combine also emits h^T columns and
a_dst2; layer-2 combine applies W2+bias and stores output rows in pass-1
order; the host un-permutes rows at the end.
"""

import numpy as np


class _StopBuild(Exception):
    pass


import concourse.bacc as bacc
import concourse.bass as bass
import concourse.mybir as mybir
import concourse.tile as tile
from concourse._compat import cdiv
from concourse.bass_utils import run_bass_kernel_spmd

AF = mybir.ActivationFunctionType
ALU = mybir.AluOpType
AX = mybir.AxisListType
DT = mybir.dt

NEG_SLOPE = 0.2
EPS = 1e-16
SENT_VAL = -1e4
GC_TARGET = 64


def _wrap_idx(idx):
    """[n] -> [128, n/16] int16: position j -> (partition j%16, col j//16),
    replicated across the 8 groups of 16 partitions."""
    idx = np.asarray(idx, dtype=np.int16)
    assert len(idx) % 16 == 0
    return np.tile(idx.reshape(-1, 16).T, (8, 1))


# ----------------------------------------------------------------------------
# Configuration + host routing (data-dependent)
# ----------------------------------------------------------------------------
class Cfg:
    def __init__(self, N, F, E, src, dst):
        ncores = 8
        self.N, self.F, self.E, self.ncores = N, F, E, ncores
        self.H, self.C1, self.C2 = 8, 8, 128
        self.d1 = 64
        self.nloc = N // ncores          # 6250
        self.half = N // 2               # 25000
        self.ntiles = cdiv(self.nloc, 128)   # 49
        self.nrp = self.ntiles * 128         # 6272
        nloc, half, ntiles = self.nloc, self.half, self.ntiles

        # ---- per-core routing part 1: degrees / orders ----
        self.cores = []
        for c in range(ncores):
            base = c * nloc
            m = (dst >= base) & (dst < base + nloc)
            s_c = src[m]
            d_c = (dst[m] - base).astype(np.int64)
            info = {"s": s_c, "d": d_c, "deg": [], "order": [], "rank": []}
            for s in (0, 1):
                m2 = (s_c // half) == s
                deg = np.bincount(d_c[m2], minlength=nloc)
                order = np.argsort(-deg, kind="stable")
                rank = np.empty(nloc, dtype=np.int64)
                rank[order] = np.arange(nloc)
                info["deg"].append(deg)
                info["order"].append(order)
                info["rank"].append(rank)
            self.cores.append(info)

        # global pass-1-order position of every node (for gidx2 / hT layout)
        self.rank1_global = np.empty(N, dtype=np.int64)
        for c in range(ncores):
            self.rank1_global[c * nloc:(c + 1) * nloc] = self.cores[c]["rank"][1]

        # ---- shared exact budgets D[s][t] = max over cores of tile max ----
        self.D = []
        for s in (0, 1):
            mx = np.zeros(ntiles, dtype=np.int64)
            for c in range(ncores):
                sd = np.sort(self.cores[c]["deg"][s])[::-1]
                pad = np.zeros(ntiles * 128, dtype=np.int64)
                pad[:nloc] = sd
                mx = np.maximum(mx, pad.reshape(ntiles, 128).max(axis=1))
            self.D.append(np.maximum(mx, 1))
        self.col_off = [np.concatenate([[0], np.cumsum(D)]).astype(int)
                        for D in self.D]
        self.total_cols = [int(D.sum()) for D in self.D]

        # ---- group packing (cap GC_TARGET cols per gather) ----
        self.groups = []
        for s in (0, 1):
            gs, t = [], 0
            while t < ntiles:
                t0, c0 = t, int(self.col_off[s][t])
                cols = 0
                while t < ntiles and (cols == 0
                                      or cols + self.D[s][t] <= GC_TARGET):
                    cols += int(self.D[s][t])
                    t += 1
                gs.append((t0, t, c0, cols))
            self.groups.append(gs)
        self.GC = max(g[3] for gs in self.groups for g in gs)


def build_routing(cfg, core):
    """Per-core runtime index arrays."""
    nloc, half, ntiles, nrp = cfg.nloc, cfg.half, cfg.ntiles, cfg.nrp
    info = cfg.cores[core]
    s_c, d_c = info["s"], info["d"]
    g1, g2, adst1p = [], [], []
    for s in (0, 1):
        m2 = (s_c // half) == s
        ss = s_c[m2]                       # global src ids
        dd = d_c[m2]
        deg = info["deg"][s]
        rank = info["rank"][s]
        eo = np.lexsort((ss, dd))
        ss_o, dd_o = ss[eo], dd[eo]
        starts = np.concatenate([[0], np.cumsum(deg)])
        j = np.arange(len(dd_o)) - starts[dd_o]
        r = rank[dd_o]
        tile_e, row_e = r // 128, r % 128
        Dv = cfg.D[s]
        assert (j < Dv[tile_e]).all(), "slot budget overflow (exact budgets)"
        flat1 = np.full(cfg.total_cols[s] * 128, half, dtype=np.int64)
        flat1[(cfg.col_off[s][tile_e] + j) * 128 + row_e] = ss_o - s * half
        g1.append(flat1)
        # layer-2 positions: owner-core pass-1 rank, table offset by half
        pos = (ss_o // nloc - 4 * s) * nloc + cfg.rank1_global[ss_o]
        flat2 = np.full(cfg.total_cols[s] * 128, half, dtype=np.int64)
        flat2[(cfg.col_off[s][tile_e] + j) * 128 + row_e] = pos
        g2.append(flat2)
        a = np.full((nrp, cfg.H), SENT_VAL, dtype=np.float32)
        a[:nloc] = cfg.adst1_full[core * nloc + info["order"][s]]
        adst1p.append(a)
    # cross: pass-1-order position j -> pass-0 partial row
    cross = np.zeros(nrp, dtype=np.int64)
    cross[:nloc] = info["rank"][0][info["order"][1]]
    # adst2 A-order: pass-0 position i -> pass-1 staged row
    a2ai = np.zeros(nrp, dtype=np.int64)
    a2ai[:nloc] = info["rank"][1][info["order"][0]]
    return {
        "gidx1": _wrap_idx(np.concatenate(g1)),
        "gidx2": _wrap_idx(np.concatenate(g2)),
        "adst1p": np.concatenate(adst1p, axis=0),
        "cross1": _wrap_idx(cross),
        "a2ai": _wrap_idx(a2ai),
    }


# ----------------------------------------------------------------------------
# Device program
# ----------------------------------------------------------------------------
def build_program(cfg, stop_after=99):
    from concourse.masks import make_identity

    nc = bacc.Bacc(None, target_bir_lowering=False, debug=True)
    H, d1, C2, F = cfg.H, cfg.d1, cfg.C2, cfg.F
    nloc, half, ntiles = cfg.nloc, cfg.half, cfg.ntiles
    nfull_tiles = cdiv(cfg.N, 128)
    NRP, GC = cfg.nrp, cfg.GC
    tail = nloc - (ntiles - 1) * 128
    ca_tiles = 25                       # hT AllGather chunk A: tiles 0..24
    ca_cols = ca_tiles * 128            # 3200
    cb_cols = nloc - ca_cols            # 3050

    # ---- external IO ----
    xT = nc.dram_tensor("xT", [F, cfg.N], DT.float32, kind="ExternalInput")
    w1aug_h = nc.dram_tensor("w1aug", [F, 128], DT.float32, kind="ExternalInput")
    tgr_h = nc.dram_tensor("tgr", [d1, 128], DT.bfloat16, kind="ExternalInput")
    w2t_h = nc.dram_tensor("w2t", [d1, C2], DT.bfloat16, kind="ExternalInput")
    b1_h = nc.dram_tensor("bias1r", [128, d1], DT.float32, kind="ExternalInput")
    b2_h = nc.dram_tensor("bias2r", [128, C2], DT.float32, kind="ExternalInput")
    sent1_h = nc.dram_tensor("sent1", [1, 128], DT.float32, kind="ExternalInput")
    sent2_h = nc.dram_tensor("sent2", [1, 128], DT.float32, kind="ExternalInput")
    tc01 = cfg.total_cols[0] + cfg.total_cols[1]
    gidx1_h = nc.dram_tensor("gidx1", [128, tc01 * 8], DT.int16, kind="ExternalInput")
    gidx2_h = nc.dram_tensor("gidx2", [128, tc01 * 8], DT.int16, kind="ExternalInput")
    adst1p_h = nc.dram_tensor("adst1p", [2 * NRP, H], DT.float32, kind="ExternalInput")
    cross1_h = nc.dram_tensor("cross1", [128, NRP // 16], DT.int16, kind="ExternalInput")
    a2ai_h = nc.dram_tensor("a2ai", [128, NRP // 16], DT.int16, kind="ExternalInput")
    out_h = nc.dram_tensor("out", [NRP, C2], DT.float32, kind="ExternalOutput")

    # ---- internal DRAM ----
    t1 = [nc.dram_tensor(f"t1_{s}", [half + 1, 128], DT.float32) for s in range(2)]
    t2 = [nc.dram_tensor(f"t2_{s}", [half + 1, 128], DT.float32) for s in range(2)]
    part1 = nc.dram_tensor("part1", [NRP, 128], DT.float32)
    part2 = nc.dram_tensor("part2", [NRP, 128], DT.float32)
    adst2sc = nc.dram_tensor("adst2sc", [NRP, 64], DT.float32)
    hT_shA = nc.dram_tensor("hT_shA", [d1, ca_cols], DT.bfloat16)
    hT_shB = nc.dram_tensor("hT_shB", [d1, cb_cols], DT.bfloat16)
    hT_fullA = nc.dram_tensor("hT_fullA", [cfg.ncores * d1, ca_cols], DT.bfloat16)
    hT_fullB = nc.dram_tensor("hT_fullB", [cfg.ncores * d1, cb_cols], DT.bfloat16)

    try:
      with tile.TileContext(nc) as tc:
        with tc.tile_pool(name="const", bufs=1) as cpool, \
             tc.tile_pool(name="p0", bufs=4) as p0, \
             tc.tile_pool(name="p0ps", bufs=2, space="PSUM") as p0ps, \
             tc.tile_pool(name="pass", bufs=2) as pp, \
             tc.tile_pool(name="gi", bufs=3) as gip, \
             tc.tile_pool(name="cmb", bufs=2) as cb, \
             tc.tile_pool(name="cps", bufs=1, space="PSUM") as cps, \
             tc.tile_pool(name="p3", bufs=4) as p3, \
             tc.tile_pool(name="p3ps", bufs=2, space="PSUM") as p3ps:
            w1s = cpool.tile([F, 128], DT.float32)
            nc.sync.dma_start(w1s[:], w1aug_h[:])
            tgr = cpool.tile([d1, 128], DT.bfloat16)
            nc.sync.dma_start(tgr[:], tgr_h[:])
            w2t = cpool.tile([d1, C2], DT.bfloat16)
            nc.sync.dma_start(w2t[:], w2t_h[:])
            b1s = cpool.tile([128, d1], DT.float32)
            nc.sync.dma_start(b1s[:], b1_h[:])
            b2s = cpool.tile([128, C2], DT.float32)
            nc.sync.dma_start(b2s[:], b2_h[:])
            ident = cpool.tile([128, 128], DT.float32)
            make_identity(nc, ident[:])
            cr1 = cpool.tile([128, NRP // 16], DT.int16)
            nc.sync.dma_start(cr1[:], cross1_h[:])
            a2ai = cpool.tile([128, NRP // 16], DT.int16)
            nc.sync.dma_start(a2ai[:], a2ai_h[:])
            adst1 = []
            for s in range(2):
                a = cpool.tile([128, ntiles, H], DT.float32, tag=f"adst1_{s}")
                nc.sync.dma_start(
                    a[:],
                    adst1p_h[s * NRP:(s + 1) * NRP, :]
                    .rearrange("(t p) h -> p t h", p=128))
                adst1.append(a)
            accB1 = cpool.tile([128, ntiles, 72], DT.float32)
            accB2 = cpool.tile([128, ntiles, 65], DT.float32)
            adst2B = cpool.tile([128, ntiles, 1], DT.float32)
            adst2A = cpool.tile([128, ntiles, 1], DT.float32)

            # sentinel rows
            sc1 = cpool.tile([1, 128], DT.float32, tag="sent1")
            nc.sync.dma_start(sc1[:], sent1_h[:])
            sc2 = cpool.tile([1, 128], DT.float32, tag="sent2")
            nc.sync.dma_start(sc2[:], sent2_h[:])
            for s in range(2):
                nc.sync.dma_start(t1[s][half:half + 1, :], sc1[:])
                nc.sync.dma_start(t2[s][half:half + 1, :], sc2[:])

            # ================= P0: layer-1 table =================
            if stop_after < 1:
                raise _StopBuild()
            for k in range(nfull_tiles):
                n0 = k * 128
                cnt = min(128, cfg.N - n0)
                xt_t = p0.tile([F, 128], DT.float32, tag="xt")
                nc.sync.dma_start(xt_t[:, :cnt], xT[:, n0:n0 + cnt])
                ps = p0ps.tile([128, 128], DT.float32, tag="ps", space="PSUM")
                nc.tensor.matmul(ps[:cnt, :], lhsT=xt_t[:, :cnt], rhs=w1s[:],
                                 start=True, stop=True)
                row = p0.tile([128, 128], DT.float32, tag="row")
                nc.any.tensor_copy(out=row[:cnt, :], in_=ps[:cnt, :])
                for s in range(2):
                    lo, hi = max(n0, s * half), min(n0 + cnt, (s + 1) * half)
                    if lo < hi:
                        nc.sync.dma_start(
                            t1[s][lo - s * half:hi - s * half, :],
                            row[lo - n0:hi - n0, :])

            # ================= pass machinery =================
            def grid_pass(layer, s, tbl, gidx_h, adst_tile, nheads, dfeat,
                          accum, part, per_tile_post):
                """One slot-grid pass.  accum=None: write partial rows to
                `part` (pass 0).  accum=tile: reduce into SBUF (pass 1) and
                call per_tile_post(t, rows) after each tile."""
                base8 = (cfg.total_cols[0] if s == 1 else 0) * 8
                for (t0, t1_, c0, ncols) in cfg.groups[s]:
                    gi = gip.tile([128, GC * 8], DT.int16, tag="gi")
                    nc.sync.dma_start(
                        gi[:, :ncols * 8],
                        gidx_h[:, base8 + c0 * 8: base8 + (c0 + ncols) * 8])
                    G = pp.tile([128, GC, 128], DT.float32, tag="G")
                    nc.gpsimd.dma_gather(G[:, :ncols, :], tbl[:],
                                         gi[:, :ncols * 8], ncols * 128,
                                         ncols * 128, 128, single_packet=False)
                    pex = pp.tile([128, GC, 64], DT.float32, tag="px")
                    for t in range(t0, t1_):
                        D = int(cfg.D[s][t])
                        o = int(cfg.col_off[s][t]) - c0
                        Gt = G[:, o:o + D, :]
                        asrc = Gt[:, :, 64:64 + nheads]
                        al = pp.tile([128, GC, 8], DT.float32, tag="al")
                        alt = al[:, :D, :nheads]
                        nc.vector.tensor_tensor(
                            out=alt, in0=asrc,
                            in1=adst_tile[:, t:t + 1, :]
                                .to_broadcast([128, D, nheads]),
                            op=ALU.add)
                        nc.vector.scalar_tensor_tensor(
                            out=alt, in0=alt, scalar=NEG_SLOPE, in1=alt,
                            op0=ALU.mult, op1=ALU.max)
                        pext = pex[:, o:o + D, :]
                        nc.scalar.activation(
                            out=pext,
                            in_=alt.rearrange("p j (h c) -> p j h c", c=1)
                                   .to_broadcast([128, D, nheads,
                                                  64 // nheads]),
                            func=AF.Exp)
                        if accum is None:
                            res = pp.tile([128, 128], DT.float32, tag="res")
                            nc.any.memset(res[:, 64 + nheads:], 0.0)
                            dn = res[:, 64:64 + nheads]
                            nm = res[:, 0:64]
                        else:
                            dn = accum[:, t, 64:64 + nheads]
                            nm = accum[:, t, 0:64]
                        nc.vector.tensor_reduce(
                            out=dn,
                            in_=pext.rearrange("p j (h c) -> p h c j",
                                               h=nheads)[:, :, 0, :],
                            axis=AX.X, op=ALU.add)
                        nc.vector.tensor_tensor(out=pext, in0=Gt[:, :, 0:64],
                                                in1=pext, op=ALU.mult)
                        nc.vector.tensor_reduce(
                            out=nm, in_=pext.rearrange("p j f -> p f j"),
                            axis=AX.X, op=ALU.add)
                        if accum is None:
                            nc.sync.dma_start(
                                part[t * 128:(t + 1) * 128, :], res[:])
                        else:
                            per_tile_post(t, 128 if t < ntiles - 1 else tail)

            # ================= layer 1 =================
            if stop_after < 2:
                raise _StopBuild()
            grid_pass(1, 0, t1[0], gidx1_h, adst1[0], H, 64, None, part1, None)

            if stop_after < 3:
                raise _StopBuild()

            def post1(t, rows):
                cg = cb.tile([128, 1, 128], DT.float32, tag="cg1")
                nc.gpsimd.dma_gather(cg[:], part1[:],
                                     cr1[:, t * 8:(t + 1) * 8], 128, 128, 128,
                                     single_packet=False)
                comb = cb.tile([128, 72], DT.float32, tag="comb1")
                nc.vector.tensor_tensor(out=comb[:], in0=cg[:, 0, 0:72],
                                        in1=accB1[:, t, :], op=ALU.add)
                rec = cb.tile([128, H], DT.float32, tag="rec1")
                nc.vector.tensor_scalar_add(rec[:], comb[:, 64:72], EPS)
                nc.vector.reciprocal(rec[:], rec[:])
                hf = cb.tile([128, 64], DT.float32, tag="hf")
                nc.vector.tensor_tensor(
                    out=hf[:].rearrange("p (h c) -> p h c", h=H),
                    in0=comb[:, 0:64].rearrange("p (h c) -> p h c", h=H),
                    in1=rec[:].rearrange("p (h c) -> p h c", c=1)
                              .to_broadcast([128, H, 8]),
                    op=ALU.mult)
                nc.vector.tensor_tensor(out=hf[:], in0=hf[:], in1=b1s[:],
                                        op=ALU.add)
                # elu
                r = cb.tile([128, 64], DT.float32, tag="r")
                nc.vector.tensor_scalar_max(r[:], hf[:], 0.0)
                nc.vector.tensor_scalar_min(hf[:], hf[:], 0.0)
                e = cb.tile([128, 64], DT.float32, tag="e")
                nc.scalar.activation(out=e[:], in_=hf[:], func=AF.Exp)
                nc.vector.tensor_tensor(out=r[:], in0=r[:], in1=e[:], op=ALU.add)
                nc.vector.tensor_scalar_add(r[:], r[:], -1.0)
                psT = cps.tile([64, 128], DT.float32, tag="psT", space="PSUM")
                nc.tensor.transpose(out=psT[:], in_=r[:, :], identity=ident[:])
                htb = cb.tile([64, 128], DT.bfloat16, tag="htb")
                nc.any.tensor_copy(out=htb[:], in_=psT[:])
                if t < ca_tiles:
                    nc.sync.dma_start(hT_shA[:, t * 128:t * 128 + rows],
                                      htb[:, :rows])
                else:
                    o = (t - ca_tiles) * 128
                    nc.sync.dma_start(hT_shB[:, o:o + rows], htb[:, :rows])
                psA = cps.tile([128, 1], DT.float32, tag="psA", space="PSUM")
                nc.tensor.matmul(psA[:], lhsT=htb[:], rhs=tgr[:, 65:66],
                                 start=True, stop=True)
                nc.any.tensor_copy(out=adst2B[:, t, :], in_=psA[:])
                adrow = cb.tile([128, 64], DT.float32, tag="adrow")
                nc.any.tensor_copy(out=adrow[:],
                                   in_=psA[:, 0:1].to_broadcast([128, 64]))
                nc.sync.dma_start(adst2sc[t * 128:(t + 1) * 128, :], adrow[:])
                if t == ca_tiles - 1:
                    nc.gpsimd.collective_compute(
                        "AllGather", ALU.bypass, ins=[hT_shA[:]],
                        outs=[hT_fullA[:]],
                        replica_groups=[list(range(cfg.ncores))])

            grid_pass(1, 1, t1[1], gidx1_h, adst1[1], H, 64, accB1, None, post1)

            nc.gpsimd.collective_compute(
                "AllGather", ALU.bypass, ins=[hT_shB[:]], outs=[hT_fullB[:]],
                replica_groups=[list(range(cfg.ncores))])

            # adst2 in pass-0 order (gpsimd idle during AllGather B)
            ga = cpool.tile([128, ntiles, 64], DT.float32, tag="ga")
            nc.gpsimd.dma_gather(ga[:], adst2sc[:], a2ai[:], NRP, NRP, 64,
                                 single_packet=False)
            nc.vector.tensor_copy(out=adst2A[:, :, 0], in_=ga[:, :, 0])

            # ================= t2 build =================
            if stop_after < 4:
                raise _StopBuild()

            def t2_tile(s2, kc4, u):
                kc = s2 * 4 + kc4
                cnt = 128 if u < ntiles - 1 else tail
                hts = p3.tile([d1, 128], DT.bfloat16, tag="hts")
                if u < ca_tiles:
                    nc.sync.dma_start(
                        hts[:, :cnt],
                        hT_fullA[kc * d1:(kc + 1) * d1,
                                 u * 128:u * 128 + cnt])
                else:
                    o = (u - ca_tiles) * 128
                    nc.sync.dma_start(
                        hts[:, :cnt],
                        hT_fullB[kc * d1:(kc + 1) * d1, o:o + cnt])
                ps = p3ps.tile([128, 128], DT.float32, tag="ps2", space="PSUM")
                nc.tensor.matmul(ps[:cnt, :], lhsT=hts[:, :cnt], rhs=tgr[:],
                                 start=True, stop=True)
                row = p3.tile([128, 128], DT.float32, tag="row2")
                nc.any.tensor_copy(out=row[:cnt, :], in_=ps[:cnt, :])
                ro = kc4 * nloc + u * 128
                nc.sync.dma_start(t2[s2][ro:ro + cnt, :], row[:cnt, :])

            for u in range(ca_tiles):
                for s2 in range(2):
                    for kc4 in range(4):
                        t2_tile(s2, kc4, u)
            for u in range(ca_tiles, ntiles):
                for s2 in range(2):
                    for kc4 in range(4):
                        t2_tile(s2, kc4, u)

            # ================= layer 2 =================
            if stop_after < 5:
                raise _StopBuild()
            grid_pass(2, 0, t2[0], gidx2_h, adst2A[:], 1, 64,
                      None, part2, None)

            if stop_after < 6:
                raise _StopBuild()

            def post2(t, rows):
                cg = cb.tile([128, 1, 128], DT.float32, tag="cg2")
                nc.gpsimd.dma_gather(cg[:], part2[:],
                                     cr1[:, t * 8:(t + 1) * 8], 128, 128, 128,
                                     single_packet=False)
                comb = cb.tile([128, 65], DT.float32, tag="comb2")
                nc.vector.tensor_tensor(out=comb[:], in0=cg[:, 0, 0:65],
                                        in1=accB2[:, t, :], op=ALU.add)
                rec = cb.tile([128, 1], DT.float32, tag="rec2")
                nc.vector.tensor_scalar_add(rec[:], comb[:, 64:65], EPS)
                nc.vector.reciprocal(rec[:], rec[:])
                gg = cb.tile([128, 64], DT.float32, tag="gg")
                nc.vector.tensor_tensor(
                    out=gg[:], in0=comb[:, 0:64],
                    in1=rec[:].to_broadcast([128, 64]), op=ALU.mult)
                psT = cps.tile([64, 128], DT.float32, tag="psT2", space="PSUM")
                nc.tensor.transpose(out=psT[:], in_=gg[:, :], identity=ident[:])
                gtb = cb.tile([64, 128], DT.bfloat16, tag="gtb")
                nc.any.tensor_copy(out=gtb[:], in_=psT[:])
                ps2 = cps.tile([128, C2], DT.float32, tag="ps22", space="PSUM")
                nc.tensor.matmul(ps2[:], lhsT=gtb[:], rhs=w2t[:],
                                 start=True, stop=True)
                o2 = cb.tile([128, C2], DT.float32, tag="o2")
                nc.vector.tensor_tensor(out=o2[:], in0=ps2[:], in1=b2s[:],
                                        op=ALU.add)
                nc.sync.dma_start(out_h[t * 128:t * 128 + 128, :], o2[:])

            grid_pass(2, 1, t2[1], gidx2_h, adst2B[:], 1, 64,
                      accB2, None, post2)

    except _StopBuild:
        pass
    nc.compile()
    return nc


# ----------------------------------------------------------------------------
# Host entry
# ----------------------------------------------------------------------------
def host_inputs(cfg, x, W1, att_src1, att_dst1, bias1, W2, att_src2,
                att_dst2, bias2):
    import ml_dtypes
    H, C1, C2, d1 = cfg.H, cfg.C1, cfg.C2, cfg.d1
    x = np.asarray(x, np.float32)
    W1 = np.asarray(W1, np.float32)
    A_src = np.zeros((d1, H), np.float32)
    A_dst = np.zeros((d1, H), np.float32)
    for h in range(H):
        A_src[h * C1:(h + 1) * C1, h] = np.asarray(att_src1, np.float32)[h]
        A_dst[h * C1:(h + 1) * C1, h] = np.asarray(att_dst1, np.float32)[h]
    w1aug = np.zeros((cfg.F, 128), np.float32)
    w1aug[:, :d1] = W1
    w1aug[:, d1:d1 + H] = W1 @ A_src
    cfg.adst1_full = x @ (W1 @ A_dst)

    W2 = np.asarray(W2, np.float32)
    tgr = np.zeros((d1, 128), np.float32)
    tgr[:, :d1] = np.eye(d1)
    tgr[:, d1:d1 + 1] = W2 @ np.asarray(att_src2, np.float32).T
    tgr[:, d1 + 1:d1 + 2] = W2 @ np.asarray(att_dst2, np.float32).T

    sent1 = np.zeros((1, 128), np.float32)
    sent1[0, d1:d1 + 2 * H] = SENT_VAL
    sent2 = np.zeros((1, 128), np.float32)
    sent2[0, d1:d1 + 2] = SENT_VAL

    common = {
        "xT": np.ascontiguousarray(x.T),
        "w1aug": w1aug,
        "tgr": tgr.astype(ml_dtypes.bfloat16),
        "w2t": W2.astype(ml_dtypes.bfloat16),
        "bias1r": np.tile(np.asarray(bias1, np.float32)[None, :], (128, 1)),
        "bias2r": np.tile(np.asarray(bias2, np.float32)[None, :], (128, 1)),
        "sent1": sent1, "sent2": sent2,
    }
    in_maps = []
    for c in range(cfg.ncores):
        r = build_routing(cfg, c)
        in_maps.append({**common, **r})
    return in_maps


_CACHE = {}


def _run(x, edge_index, W1, att_src1, att_dst1, bias1, W2, att_src2,
         att_dst2, bias2, **run_kwargs):
    x = np.asarray(x, dtype=np.float32)
    N, F = x.shape
    ei = np.asarray(edge_index).astype(np.int64)
    E = ei.shape[1]
    loops = np.arange(N, dtype=np.int64)
    src = np.concatenate([ei[0], loops])
    dst = np.concatenate([ei[1], loops])
    cfg = Cfg(N, F, E, src, dst)
    key = (N, F, E, tuple(cfg.D[0]), tuple(cfg.D[1]))
    if key not in _CACHE:
        _CACHE[key] = build_program(cfg)
    nc = _CACHE[key]
    in_maps = host_inputs(cfg, x, W1, att_src1, att_dst1, bias1,
                          W2, att_src2, att_dst2, bias2)
    res = run_bass_kernel_spmd(nc, in_maps, list(range(cfg.ncores)),
                               **run_kwargs)
    out = np.empty((N, cfg.C2), dtype=np.float32)
    for c in range(cfg.ncores):
        r = np.asarray(res.results[c]["out"], dtype=np.float32)
        out[c * cfg.nloc + cfg.cores[c]["order"][1]] = r[:cfg.nloc]
    return out, res


def kernel(x, edge_index, W1, att_src1, att_dst1, bias1, W2, att_src2,
           att_dst2, bias2):
    out, _ = _run(x, edge_index, W1, att_src1, att_dst1, bias1, W2,
                  att_src2, att_dst2, bias2)
    return out


# revision 11
# speedup vs baseline: 1.4427x; 1.1895x over previous
"""2-layer GAT kernel for Trainium2 (8 NeuronCores), Bass/Tile.  v3.

Sharding: nodes by dst across 8 cores; edges routed to the dst owner.
Per core, edges split into two passes by src half (dma_gather idx is int16
-> gather tables limited to <=32768 rows).  Per pass, dst nodes are sorted
by per-pass degree and packed into 128-partition tiles with DATA-DEPENDENT
exact slot budgets D[s][t] (max over cores; program compiled per budget
vector).  Edge j of dst node d sits at (partition rank%128, tile rank//128,
slot j).  Pad slots point at a sentinel table row (a_src = -1e4 => p = 0).

Both layers share the SAME slot grids (same edges, same orders):
  gidx1[slot] = src id within its half (layer-1 table row)
  gidx2[slot] = global pass-order position of src (layer-2 table row)

Layer tables:
  t1[s] = [x @ W1 | x @ W1 @ Asrc] rows (f32, 512B), built on-device from
    host-pre-transposed x; a_dst1 comes host-computed+permuted (adst1p).
  t2[s] = [g | g@(W2 a_src2^T) | g@(W2 a_dst2^T)] rows (f32, 512B) where
    g = elu(out1 + b1).  h^T (bf16, pass-1-order) is AllGathered in two
    column chunks; t2 rows are one matmul per tile vs [I | w2a] rhs.
    Layer-2 aggregates 64-wide g; W2 is applied after normalization
    (out2 = (gagg/den) @ W2 + b2), valid because W2 is linear.

Per pass: pass 0 writes per-tile partial rows [num|den] to HBM; pass 1
reduces directly into an SBUF accumulator and immediately combines each
tile: gather the pass-0 partial rows for the tile's nodes (cross-rank
permutation), add, normalize.  Layer-1 combine also emits h^T columns and
a_dst2; layer-2 combine applies W2+bias and stores output rows in pass-1
order; the host un-permutes rows at the end.
"""

import numpy as np


class _StopBuild(Exception):
    pass


import concourse.bacc as bacc
import concourse.bass as bass
import concourse.mybir as mybir
import concourse.tile as tile
from concourse._compat import cdiv
from concourse.bass_utils import run_bass_kernel_spmd

AF = mybir.ActivationFunctionType
ALU = mybir.AluOpType
AX = mybir.AxisListType
DT = mybir.dt

NEG_SLOPE = 0.2
EPS = 1e-16
SENT_VAL = -1e4
GC_TARGET = 64


def _wrap_idx(idx):
    """[n] -> [128, n/16] int16: position j -> (partition j%16, col j//16),
    replicated across the 8 groups of 16 partitions."""
    idx = np.asarray(idx, dtype=np.int16)
    assert len(idx) % 16 == 0
    return np.tile(idx.reshape(-1, 16).T, (8, 1))


# ----------------------------------------------------------------------------
# Configuration + host routing (data-dependent)
# ----------------------------------------------------------------------------
class Cfg:
    def __init__(self, N, F, E, src, dst):
        ncores = 8
        self.N, self.F, self.E, self.ncores = N, F, E, ncores
        self.H, self.C1, self.C2 = 8, 8, 128
        self.d1 = 64
        self.nloc = N // ncores          # 6250
        self.half = N // 2               # 25000
        self.ntiles = cdiv(self.nloc, 128)   # 49
        self.nrp = self.ntiles * 128         # 6272
        nloc, half, ntiles = self.nloc, self.half, self.ntiles

        # ---- per-core routing part 1: degrees / orders ----
        self.cores = []
        for c in range(ncores):
            base = c * nloc
            m = (dst >= base) & (dst < base + nloc)
            s_c = src[m]
            d_c = (dst[m] - base).astype(np.int64)
            info = {"s": s_c, "d": d_c, "deg": [], "order": [], "rank": []}
            for s in (0, 1):
                m2 = (s_c // half) == s
                deg = np.bincount(d_c[m2], minlength=nloc)
                order = np.argsort(-deg, kind="stable")
                rank = np.empty(nloc, dtype=np.int64)
                rank[order] = np.arange(nloc)
                info["deg"].append(deg)
                info["order"].append(order)
                info["rank"].append(rank)
            self.cores.append(info)

        # global pass-1-order position of every node (for gidx2 / hT layout)
        self.rank1_global = np.empty(N, dtype=np.int64)
        for c in range(ncores):
            self.rank1_global[c * nloc:(c + 1) * nloc] = self.cores[c]["rank"][1]

        # ---- shared exact budgets D[s][t] = max over cores of tile max ----
        self.D = []
        for s in (0, 1):
            mx = np.zeros(ntiles, dtype=np.int64)
            for c in range(ncores):
                sd = np.sort(self.cores[c]["deg"][s])[::-1]
                pad = np.zeros(ntiles * 128, dtype=np.int64)
                pad[:nloc] = sd
                mx = np.maximum(mx, pad.reshape(ntiles, 128).max(axis=1))
            self.D.append(np.maximum(mx, 1))
        self.col_off = [np.concatenate([[0], np.cumsum(D)]).astype(int)
                        for D in self.D]
        self.total_cols = [int(D.sum()) for D in self.D]

        # ---- group packing (cap GC_TARGET cols per gather) ----
        self.groups = []
        for s in (0, 1):
            gs, t = [], 0
            while t < ntiles:
                t0, c0 = t, int(self.col_off[s][t])
                cols = 0
                while t < ntiles and (cols == 0
                                      or cols + self.D[s][t] <= GC_TARGET):
                    cols += int(self.D[s][t])
                    t += 1
                gs.append((t0, t, c0, cols))
            self.groups.append(gs)
        self.GC = max(g[3] for gs in self.groups for g in gs)


def build_routing(cfg, core):
    """Per-core runtime index arrays."""
    nloc, half, ntiles, nrp = cfg.nloc, cfg.half, cfg.ntiles, cfg.nrp
    info = cfg.cores[core]
    s_c, d_c = info["s"], info["d"]
    g1, g2, adst1p = [], [], []
    for s in (0, 1):
        m2 = (s_c // half) == s
        ss = s_c[m2]                       # global src ids
        dd = d_c[m2]
        deg = info["deg"][s]
        rank = info["rank"][s]
        eo = np.lexsort((ss, dd))
        ss_o, dd_o = ss[eo], dd[eo]
        starts = np.concatenate([[0], np.cumsum(deg)])
        j = np.arange(len(dd_o)) - starts[dd_o]
        r = rank[dd_o]
        tile_e, row_e = r // 128, r % 128
        Dv = cfg.D[s]
        assert (j < Dv[tile_e]).all(), "slot budget overflow (exact budgets)"
        flat1 = np.full(cfg.total_cols[s] * 128, half, dtype=np.int64)
        flat1[(cfg.col_off[s][tile_e] + j) * 128 + row_e] = ss_o - s * half
        g1.append(flat1)
        # layer-2 positions: owner-core pass-1 rank, table offset by half
        pos = (ss_o // nloc - 4 * s) * nloc + cfg.rank1_global[ss_o]
        flat2 = np.full(cfg.total_cols[s] * 128, half, dtype=np.int64)
        flat2[(cfg.col_off[s][tile_e] + j) * 128 + row_e] = pos
        g2.append(flat2)
        a = np.full((nrp, cfg.H), SENT_VAL, dtype=np.float32)
        a[:nloc] = cfg.adst1_full[core * nloc + info["order"][s]]
        adst1p.append(a)
    # cross: pass-1-order position j -> pass-0 partial row
    cross = np.zeros(nrp, dtype=np.int64)
    cross[:nloc] = info["rank"][0][info["order"][1]]
    # adst2 A-order: pass-0 position i -> pass-1 staged row
    a2ai = np.zeros(nrp, dtype=np.int64)
    a2ai[:nloc] = info["rank"][1][info["order"][0]]
    return {
        "gidx1": _wrap_idx(np.concatenate(g1)),
        "gidx2": _wrap_idx(np.concatenate(g2)),
        "adst1p": np.concatenate(adst1p, axis=0),
        "cross1": _wrap_idx(cross),
        "a2ai": _wrap_idx(a2ai),
    }


# ----------------------------------------------------------------------------
# Device program
# ----------------------------------------------------------------------------
def build_program(cfg, stop_after=99):
    from concourse.masks import make_identity

    nc = bacc.Bacc(None, target_bir_lowering=False, debug=True)
    H, d1, C2, F = cfg.H, cfg.d1, cfg.C2, cfg.F
    nloc, half, ntiles = cfg.nloc, cfg.half, cfg.ntiles
    nfull_tiles = cdiv(cfg.N, 128)
    NRP, GC = cfg.nrp, cfg.GC
    tail = nloc - (ntiles - 1) * 128
    ca_tiles = 25                       # hT AllGather chunk A: tiles 0..24
    ca_cols = ca_tiles * 128            # 3200
    cb_cols = nloc - ca_cols            # 3050

    # ---- external IO ----
    xT = nc.dram_tensor("xT", [F, cfg.N], DT.float32, kind="ExternalInput")
    w1aug_h = nc.dram_tensor("w1aug", [F, 128], DT.float32, kind="ExternalInput")
    tgr_h = nc.dram_tensor("tgr", [d1, 128], DT.bfloat16, kind="ExternalInput")
    w2t_h = nc.dram_tensor("w2t", [d1, C2], DT.bfloat16, kind="ExternalInput")
    b1_h = nc.dram_tensor("bias1r", [128, d1], DT.float32, kind="ExternalInput")
    b2_h = nc.dram_tensor("bias2r", [128, C2], DT.float32, kind="ExternalInput")
    sent1_h = nc.dram_tensor("sent1", [1, 256], DT.bfloat16, kind="ExternalInput")
    sent2_h = nc.dram_tensor("sent2", [1, 256], DT.bfloat16, kind="ExternalInput")
    tc01 = cfg.total_cols[0] + cfg.total_cols[1]
    gidx1_h = nc.dram_tensor("gidx1", [128, tc01 * 8], DT.int16, kind="ExternalInput")
    gidx2_h = nc.dram_tensor("gidx2", [128, tc01 * 8], DT.int16, kind="ExternalInput")
    adst1p_h = nc.dram_tensor("adst1p", [2 * NRP, H], DT.bfloat16, kind="ExternalInput")
    cross1_h = nc.dram_tensor("cross1", [128, NRP // 16], DT.int16, kind="ExternalInput")
    a2ai_h = nc.dram_tensor("a2ai", [128, NRP // 16], DT.int16, kind="ExternalInput")
    out_h = nc.dram_tensor("out", [NRP, C2], DT.float32, kind="ExternalOutput")

    # ---- internal DRAM ----
    t1 = [nc.dram_tensor(f"t1_{s}", [half + 1, 256], DT.bfloat16) for s in range(2)]
    t2 = [nc.dram_tensor(f"t2_{s}", [half + 1, 256], DT.bfloat16) for s in range(2)]
    part1 = nc.dram_tensor("part1", [NRP, 128], DT.float32)
    part2 = nc.dram_tensor("part2", [NRP, 128], DT.float32)
    adst2sc = nc.dram_tensor("adst2sc", [NRP, 64], DT.float32)
    hT_shA = nc.dram_tensor("hT_shA", [d1, ca_cols], DT.bfloat16)
    hT_shB = nc.dram_tensor("hT_shB", [d1, cb_cols], DT.bfloat16)
    hT_fullA = nc.dram_tensor("hT_fullA", [cfg.ncores * d1, ca_cols], DT.bfloat16)
    hT_fullB = nc.dram_tensor("hT_fullB", [cfg.ncores * d1, cb_cols], DT.bfloat16)

    try:
      with tile.TileContext(nc) as tc:
        with tc.tile_pool(name="const", bufs=1) as cpool, \
             tc.tile_pool(name="p0", bufs=4) as p0, \
             tc.tile_pool(name="p0ps", bufs=2, space="PSUM") as p0ps, \
             tc.tile_pool(name="pass", bufs=3) as pp, \
             tc.tile_pool(name="px", bufs=2) as pxp, \
             tc.tile_pool(name="gi", bufs=4) as gip, \
             tc.tile_pool(name="cmb", bufs=2) as cb, \
             tc.tile_pool(name="cps", bufs=1, space="PSUM") as cps, \
             tc.tile_pool(name="p3", bufs=4) as p3, \
             tc.tile_pool(name="p3ps", bufs=2, space="PSUM") as p3ps:
            w1s = cpool.tile([F, 128], DT.float32)
            nc.sync.dma_start(w1s[:], w1aug_h[:])
            tgr = cpool.tile([d1, 128], DT.bfloat16)
            nc.sync.dma_start(tgr[:], tgr_h[:])
            w2t = cpool.tile([d1, C2], DT.bfloat16)
            nc.sync.dma_start(w2t[:], w2t_h[:])
            b1s = cpool.tile([128, d1], DT.float32)
            nc.sync.dma_start(b1s[:], b1_h[:])
            b2s = cpool.tile([128, C2], DT.float32)
            nc.sync.dma_start(b2s[:], b2_h[:])
            ident = cpool.tile([128, 128], DT.float32)
            make_identity(nc, ident[:])
            cr1 = cpool.tile([128, NRP // 16], DT.int16)
            nc.sync.dma_start(cr1[:], cross1_h[:])
            a2ai = cpool.tile([128, NRP // 16], DT.int16)
            nc.sync.dma_start(a2ai[:], a2ai_h[:])
            adst1 = []
            for s in range(2):
                a = cpool.tile([128, ntiles, H], DT.bfloat16, tag=f"adst1_{s}")
                nc.sync.dma_start(
                    a[:],
                    adst1p_h[s * NRP:(s + 1) * NRP, :]
                    .rearrange("(t p) h -> p t h", p=128))
                adst1.append(a)
            accB1 = cpool.tile([128, ntiles, 72], DT.float32)
            accB2 = cpool.tile([128, ntiles, 65], DT.float32)
            adst2B = cpool.tile([128, ntiles, 1], DT.bfloat16)
            adst2A = cpool.tile([128, ntiles, 1], DT.bfloat16)

            # sentinel rows
            sc1 = cpool.tile([1, 256], DT.bfloat16, tag="sent1")
            nc.sync.dma_start(sc1[:], sent1_h[:])
            sc2 = cpool.tile([1, 256], DT.bfloat16, tag="sent2")
            nc.sync.dma_start(sc2[:], sent2_h[:])
            for s in range(2):
                nc.sync.dma_start(t1[s][half:half + 1, :], sc1[:])
                nc.sync.dma_start(t2[s][half:half + 1, :], sc2[:])

            # ================= P0: layer-1 table =================
            if stop_after < 1:
                raise _StopBuild()

            def p0_tile(k):
                n0 = k * 128
                cnt = min(128, cfg.N - n0)
                xt_t = p0.tile([F, 128], DT.float32, tag="xt")
                nc.scalar.dma_start(xt_t[:, :cnt], xT[:, n0:n0 + cnt])
                ps = p0ps.tile([128, 128], DT.float32, tag="ps", space="PSUM")
                nc.tensor.matmul(ps[:cnt, :], lhsT=xt_t[:, :cnt], rhs=w1s[:],
                                 start=True, stop=True)
                row = p0.tile([128, 72], DT.bfloat16, tag="row")
                nc.vector.tensor_copy(out=row[:cnt, :], in_=ps[:cnt, 0:72])
                for s in range(2):
                    lo, hi = max(n0, s * half), min(n0 + cnt, (s + 1) * half)
                    if lo < hi:
                        nc.scalar.dma_start(
                            t1[s][lo - s * half:hi - s * half, 0:72],
                            row[lo - n0:hi - n0, :])

            half0_tiles = half // 128 + 1          # tiles covering src half 0
            for k in range(half0_tiles):
                p0_tile(k)

            # ================= pass machinery =================
            def grid_pass(layer, s, tbl, gidx_h, adst_tile, nheads, dfeat,
                          accum, part, per_tile_post, pre_group=None):
                """One slot-grid pass.  accum=None: write partial rows to
                `part` (pass 0).  accum=tile: reduce into SBUF (pass 1) and
                call per_tile_post(t, rows) after each tile."""
                base8 = (cfg.total_cols[0] if s == 1 else 0) * 8
                for gidx_i, (t0, t1_, c0, ncols) in enumerate(cfg.groups[s]):
                    if pre_group is not None:
                        pre_group(gidx_i)
                    gi = gip.tile([128, GC * 8], DT.int16, tag="gi")
                    nc.sync.dma_start(
                        gi[:, :ncols * 8],
                        gidx_h[:, base8 + c0 * 8: base8 + (c0 + ncols) * 8])
                    G = pp.tile([128, GC, 256], DT.bfloat16, tag="G")
                    nc.gpsimd.dma_gather(G[:, :ncols, :], tbl[:],
                                         gi[:, :ncols * 8], ncols * 128,
                                         ncols * 128, 256, single_packet=False)
                    pex = pxp.tile([128, GC, 64], DT.bfloat16, tag="px")
                    for t in range(t0, t1_):
                        D = int(cfg.D[s][t])
                        o = int(cfg.col_off[s][t]) - c0
                        Gt = G[:, o:o + D, :]
                        asrc = Gt[:, :, 64:64 + nheads]
                        al = pp.tile([128, GC, 8], DT.bfloat16, tag="al")
                        alt = al[:, :D, :nheads]
                        nc.vector.tensor_tensor(
                            out=alt, in0=asrc,
                            in1=adst_tile[:, t:t + 1, :]
                                .to_broadcast([128, D, nheads]),
                            op=ALU.add)
                        nc.vector.scalar_tensor_tensor(
                            out=alt, in0=alt, scalar=NEG_SLOPE, in1=alt,
                            op0=ALU.mult, op1=ALU.max)
                        pext = pex[:, o:o + D, :]
                        nc.scalar.activation(
                            out=pext,
                            in_=alt.rearrange("p j (h c) -> p j h c", c=1)
                                   .to_broadcast([128, D, nheads,
                                                  64 // nheads]),
                            func=AF.Exp)
                        if accum is None:
                            res = pp.tile([128, 128], DT.float32, tag="res")
                            nc.any.memset(res[:, 64 + nheads:], 0.0)
                            dn = res[:, 64:64 + nheads]
                            nm = res[:, 0:64]
                        else:
                            dn = accum[:, t, 64:64 + nheads]
                            nm = accum[:, t, 0:64]
                        nc.vector.tensor_reduce(
                            out=dn,
                            in_=pext.rearrange("p j (h c) -> p h c j",
                                               h=nheads)[:, :, 0, :],
                            axis=AX.X, op=ALU.add)
                        nc.vector.tensor_tensor(out=pext, in0=Gt[:, :, 0:64],
                                                in1=pext, op=ALU.mult)
                        nc.vector.tensor_reduce(
                            out=nm, in_=pext.rearrange("p j f -> p f j"),
                            axis=AX.X, op=ALU.add)
                        if accum is None:
                            nc.sync.dma_start(
                                part[t * 128:(t + 1) * 128, :], res[:])
                        else:
                            per_tile_post(t, 128 if t < ntiles - 1 else tail)

            # ================= layer 1 =================
            if stop_after < 2:
                raise _StopBuild()
            p0_rest = list(range(half0_tiles, nfull_tiles))
            ngr0 = len(cfg.groups[0])
            per_g = cdiv(len(p0_rest), max(ngr0 - 1, 1))

            def preA(gi_i):
                for k in p0_rest[gi_i * per_g:(gi_i + 1) * per_g]:
                    p0_tile(k)

            grid_pass(1, 0, t1[0], gidx1_h, adst1[0], H, 64, None, part1,
                      None, pre_group=preA)

            if stop_after < 3:
                raise _StopBuild()

            def post1(t, rows):
                cg = cb.tile([128, 1, 128], DT.float32, tag="cg1")
                nc.gpsimd.dma_gather(cg[:], part1[:],
                                     cr1[:, t * 8:(t + 1) * 8], 128, 128, 128,
                                     single_packet=False)
                comb = cb.tile([128, 72], DT.float32, tag="comb1")
                nc.vector.tensor_tensor(out=comb[:], in0=cg[:, 0, 0:72],
                                        in1=accB1[:, t, :], op=ALU.add)
                rec = cb.tile([128, H], DT.float32, tag="rec1")
                nc.vector.tensor_scalar_add(rec[:], comb[:, 64:72], EPS)
                nc.vector.reciprocal(rec[:], rec[:])
                hf = cb.tile([128, 64], DT.float32, tag="hf")
                nc.vector.tensor_tensor(
                    out=hf[:].rearrange("p (h c) -> p h c", h=H),
                    in0=comb[:, 0:64].rearrange("p (h c) -> p h c", h=H),
                    in1=rec[:].rearrange("p (h c) -> p h c", c=1)
                              .to_broadcast([128, H, 8]),
                    op=ALU.mult)
                nc.vector.tensor_tensor(out=hf[:], in0=hf[:], in1=b1s[:],
                                        op=ALU.add)
                # elu
                r = cb.tile([128, 64], DT.float32, tag="r")
                nc.vector.tensor_scalar_max(r[:], hf[:], 0.0)
                nc.vector.tensor_scalar_min(hf[:], hf[:], 0.0)
                e = cb.tile([128, 64], DT.float32, tag="e")
                nc.scalar.activation(out=e[:], in_=hf[:], func=AF.Exp)
                nc.vector.tensor_tensor(out=r[:], in0=r[:], in1=e[:], op=ALU.add)
                nc.vector.tensor_scalar_add(r[:], r[:], -1.0)
                psT = cps.tile([64, 128], DT.float32, tag="psT", space="PSUM")
                nc.tensor.transpose(out=psT[:], in_=r[:, :], identity=ident[:])
                htb = cb.tile([64, 128], DT.bfloat16, tag="htb")
                nc.any.tensor_copy(out=htb[:], in_=psT[:])
                if t < ca_tiles:
                    nc.sync.dma_start(hT_shA[:, t * 128:t * 128 + rows],
                                      htb[:, :rows])
                else:
                    o = (t - ca_tiles) * 128
                    nc.sync.dma_start(hT_shB[:, o:o + rows], htb[:, :rows])
                psA = cps.tile([128, 1], DT.float32, tag="psA", space="PSUM")
                nc.tensor.matmul(psA[:], lhsT=htb[:], rhs=tgr[:, 65:66],
                                 start=True, stop=True)
                nc.any.tensor_copy(out=adst2B[:, t, :], in_=psA[:])
                adrow = cb.tile([128, 64], DT.float32, tag="adrow")
                nc.any.tensor_copy(out=adrow[:],
                                   in_=psA[:, 0:1].to_broadcast([128, 64]))
                nc.sync.dma_start(adst2sc[t * 128:(t + 1) * 128, :], adrow[:])
                if t == ca_tiles - 1:
                    nc.gpsimd.collective_compute(
                        "AllGather", ALU.bypass, ins=[hT_shA[:]],
                        outs=[hT_fullA[:]],
                        replica_groups=[list(range(cfg.ncores))])

            def t2_tile(s2, kc4, u):
                kc = s2 * 4 + kc4
                cnt = 128 if u < ntiles - 1 else tail
                hts = p3.tile([d1, 128], DT.bfloat16, tag="hts")
                if u < ca_tiles:
                    nc.scalar.dma_start(
                        hts[:, :cnt],
                        hT_fullA[kc * d1:(kc + 1) * d1,
                                 u * 128:u * 128 + cnt])
                else:
                    o = (u - ca_tiles) * 128
                    nc.scalar.dma_start(
                        hts[:, :cnt],
                        hT_fullB[kc * d1:(kc + 1) * d1, o:o + cnt])
                ps = p3ps.tile([128, 128], DT.float32, tag="ps2", space="PSUM")
                nc.tensor.matmul(ps[:cnt, :], lhsT=hts[:, :cnt], rhs=tgr[:],
                                 start=True, stop=True)
                row = p3.tile([128, 66], DT.bfloat16, tag="row2")
                nc.vector.tensor_copy(out=row[:cnt, :], in_=ps[:cnt, 0:66])
                ro = kc4 * nloc + u * 128
                nc.scalar.dma_start(t2[s2][ro:ro + cnt, 0:66], row[:cnt, :])

            # sprinkle t2 chunk-A builds into the tail groups of pass B
            ngr1 = len(cfg.groups[1])
            t2a = [(s2, kc4, u) for u in range(ca_tiles)
                   for s2 in range(2) for kc4 in range(4)]
            spr_start = ngr1 - 5
            per_g2 = cdiv(len(t2a), 5)

            def preB(gi_i):
                if gi_i >= spr_start:
                    i = gi_i - spr_start
                    for (s2, kc4, u) in t2a[i * per_g2:(i + 1) * per_g2]:
                        t2_tile(s2, kc4, u)

            grid_pass(1, 1, t1[1], gidx1_h, adst1[1], H, 64, accB1, None,
                      post1, pre_group=preB)

            nc.gpsimd.collective_compute(
                "AllGather", ALU.bypass, ins=[hT_shB[:]], outs=[hT_fullB[:]],
                replica_groups=[list(range(cfg.ncores))])

            # adst2 in pass-0 order (gpsimd idle during AllGather B)
            ga = cpool.tile([128, ntiles, 64], DT.float32, tag="ga")
            nc.gpsimd.dma_gather(ga[:], adst2sc[:], a2ai[:], NRP, NRP, 64,
                                 single_packet=False)
            nc.vector.tensor_copy(out=adst2A[:, :, 0], in_=ga[:, :, 0])

            # ================= t2 build (chunk B; A was sprinkled) =========
            if stop_after < 4:
                raise _StopBuild()
            for u in range(ca_tiles, ntiles):
                for s2 in range(2):
                    for kc4 in range(4):
                        t2_tile(s2, kc4, u)

            # ================= layer 2 =================
            if stop_after < 5:
                raise _StopBuild()
            grid_pass(2, 0, t2[0], gidx2_h, adst2A[:], 1, 64,
                      None, part2, None)

            if stop_after < 6:
                raise _StopBuild()

            def post2(t, rows):
                cg = cb.tile([128, 1, 128], DT.float32, tag="cg2")
                nc.gpsimd.dma_gather(cg[:], part2[:],
                                     cr1[:, t * 8:(t + 1) * 8], 128, 128, 128,
                                     single_packet=False)
                comb = cb.tile([128, 65], DT.float32, tag="comb2")
                nc.vector.tensor_tensor(out=comb[:], in0=cg[:, 0, 0:65],
                                        in1=accB2[:, t, :], op=ALU.add)
                rec = cb.tile([128, 1], DT.float32, tag="rec2")
                nc.vector.tensor_scalar_add(rec[:], comb[:, 64:65], EPS)
                nc.vector.reciprocal(rec[:], rec[:])
                gg = cb.tile([128, 64], DT.float32, tag="gg")
                nc.vector.tensor_tensor(
                    out=gg[:], in0=comb[:, 0:64],
                    in1=rec[:].to_broadcast([128, 64]), op=ALU.mult)
                psT = cps.tile([64, 128], DT.float32, tag="psT2", space="PSUM")
                nc.tensor.transpose(out=psT[:], in_=gg[:, :], identity=ident[:])
                gtb = cb.tile([64, 128], DT.bfloat16, tag="gtb")
                nc.any.tensor_copy(out=gtb[:], in_=psT[:])
                ps2 = cps.tile([128, C2], DT.float32, tag="ps22", space="PSUM")
                nc.tensor.matmul(ps2[:], lhsT=gtb[:], rhs=w2t[:],
                                 start=True, stop=True)
                o2 = cb.tile([128, C2], DT.float32, tag="o2")
                nc.vector.tensor_tensor(out=o2[:], in0=ps2[:], in1=b2s[:],
                                        op=ALU.add)
                nc.sync.dma_start(out_h[t * 128:t * 128 + 128, :], o2[:])

            grid_pass(2, 1, t2[1], gidx2_h, adst2B[:], 1, 64,
                      accB2, None, post2)

    except _StopBuild:
        pass
    nc.compile()
    return nc


# ----------------------------------------------------------------------------
# Host entry
# ----------------------------------------------------------------------------
def host_inputs(cfg, x, W1, att_src1, att_dst1, bias1, W2, att_src2,
                att_dst2, bias2):
    import ml_dtypes
    H, C1, C2, d1 = cfg.H, cfg.C1, cfg.C2, cfg.d1
    x = np.asarray(x, np.float32)
    W1 = np.asarray(W1, np.float32)
    A_src = np.zeros((d1, H), np.float32)
    A_dst = np.zeros((d1, H), np.float32)
    for h in range(H):
        A_src[h * C1:(h + 1) * C1, h] = np.asarray(att_src1, np.float32)[h]
        A_dst[h * C1:(h + 1) * C1, h] = np.asarray(att_dst1, np.float32)[h]
    w1aug = np.zeros((cfg.F, 128), np.float32)
    w1aug[:, :d1] = W1
    w1aug[:, d1:d1 + H] = W1 @ A_src
    cfg.adst1_full = x @ (W1 @ A_dst)

    W2 = np.asarray(W2, np.float32)
    tgr = np.zeros((d1, 128), np.float32)
    tgr[:, :d1] = np.eye(d1)
    tgr[:, d1:d1 + 1] = W2 @ np.asarray(att_src2, np.float32).T
    tgr[:, d1 + 1:d1 + 2] = W2 @ np.asarray(att_dst2, np.float32).T

    sent1 = np.zeros((1, 256), np.float32)
    sent1[0, d1:d1 + 2 * H] = SENT_VAL
    sent2 = np.zeros((1, 256), np.float32)
    sent2[0, d1:d1 + 2] = SENT_VAL

    common = {
        "xT": np.ascontiguousarray(x.T),
        "w1aug": w1aug,
        "tgr": tgr.astype(ml_dtypes.bfloat16),
        "w2t": W2.astype(ml_dtypes.bfloat16),
        "bias1r": np.tile(np.asarray(bias1, np.float32)[None, :], (128, 1)),
        "bias2r": np.tile(np.asarray(bias2, np.float32)[None, :], (128, 1)),
        "sent1": sent1.astype(ml_dtypes.bfloat16),
        "sent2": sent2.astype(ml_dtypes.bfloat16),
    }
    in_maps = []
    for c in range(cfg.ncores):
        r = build_routing(cfg, c)
        r["adst1p"] = r["adst1p"].astype(ml_dtypes.bfloat16)
        in_maps.append({**common, **r})
    return in_maps


_CACHE = {}


def _run(x, edge_index, W1, att_src1, att_dst1, bias1, W2, att_src2,
         att_dst2, bias2, **run_kwargs):
    x = np.asarray(x, dtype=np.float32)
    N, F = x.shape
    ei = np.asarray(edge_index).astype(np.int64)
    E = ei.shape[1]
    loops = np.arange(N, dtype=np.int64)
    src = np.concatenate([ei[0], loops])
    dst = np.concatenate([ei[1], loops])
    cfg = Cfg(N, F, E, src, dst)
    key = (N, F, E, tuple(cfg.D[0]), tuple(cfg.D[1]))
    if key not in _CACHE:
        _CACHE[key] = build_program(cfg)
    nc = _CACHE[key]
    in_maps = host_inputs(cfg, x, W1, att_src1, att_dst1, bias1,
                          W2, att_src2, att_dst2, bias2)
    res = run_bass_kernel_spmd(nc, in_maps, list(range(cfg.ncores)),
                               **run_kwargs)
    out = np.empty((N, cfg.C2), dtype=np.float32)
    for c in range(cfg.ncores):
        r = np.asarray(res.results[c]["out"], dtype=np.float32)
        out[c * cfg.nloc + cfg.cores[c]["order"][1]] = r[:cfg.nloc]
    return out, res


def kernel(x, edge_index, W1, att_src1, att_dst1, bias1, W2, att_src2,
           att_dst2, bias2):
    out, _ = _run(x, edge_index, W1, att_src1, att_dst1, bias1, W2,
                  att_src2, att_dst2, bias2)
    return out


# revision 16
# speedup vs baseline: 1.5480x; 1.0730x over previous
"""2-layer GAT kernel for Trainium2 (8 NeuronCores), Bass/Tile.  v3.

Sharding: nodes by dst across 8 cores; edges routed to the dst owner.
Per core, edges split into two passes by src half (dma_gather idx is int16
-> gather tables limited to <=32768 rows).  Per pass, dst nodes are sorted
by per-pass degree and packed into 128-partition tiles with DATA-DEPENDENT
exact slot budgets D[s][t] (max over cores; program compiled per budget
vector).  Edge j of dst node d sits at (partition rank%128, tile rank//128,
slot j).  Pad slots point at a sentinel table row (a_src = -1e4 => p = 0).

Both layers share the SAME slot grids (same edges, same orders):
  gidx1[slot] = src id within its half (layer-1 table row)
  gidx2[slot] = global pass-order position of src (layer-2 table row)

Layer tables:
  t1[s] = [x @ W1 | x @ W1 @ Asrc] rows (f32, 512B), built on-device from
    host-pre-transposed x; a_dst1 comes host-computed+permuted (adst1p).
  t2[s] = [g | g@(W2 a_src2^T) | g@(W2 a_dst2^T)] rows (f32, 512B) where
    g = elu(out1 + b1).  h^T (bf16, pass-1-order) is AllGathered in two
    column chunks; t2 rows are one matmul per tile vs [I | w2a] rhs.
    Layer-2 aggregates 64-wide g; W2 is applied after normalization
    (out2 = (gagg/den) @ W2 + b2), valid because W2 is linear.

Per pass: pass 0 writes per-tile partial rows [num|den] to HBM; pass 1
reduces directly into an SBUF accumulator and immediately combines each
tile: gather the pass-0 partial rows for the tile's nodes (cross-rank
permutation), add, normalize.  Layer-1 combine also emits h^T columns and
a_dst2; layer-2 combine applies W2+bias and stores output rows in pass-1
order; the host un-permutes rows at the end.
"""

import numpy as np


class _StopBuild(Exception):
    pass


import concourse.bacc as bacc
import concourse.bass as bass
import concourse.mybir as mybir
import concourse.tile as tile
from concourse._compat import cdiv
from concourse.bass_utils import run_bass_kernel_spmd

AF = mybir.ActivationFunctionType
ALU = mybir.AluOpType
AX = mybir.AxisListType
DT = mybir.dt

NEG_SLOPE = 0.2
EPS = 1e-16
SENT_VAL = -1e4
GC_TARGET = 64


def _wrap_idx(idx):
    """[n] -> [128, n/16] int16: position j -> (partition j%16, col j//16),
    replicated across the 8 groups of 16 partitions."""
    idx = np.asarray(idx, dtype=np.int16)
    assert len(idx) % 16 == 0
    return np.tile(idx.reshape(-1, 16).T, (8, 1))


# ----------------------------------------------------------------------------
# Configuration + host routing (data-dependent)
# ----------------------------------------------------------------------------
class Cfg:
    def __init__(self, N, F, E, src, dst):
        ncores = 8
        self.N, self.F, self.E, self.ncores = N, F, E, ncores
        self.H, self.C1, self.C2 = 8, 8, 128
        self.d1 = 64
        self.nloc = N // ncores          # 6250
        self.half = N // 2               # 25000
        self.ntiles = cdiv(self.nloc, 128)   # 49
        self.nrp = self.ntiles * 128         # 6272
        nloc, half, ntiles = self.nloc, self.half, self.ntiles

        # ---- per-core routing part 1: degrees / orders ----
        self.cores = []
        for c in range(ncores):
            base = c * nloc
            m = (dst >= base) & (dst < base + nloc)
            s_c = src[m]
            d_c = (dst[m] - base).astype(np.int64)
            info = {"s": s_c, "d": d_c, "deg": [], "order": [], "rank": []}
            for s in (0, 1):
                m2 = (s_c // half) == s
                deg = np.bincount(d_c[m2], minlength=nloc)
                order = np.argsort(-deg, kind="stable")
                rank = np.empty(nloc, dtype=np.int64)
                rank[order] = np.arange(nloc)
                info["deg"].append(deg)
                info["order"].append(order)
                info["rank"].append(rank)
            self.cores.append(info)

        # global pass-1-order position of every node (for gidx2 / hT layout)
        self.rank1_global = np.empty(N, dtype=np.int64)
        for c in range(ncores):
            self.rank1_global[c * nloc:(c + 1) * nloc] = self.cores[c]["rank"][1]

        # ---- shared exact budgets D[s][t] = max over cores of tile max ----
        self.D = []
        for s in (0, 1):
            mx = np.zeros(ntiles, dtype=np.int64)
            for c in range(ncores):
                sd = np.sort(self.cores[c]["deg"][s])[::-1]
                pad = np.zeros(ntiles * 128, dtype=np.int64)
                pad[:nloc] = sd
                mx = np.maximum(mx, pad.reshape(ntiles, 128).max(axis=1))
            self.D.append(np.maximum(mx, 1))
        self.col_off = [np.concatenate([[0], np.cumsum(D)]).astype(int)
                        for D in self.D]
        self.total_cols = [int(D.sum()) for D in self.D]

        # ---- group packing (cap GC_TARGET cols per gather) ----
        self.groups = []
        for s in (0, 1):
            gs, t = [], 0
            while t < ntiles:
                t0, c0 = t, int(self.col_off[s][t])
                cols = 0
                while t < ntiles and (cols == 0
                                      or cols + self.D[s][t] <= GC_TARGET):
                    cols += int(self.D[s][t])
                    t += 1
                gs.append((t0, t, c0, cols))
            self.groups.append(gs)
        self.GC = max(g[3] for gs in self.groups for g in gs)


def build_routing(cfg, core):
    """Per-core runtime index arrays."""
    nloc, half, ntiles, nrp = cfg.nloc, cfg.half, cfg.ntiles, cfg.nrp
    info = cfg.cores[core]
    s_c, d_c = info["s"], info["d"]
    g1, g2, adst1p = [], [], []
    for s in (0, 1):
        m2 = (s_c // half) == s
        ss = s_c[m2]                       # global src ids
        dd = d_c[m2]
        deg = info["deg"][s]
        rank = info["rank"][s]
        eo = np.lexsort((ss, dd))
        ss_o, dd_o = ss[eo], dd[eo]
        starts = np.concatenate([[0], np.cumsum(deg)])
        j = np.arange(len(dd_o)) - starts[dd_o]
        r = rank[dd_o]
        tile_e, row_e = r // 128, r % 128
        Dv = cfg.D[s]
        assert (j < Dv[tile_e]).all(), "slot budget overflow (exact budgets)"
        flat1 = np.full(cfg.total_cols[s] * 128, half, dtype=np.int64)
        flat1[(cfg.col_off[s][tile_e] + j) * 128 + row_e] = ss_o - s * half
        g1.append(flat1)
        # layer-2 positions: owner-core pass-1 rank, table offset by half
        pos = (ss_o // nloc - 4 * s) * nloc + cfg.rank1_global[ss_o]
        flat2 = np.full(cfg.total_cols[s] * 128, half, dtype=np.int64)
        flat2[(cfg.col_off[s][tile_e] + j) * 128 + row_e] = pos
        g2.append(flat2)
        a = np.full((nrp, cfg.H), SENT_VAL, dtype=np.float32)
        a[:nloc] = cfg.adst1_full[core * nloc + info["order"][s]]
        adst1p.append(a)
    # cross: pass-1-order position j -> pass-0 partial row
    cross = np.zeros(nrp, dtype=np.int64)
    cross[:nloc] = info["rank"][0][info["order"][1]]
    # adst2 A-order: pass-0 position i -> pass-1 staged row
    a2ai = np.zeros(nrp, dtype=np.int64)
    a2ai[:nloc] = info["rank"][1][info["order"][0]]
    return {
        "gidx1": _wrap_idx(np.concatenate(g1)),
        "gidx2": _wrap_idx(np.concatenate(g2)),
        "adst1p": np.concatenate(adst1p, axis=0),
        "cross1": _wrap_idx(cross),
        "a2ai": _wrap_idx(a2ai),
    }


# ----------------------------------------------------------------------------
# Device program
# ----------------------------------------------------------------------------
def build_program(cfg, stop_after=99):
    from concourse.masks import make_identity

    nc = bacc.Bacc(None, target_bir_lowering=False, debug=True)
    H, d1, C2, F = cfg.H, cfg.d1, cfg.C2, cfg.F
    nloc, half, ntiles = cfg.nloc, cfg.half, cfg.ntiles
    nfull_tiles = cdiv(cfg.N, 128)
    NRP, GC = cfg.nrp, cfg.GC
    tail = nloc - (ntiles - 1) * 128
    ca_tiles = 33                       # hT AllGather chunk A: tiles 0..32
    ca_cols = ca_tiles * 128            # 3200
    cb_cols = nloc - ca_cols            # 3050

    # ---- external IO ----
    xT = nc.dram_tensor("xT", [F, cfg.N], DT.float32, kind="ExternalInput")
    w1aug_h = nc.dram_tensor("w1aug", [F, 128], DT.float32, kind="ExternalInput")
    tgr_h = nc.dram_tensor("tgr", [d1, 128], DT.bfloat16, kind="ExternalInput")
    w2t_h = nc.dram_tensor("w2t", [d1, C2], DT.bfloat16, kind="ExternalInput")
    b1_h = nc.dram_tensor("bias1r", [128, d1], DT.float32, kind="ExternalInput")
    b2_h = nc.dram_tensor("bias2r", [128, C2], DT.float32, kind="ExternalInput")
    sent1_h = nc.dram_tensor("sent1", [1, 256], DT.bfloat16, kind="ExternalInput")
    sent2_h = nc.dram_tensor("sent2", [1, 256], DT.bfloat16, kind="ExternalInput")
    tc01 = cfg.total_cols[0] + cfg.total_cols[1]
    gidx1_h = nc.dram_tensor("gidx1", [128, tc01 * 8], DT.int16, kind="ExternalInput")
    gidx2_h = nc.dram_tensor("gidx2", [128, tc01 * 8], DT.int16, kind="ExternalInput")
    adst1p_h = nc.dram_tensor("adst1p", [2 * NRP, H], DT.bfloat16, kind="ExternalInput")
    cross1_h = nc.dram_tensor("cross1", [128, NRP // 16], DT.int16, kind="ExternalInput")
    a2ai_h = nc.dram_tensor("a2ai", [128, NRP // 16], DT.int16, kind="ExternalInput")
    out_h = nc.dram_tensor("out", [NRP, C2], DT.float32, kind="ExternalOutput")

    # ---- internal DRAM ----
    t1 = [nc.dram_tensor(f"t1_{s}", [half + 1, 256], DT.bfloat16) for s in range(2)]
    t2 = [nc.dram_tensor(f"t2_{s}", [half + 1, 256], DT.bfloat16) for s in range(2)]
    part1 = nc.dram_tensor("part1", [NRP, 128], DT.float32)
    part2 = nc.dram_tensor("part2", [NRP, 128], DT.float32)
    adst2sc = nc.dram_tensor("adst2sc", [NRP, 64], DT.float32)
    hT_shA = nc.dram_tensor("hT_shA", [d1, ca_cols], DT.bfloat16)
    hT_shB = nc.dram_tensor("hT_shB", [d1, cb_cols], DT.bfloat16)
    hT_fullA = nc.dram_tensor("hT_fullA", [cfg.ncores * d1, ca_cols], DT.bfloat16)
    hT_fullB = nc.dram_tensor("hT_fullB", [cfg.ncores * d1, cb_cols], DT.bfloat16)

    try:
      with tile.TileContext(nc) as tc:
        with tc.tile_pool(name="const", bufs=1) as cpool, \
             tc.tile_pool(name="p0", bufs=4) as p0, \
             tc.tile_pool(name="p0ps", bufs=2, space="PSUM") as p0ps, \
             tc.tile_pool(name="pass", bufs=3) as pp, \
             tc.tile_pool(name="px", bufs=2) as pxp, \
             tc.tile_pool(name="gi", bufs=4) as gip, \
             tc.tile_pool(name="cmb", bufs=2) as cb, \
             tc.tile_pool(name="cps", bufs=1, space="PSUM") as cps, \
             tc.tile_pool(name="p3", bufs=4) as p3, \
             tc.tile_pool(name="p3ps", bufs=2, space="PSUM") as p3ps:
            w1s = cpool.tile([F, 128], DT.float32)
            nc.sync.dma_start(w1s[:], w1aug_h[:])
            tgr = cpool.tile([d1, 128], DT.bfloat16)
            nc.sync.dma_start(tgr[:], tgr_h[:])
            w2t = cpool.tile([d1, C2], DT.bfloat16)
            nc.sync.dma_start(w2t[:], w2t_h[:])
            b1s = cpool.tile([128, d1], DT.float32)
            nc.sync.dma_start(b1s[:], b1_h[:])
            b2s = cpool.tile([128, C2], DT.float32)
            nc.sync.dma_start(b2s[:], b2_h[:])
            ident = cpool.tile([128, 128], DT.float32)
            make_identity(nc, ident[:])
            cr1 = cpool.tile([128, NRP // 16], DT.int16)
            nc.sync.dma_start(cr1[:], cross1_h[:])
            a2ai = cpool.tile([128, NRP // 16], DT.int16)
            nc.sync.dma_start(a2ai[:], a2ai_h[:])
            adst1 = []
            for s in range(2):
                a = cpool.tile([128, ntiles, H], DT.bfloat16, tag=f"adst1_{s}")
                nc.sync.dma_start(
                    a[:],
                    adst1p_h[s * NRP:(s + 1) * NRP, :]
                    .rearrange("(t p) h -> p t h", p=128))
                adst1.append(a)
            accB1 = cpool.tile([128, ntiles, 72], DT.float32)
            accB2 = cpool.tile([128, ntiles, 65], DT.float32)
            adst2B = cpool.tile([128, ntiles, 1], DT.bfloat16)
            adst2A = cpool.tile([128, ntiles, 1], DT.bfloat16)

            # sentinel rows
            sc1 = cpool.tile([1, 256], DT.bfloat16, tag="sent1")
            nc.sync.dma_start(sc1[:], sent1_h[:])
            sc2 = cpool.tile([1, 256], DT.bfloat16, tag="sent2")
            nc.sync.dma_start(sc2[:], sent2_h[:])
            for s in range(2):
                nc.sync.dma_start(t1[s][half:half + 1, :], sc1[:])
                nc.sync.dma_start(t2[s][half:half + 1, :], sc2[:])

            # ================= P0: layer-1 table =================
            if stop_after < 1:
                raise _StopBuild()

            def p0_block(k0, nk):
                n0 = k0 * 128
                cnt = min(nk * 128, cfg.N - n0)
                xt_t = p0.tile([F, 4 * 128], DT.float32, tag="xt")
                nc.scalar.dma_start(xt_t[:, :cnt], xT[:, n0:n0 + cnt])
                row = p0.tile([128, 4, 72], DT.bfloat16, tag="row")
                for i in range(nk):
                    c = min(128, cfg.N - n0 - i * 128)
                    if c <= 0:
                        break
                    ps = p0ps.tile([128, 128], DT.float32, tag="ps",
                                   space="PSUM")
                    nc.tensor.matmul(ps[:c, :],
                                     lhsT=xt_t[:, i * 128:i * 128 + c],
                                     rhs=w1s[:], start=True, stop=True)
                    nc.vector.tensor_copy(out=row[:c, i, :],
                                          in_=ps[:c, 0:72])
                if True:
                    for i in range(nk):
                        m0 = n0 + i * 128
                        c = min(128, cfg.N - m0)
                        if c <= 0:
                            break
                        for s in range(2):
                            lo, hi = max(m0, s * half), min(m0 + c,
                                                           (s + 1) * half)
                            if lo < hi:
                                nc.scalar.dma_start(
                                    t1[s][lo - s * half:hi - s * half, 0:72],
                                    row[lo - m0:hi - m0, i, :])

            half0_tiles = half // 128 + 1          # tiles covering src half 0
            p0_blocks = [(k, min(4, nfull_tiles - k))
                         for k in range(0, nfull_tiles, 4)]
            nb_half0 = (half0_tiles + 3) // 4
            for (k0, nk) in p0_blocks[:nb_half0]:
                p0_block(k0, nk)

            # ================= pass machinery =================
            def grid_pass(layer, s, tbl, gidx_h, adst_tile, nheads, dfeat,
                          accum, part, per_tile_post, pre_group=None):
                """One slot-grid pass.  accum=None: write partial rows to
                `part` (pass 0).  accum=tile: reduce into SBUF (pass 1) and
                call per_tile_post(t, rows) after each tile."""
                base8 = (cfg.total_cols[0] if s == 1 else 0) * 8
                for gidx_i, (t0, t1_, c0, ncols) in enumerate(cfg.groups[s]):
                    if pre_group is not None:
                        pre_group(gidx_i)
                    gi = gip.tile([128, GC * 8], DT.int16, tag="gi")
                    nc.sync.dma_start(
                        gi[:, :ncols * 8],
                        gidx_h[:, base8 + c0 * 8: base8 + (c0 + ncols) * 8])
                    G = pp.tile([128, GC, 256], DT.bfloat16, tag="G")
                    nc.gpsimd.dma_gather(G[:, :ncols, :], tbl[:],
                                         gi[:, :ncols * 8], ncols * 128,
                                         ncols * 128, 256, single_packet=False)
                    pex = pxp.tile([128, GC, 64], DT.bfloat16, tag="px")
                    for t in range(t0, t1_):
                        D = int(cfg.D[s][t])
                        o = int(cfg.col_off[s][t]) - c0
                        Gt = G[:, o:o + D, :]
                        asrc = Gt[:, :, 64:64 + nheads]
                        if accum is None:
                            res = pp.tile([128, 128], DT.float32, tag="res")
                            nc.any.memset(res[:, 64 + nheads:], 0.0)
                            dn = res[:, 64:64 + nheads]
                            nm = res[:, 0:64]
                        else:
                            dn = accum[:, t, 64:64 + nheads]
                            nm = accum[:, t, 0:64]
                        pext = pex[:, o:o + D, :]
                        if False:
                            pass
                        else:
                            al = pp.tile([128, GC, 8], DT.bfloat16, tag="al")
                            alt = al[:, :D, :nheads]
                            nc.vector.tensor_tensor(
                                out=alt, in0=asrc,
                                in1=adst_tile[:, t:t + 1, :]
                                    .to_broadcast([128, D, nheads]),
                                op=ALU.add)
                            nc.vector.scalar_tensor_tensor(
                                out=alt, in0=alt, scalar=NEG_SLOPE, in1=alt,
                                op0=ALU.mult, op1=ALU.max)
                            nc.scalar.activation(
                                out=pext,
                                in_=alt.rearrange("p j (h c) -> p j h c", c=1)
                                       .to_broadcast([128, D, nheads,
                                                      64 // nheads]),
                                func=AF.Exp)
                            nc.vector.tensor_reduce(
                                out=dn,
                                in_=pext.rearrange("p j (h c) -> p h c j",
                                                   h=nheads)[:, :, 0, :],
                                axis=AX.X, op=ALU.add)
                            nc.vector.tensor_tensor(
                                out=pext, in0=Gt[:, :, 0:64],
                                in1=pext, op=ALU.mult)
                        nc.vector.tensor_reduce(
                            out=nm, in_=pext.rearrange("p j f -> p f j"),
                            axis=AX.X, op=ALU.add)
                        if accum is None:
                            nc.sync.dma_start(
                                part[t * 128:(t + 1) * 128, :], res[:])
                        else:
                            per_tile_post(t, 128 if t < ntiles - 1 else tail)

            # ================= layer 1 =================
            if stop_after < 2:
                raise _StopBuild()
            p0_rest = p0_blocks[nb_half0:]
            ngr0 = len(cfg.groups[0])
            per_g = cdiv(len(p0_rest), max(ngr0 - 1, 1))

            def preA(gi_i):
                for (k0, nk) in p0_rest[gi_i * per_g:(gi_i + 1) * per_g]:
                    p0_block(k0, nk)

            grid_pass(1, 0, t1[0], gidx1_h, adst1[0], H, 64, None, part1,
                      None, pre_group=preA)

            if stop_after < 3:
                raise _StopBuild()

            def post1(t, rows):
                cg = cb.tile([128, 1, 128], DT.float32, tag="cg1")
                nc.gpsimd.dma_gather(cg[:], part1[:],
                                     cr1[:, t * 8:(t + 1) * 8], 128, 128, 128,
                                     single_packet=False)
                comb = cb.tile([128, 72], DT.float32, tag="comb1")
                nc.vector.tensor_tensor(out=comb[:], in0=cg[:, 0, 0:72],
                                        in1=accB1[:, t, :], op=ALU.add)
                rec = cb.tile([128, H], DT.float32, tag="rec1")
                nc.vector.tensor_scalar_add(rec[:], comb[:, 64:72], EPS)
                nc.vector.reciprocal(rec[:], rec[:])
                hf = cb.tile([128, 64], DT.float32, tag="hf")
                nc.vector.tensor_tensor(
                    out=hf[:].rearrange("p (h c) -> p h c", h=H),
                    in0=comb[:, 0:64].rearrange("p (h c) -> p h c", h=H),
                    in1=rec[:].rearrange("p (h c) -> p h c", c=1)
                              .to_broadcast([128, H, 8]),
                    op=ALU.mult)
                nc.vector.tensor_tensor(out=hf[:], in0=hf[:], in1=b1s[:],
                                        op=ALU.add)
                # elu(x) = relu(x) + exp(-relu(-x)) - 1
                r = cb.tile([128, 64], DT.float32, tag="r")
                nc.scalar.activation(out=r[:], in_=hf[:], func=AF.Relu)
                m = cb.tile([128, 64], DT.float32, tag="m")
                nc.scalar.activation(out=m[:], in_=hf[:], func=AF.Relu,
                                     scale=-1.0)
                e = cb.tile([128, 64], DT.float32, tag="e")
                nc.scalar.activation(out=e[:], in_=m[:], func=AF.Exp,
                                     scale=-1.0)
                nc.vector.tensor_tensor(out=r[:], in0=r[:], in1=e[:], op=ALU.add)
                nc.vector.tensor_scalar_add(r[:], r[:], -1.0)
                psT = cps.tile([64, 128], DT.float32, tag="psT", space="PSUM")
                nc.tensor.transpose(out=psT[:], in_=r[:, :], identity=ident[:])
                htb = cb.tile([64, 128], DT.bfloat16, tag="htb")
                nc.any.tensor_copy(out=htb[:], in_=psT[:])
                if t < ca_tiles:
                    nc.sync.dma_start(hT_shA[:, t * 128:t * 128 + rows],
                                      htb[:, :rows])
                else:
                    o = (t - ca_tiles) * 128
                    nc.sync.dma_start(hT_shB[:, o:o + rows], htb[:, :rows])
                psA = cps.tile([128, 1], DT.float32, tag="psA", space="PSUM")
                nc.tensor.matmul(psA[:], lhsT=htb[:], rhs=tgr[:, 65:66],
                                 start=True, stop=True)
                nc.any.tensor_copy(out=adst2B[:, t, :], in_=psA[:])
                adrow = cb.tile([128, 64], DT.float32, tag="adrow")
                nc.any.tensor_copy(out=adrow[:],
                                   in_=psA[:, 0:1].to_broadcast([128, 64]))
                nc.sync.dma_start(adst2sc[t * 128:(t + 1) * 128, :], adrow[:])
                if t == ca_tiles - 1:
                    nc.gpsimd.collective_compute(
                        "AllGather", ALU.bypass, ins=[hT_shA[:]],
                        outs=[hT_fullA[:]],
                        replica_groups=[list(range(cfg.ncores))])

            def t2_block(s2, kc4, u0, nu):
                kc = s2 * 4 + kc4
                u_end = u0 + nu
                cols = (min(u_end * 128, nloc)) - u0 * 128
                hts = p3.tile([d1, 4 * 128], DT.bfloat16, tag="hts")
                if u_end <= ca_tiles:
                    nc.scalar.dma_start(
                        hts[:, :cols],
                        hT_fullA[kc * d1:(kc + 1) * d1,
                                 u0 * 128:u0 * 128 + cols])
                else:
                    o = (u0 - ca_tiles) * 128
                    nc.scalar.dma_start(
                        hts[:, :cols],
                        hT_fullB[kc * d1:(kc + 1) * d1, o:o + cols])
                row = p3.tile([128, 4, 66], DT.bfloat16, tag="row2")
                for i in range(nu):
                    c = min(128, nloc - (u0 + i) * 128)
                    ps = p3ps.tile([128, 128], DT.float32, tag="ps2",
                                   space="PSUM")
                    nc.tensor.matmul(ps[:c, :],
                                     lhsT=hts[:, i * 128:i * 128 + c],
                                     rhs=tgr[:], start=True, stop=True)
                    nc.vector.tensor_copy(out=row[:c, i, :],
                                          in_=ps[:c, 0:66])
                ro = kc4 * nloc + u0 * 128
                if True:
                    for i in range(nu):
                        c = min(128, nloc - (u0 + i) * 128)
                        nc.scalar.dma_start(
                            t2[s2][ro + i * 128:ro + i * 128 + c, 0:66],
                            row[:c, i, :])

            # chunk-A u-blocks (never straddle the ca_tiles boundary)
            t2a_blocks, t2b_blocks = [], []
            u = 0
            while u < ca_tiles:
                nu = min(4, ca_tiles - u)
                t2a_blocks += [(s2, kc4, u, nu)
                               for s2 in range(2) for kc4 in range(4)]
                u += nu
            while u < ntiles:
                nu = min(4, ntiles - u)
                t2b_blocks += [(s2, kc4, u, nu)
                               for s2 in range(2) for kc4 in range(4)]
                u += nu

            # sprinkle t2 chunk-A builds into the tail groups of pass B
            ngr1 = len(cfg.groups[1])
            nspr = 4
            spr_start = ngr1 - nspr
            per_g2 = cdiv(len(t2a_blocks), nspr)

            def preB(gi_i):
                if gi_i >= spr_start:
                    i = gi_i - spr_start
                    for (s2, kc4, u0, nu) in \
                            t2a_blocks[i * per_g2:(i + 1) * per_g2]:
                        t2_block(s2, kc4, u0, nu)

            grid_pass(1, 1, t1[1], gidx1_h, adst1[1], H, 64, accB1, None,
                      post1, pre_group=preB)

            nc.gpsimd.collective_compute(
                "AllGather", ALU.bypass, ins=[hT_shB[:]], outs=[hT_fullB[:]],
                replica_groups=[list(range(cfg.ncores))])

            # adst2 in pass-0 order (gpsimd idle during AllGather B)
            ga = cpool.tile([128, ntiles, 64], DT.float32, tag="ga")
            nc.gpsimd.dma_gather(ga[:], adst2sc[:], a2ai[:], NRP, NRP, 64,
                                 single_packet=False)
            nc.vector.tensor_copy(out=adst2A[:, :, 0], in_=ga[:, :, 0])

            # ================= t2 build (chunk B; A was sprinkled) =========
            if stop_after < 4:
                raise _StopBuild()
            for (s2, kc4, u0, nu) in t2b_blocks:
                t2_block(s2, kc4, u0, nu)

            # ================= layer 2 =================
            if stop_after < 5:
                raise _StopBuild()
            grid_pass(2, 0, t2[0], gidx2_h, adst2A[:], 1, 64,
                      None, part2, None)

            if stop_after < 6:
                raise _StopBuild()

            def post2(t, rows):
                cg = cb.tile([128, 1, 128], DT.float32, tag="cg2")
                nc.gpsimd.dma_gather(cg[:], part2[:],
                                     cr1[:, t * 8:(t + 1) * 8], 128, 128, 128,
                                     single_packet=False)
                comb = cb.tile([128, 65], DT.float32, tag="comb2")
                nc.vector.tensor_tensor(out=comb[:], in0=cg[:, 0, 0:65],
                                        in1=accB2[:, t, :], op=ALU.add)
                rec = cb.tile([128, 1], DT.float32, tag="rec2")
                nc.vector.tensor_scalar_add(rec[:], comb[:, 64:65], EPS)
                nc.vector.reciprocal(rec[:], rec[:])
                gg = cb.tile([128, 64], DT.float32, tag="gg")
                nc.vector.tensor_tensor(
                    out=gg[:], in0=comb[:, 0:64],
                    in1=rec[:].to_broadcast([128, 64]), op=ALU.mult)
                psT = cps.tile([64, 128], DT.float32, tag="psT2", space="PSUM")
                nc.tensor.transpose(out=psT[:], in_=gg[:, :], identity=ident[:])
                gtb = cb.tile([64, 128], DT.bfloat16, tag="gtb")
                nc.any.tensor_copy(out=gtb[:], in_=psT[:])
                ps2 = cps.tile([128, C2], DT.float32, tag="ps22", space="PSUM")
                nc.tensor.matmul(ps2[:], lhsT=gtb[:], rhs=w2t[:],
                                 start=True, stop=True)
                o2 = cb.tile([128, C2], DT.float32, tag="o2")
                nc.vector.tensor_tensor(out=o2[:], in0=ps2[:], in1=b2s[:],
                                        op=ALU.add)
                nc.sync.dma_start(out_h[t * 128:t * 128 + 128, :], o2[:])

            grid_pass(2, 1, t2[1], gidx2_h, adst2B[:], 1, 64,
                      accB2, None, post2)

    except _StopBuild:
        pass
    nc.compile()
    return nc


# ----------------------------------------------------------------------------
# Host entry
# ----------------------------------------------------------------------------
def host_inputs(cfg, x, W1, att_src1, att_dst1, bias1, W2, att_src2,
                att_dst2, bias2):
    import ml_dtypes
    H, C1, C2, d1 = cfg.H, cfg.C1, cfg.C2, cfg.d1
    x = np.asarray(x, np.float32)
    W1 = np.asarray(W1, np.float32)
    A_src = np.zeros((d1, H), np.float32)
    A_dst = np.zeros((d1, H), np.float32)
    for h in range(H):
        A_src[h * C1:(h + 1) * C1, h] = np.asarray(att_src1, np.float32)[h]
        A_dst[h * C1:(h + 1) * C1, h] = np.asarray(att_dst1, np.float32)[h]
    w1aug = np.zeros((cfg.F, 128), np.float32)
    w1aug[:, :d1] = W1
    w1aug[:, d1:d1 + H] = W1 @ A_src
    cfg.adst1_full = x @ (W1 @ A_dst)

    W2 = np.asarray(W2, np.float32)
    tgr = np.zeros((d1, 128), np.float32)
    tgr[:, :d1] = np.eye(d1)
    tgr[:, d1:d1 + 1] = W2 @ np.asarray(att_src2, np.float32).T
    tgr[:, d1 + 1:d1 + 2] = W2 @ np.asarray(att_dst2, np.float32).T

    sent1 = np.zeros((1, 256), np.float32)
    sent1[0, d1:d1 + 2 * H] = SENT_VAL
    sent2 = np.zeros((1, 256), np.float32)
    sent2[0, d1:d1 + 2] = SENT_VAL

    common = {
        "xT": np.ascontiguousarray(x.T),
        "w1aug": w1aug,
        "tgr": tgr.astype(ml_dtypes.bfloat16),
        "w2t": W2.astype(ml_dtypes.bfloat16),
        "bias1r": np.tile(np.asarray(bias1, np.float32)[None, :], (128, 1)),
        "bias2r": np.tile(np.asarray(bias2, np.float32)[None, :], (128, 1)),
        "sent1": sent1.astype(ml_dtypes.bfloat16),
        "sent2": sent2.astype(ml_dtypes.bfloat16),
    }
    in_maps = []
    for c in range(cfg.ncores):
        r = build_routing(cfg, c)
        r["adst1p"] = r["adst1p"].astype(ml_dtypes.bfloat16)
        in_maps.append({**common, **r})
    return in_maps


_CACHE = {}


def _run(x, edge_index, W1, att_src1, att_dst1, bias1, W2, att_src2,
         att_dst2, bias2, **run_kwargs):
    x = np.asarray(x, dtype=np.float32)
    N, F = x.shape
    ei = np.asarray(edge_index).astype(np.int64)
    E = ei.shape[1]
    loops = np.arange(N, dtype=np.int64)
    src = np.concatenate([ei[0], loops])
    dst = np.concatenate([ei[1], loops])
    cfg = Cfg(N, F, E, src, dst)
    key = (N, F, E, tuple(cfg.D[0]), tuple(cfg.D[1]))
    if key not in _CACHE:
        _CACHE[key] = build_program(cfg)
    nc = _CACHE[key]
    in_maps = host_inputs(cfg, x, W1, att_src1, att_dst1, bias1,
                          W2, att_src2, att_dst2, bias2)
    res = run_bass_kernel_spmd(nc, in_maps, list(range(cfg.ncores)),
                               **run_kwargs)
    out = np.empty((N, cfg.C2), dtype=np.float32)
    for c in range(cfg.ncores):
        r = np.asarray(res.results[c]["out"], dtype=np.float32)
        out[c * cfg.nloc + cfg.cores[c]["order"][1]] = r[:cfg.nloc]
    return out, res


def kernel(x, edge_index, W1, att_src1, att_dst1, bias1, W2, att_src2,
           att_dst2, bias2):
    out, _ = _run(x, edge_index, W1, att_src1, att_dst1, bias1, W2,
                  att_src2, att_dst2, bias2)
    return out


# revision 18
# speedup vs baseline: 1.6035x; 1.0359x over previous
"""2-layer GAT kernel for Trainium2 (8 NeuronCores), Bass/Tile.  v3.

Sharding: nodes by dst across 8 cores; edges routed to the dst owner.
Per core, edges split into two passes by src half (dma_gather idx is int16
-> gather tables limited to <=32768 rows).  Per pass, dst nodes are sorted
by per-pass degree and packed into 128-partition tiles with DATA-DEPENDENT
exact slot budgets D[s][t] (max over cores; program compiled per budget
vector).  Edge j of dst node d sits at (partition rank%128, tile rank//128,
slot j).  Pad slots point at a sentinel table row (a_src = -1e4 => p = 0).

Both layers share the SAME slot grids (same edges, same orders):
  gidx1[slot] = src id within its half (layer-1 table row)
  gidx2[slot] = global pass-order position of src (layer-2 table row)

Layer tables:
  t1[s] = [x @ W1 | x @ W1 @ Asrc] rows (f32, 512B), built on-device from
    host-pre-transposed x; a_dst1 comes host-computed+permuted (adst1p).
  t2[s] = [g | g@(W2 a_src2^T) | g@(W2 a_dst2^T)] rows (f32, 512B) where
    g = elu(out1 + b1).  h^T (bf16, pass-1-order) is AllGathered in two
    column chunks; t2 rows are one matmul per tile vs [I | w2a] rhs.
    Layer-2 aggregates 64-wide g; W2 is applied after normalization
    (out2 = (gagg/den) @ W2 + b2), valid because W2 is linear.

Per pass: pass 0 writes per-tile partial rows [num|den] to HBM; pass 1
reduces directly into an SBUF accumulator and immediately combines each
tile: gather the pass-0 partial rows for the tile's nodes (cross-rank
permutation), add, normalize.  Layer-1 combine also emits h^T columns and
a_dst2; layer-2 combine applies W2+bias and stores output rows in pass-1
order; the host un-permutes rows at the end.
"""

import numpy as np


class _StopBuild(Exception):
    pass


import concourse.bacc as bacc
import concourse.bass as bass
import concourse.mybir as mybir
import concourse.tile as tile
from concourse._compat import cdiv
from concourse.bass_utils import run_bass_kernel_spmd

AF = mybir.ActivationFunctionType
ALU = mybir.AluOpType
AX = mybir.AxisListType
DT = mybir.dt

NEG_SLOPE = 0.2
EPS = 1e-16
SENT_VAL = -1e4
GC_TARGET = 64


def _wrap_idx(idx):
    """[n] -> [128, n/16] int16: position j -> (partition j%16, col j//16),
    replicated across the 8 groups of 16 partitions."""
    idx = np.asarray(idx, dtype=np.int16)
    assert len(idx) % 16 == 0
    return np.tile(idx.reshape(-1, 16).T, (8, 1))


# ----------------------------------------------------------------------------
# Configuration + host routing (data-dependent)
# ----------------------------------------------------------------------------
class Cfg:
    def __init__(self, N, F, E, src, dst):
        ncores = 8
        self.N, self.F, self.E, self.ncores = N, F, E, ncores
        self.H, self.C1, self.C2 = 8, 8, 128
        self.d1 = 64
        self.nloc = N // ncores          # 6250
        self.half = N // 2               # 25000
        self.ntiles = cdiv(self.nloc, 128)   # 49
        self.nrp = self.ntiles * 128         # 6272
        nloc, half, ntiles = self.nloc, self.half, self.ntiles

        # ---- per-core routing part 1: degrees / orders ----
        self.cores = []
        for c in range(ncores):
            base = c * nloc
            m = (dst >= base) & (dst < base + nloc)
            s_c = src[m]
            d_c = (dst[m] - base).astype(np.int64)
            info = {"s": s_c, "d": d_c, "deg": [], "order": [], "rank": []}
            for s in (0, 1):
                m2 = (s_c // half) == s
                deg = np.bincount(d_c[m2], minlength=nloc)
                order = np.argsort(-deg, kind="stable")
                rank = np.empty(nloc, dtype=np.int64)
                rank[order] = np.arange(nloc)
                info["deg"].append(deg)
                info["order"].append(order)
                info["rank"].append(rank)
            self.cores.append(info)

        # global pass-1-order position of every node (for gidx2 / hT layout)
        self.rank1_global = np.empty(N, dtype=np.int64)
        for c in range(ncores):
            self.rank1_global[c * nloc:(c + 1) * nloc] = self.cores[c]["rank"][1]

        # ---- shared exact budgets D[s][t] = max over cores of tile max ----
        self.D = []
        for s in (0, 1):
            mx = np.zeros(ntiles, dtype=np.int64)
            for c in range(ncores):
                sd = np.sort(self.cores[c]["deg"][s])[::-1]
                pad = np.zeros(ntiles * 128, dtype=np.int64)
                pad[:nloc] = sd
                mx = np.maximum(mx, pad.reshape(ntiles, 128).max(axis=1))
            self.D.append(np.maximum(mx, 1))
        self.col_off = [np.concatenate([[0], np.cumsum(D)]).astype(int)
                        for D in self.D]
        self.total_cols = [int(D.sum()) for D in self.D]

        # ---- group packing (cap GC_TARGET cols per gather) ----
        self.groups = []
        for s in (0, 1):
            gs, t = [], 0
            while t < ntiles:
                t0, c0 = t, int(self.col_off[s][t])
                cols = 0
                while t < ntiles and (cols == 0
                                      or cols + self.D[s][t] <= GC_TARGET):
                    cols += int(self.D[s][t])
                    t += 1
                gs.append((t0, t, c0, cols))
            self.groups.append(gs)
        self.GC = max(g[3] for gs in self.groups for g in gs)


def build_routing(cfg, core):
    """Per-core runtime index arrays."""
    nloc, half, ntiles, nrp = cfg.nloc, cfg.half, cfg.ntiles, cfg.nrp
    info = cfg.cores[core]
    s_c, d_c = info["s"], info["d"]
    g1, g2, adst1p = [], [], []
    for s in (0, 1):
        m2 = (s_c // half) == s
        ss = s_c[m2]                       # global src ids
        dd = d_c[m2]
        deg = info["deg"][s]
        rank = info["rank"][s]
        eo = np.lexsort((ss, dd))
        ss_o, dd_o = ss[eo], dd[eo]
        starts = np.concatenate([[0], np.cumsum(deg)])
        j = np.arange(len(dd_o)) - starts[dd_o]
        r = rank[dd_o]
        tile_e, row_e = r // 128, r % 128
        Dv = cfg.D[s]
        assert (j < Dv[tile_e]).all(), "slot budget overflow (exact budgets)"
        flat1 = np.full(cfg.total_cols[s] * 128, half, dtype=np.int64)
        flat1[(cfg.col_off[s][tile_e] + j) * 128 + row_e] = ss_o - s * half
        g1.append(flat1)
        # layer-2 positions: owner-core pass-1 rank, table offset by half
        pos = (ss_o // nloc - 4 * s) * nloc + cfg.rank1_global[ss_o]
        flat2 = np.full(cfg.total_cols[s] * 128, half, dtype=np.int64)
        flat2[(cfg.col_off[s][tile_e] + j) * 128 + row_e] = pos
        g2.append(flat2)
        a = np.full((nrp, cfg.H), SENT_VAL, dtype=np.float32)
        a[:nloc] = cfg.adst1_full[core * nloc + info["order"][s]]
        adst1p.append(a)
    # cross: pass-1-order position j -> pass-0 partial row
    cross = np.zeros(nrp, dtype=np.int64)
    cross[:nloc] = info["rank"][0][info["order"][1]]
    # adst2 A-order: pass-0 position i -> pass-1 staged row
    a2ai = np.zeros(nrp, dtype=np.int64)
    a2ai[:nloc] = info["rank"][1][info["order"][0]]
    return {
        "gidx1": _wrap_idx(np.concatenate(g1)),
        "gidx2": _wrap_idx(np.concatenate(g2)),
        "adst1p": np.concatenate(adst1p, axis=0),
        "cross1": _wrap_idx(cross),
        "a2ai": _wrap_idx(a2ai),
    }


# ----------------------------------------------------------------------------
# Device program
# ----------------------------------------------------------------------------
def build_program(cfg, stop_after=99):
    from concourse.masks import make_identity

    nc = bacc.Bacc(None, target_bir_lowering=False, debug=True)
    H, d1, C2, F = cfg.H, cfg.d1, cfg.C2, cfg.F
    nloc, half, ntiles = cfg.nloc, cfg.half, cfg.ntiles
    nfull_tiles = cdiv(cfg.N, 128)
    NRP, GC = cfg.nrp, cfg.GC
    tail = nloc - (ntiles - 1) * 128
    ca_tiles = 33                       # hT AllGather chunk A: tiles 0..32
    ca_cols = ca_tiles * 128            # 3200
    cb_cols = nloc - ca_cols            # 3050

    # ---- external IO ----
    xT = nc.dram_tensor("xT", [F, cfg.N], DT.float32, kind="ExternalInput")
    w1aug_h = nc.dram_tensor("w1aug", [F, 128], DT.float32, kind="ExternalInput")
    tgr_h = nc.dram_tensor("tgr", [d1, 128], DT.bfloat16, kind="ExternalInput")
    w2t_h = nc.dram_tensor("w2t", [d1, C2], DT.bfloat16, kind="ExternalInput")
    b1_h = nc.dram_tensor("bias1r", [128, d1], DT.float32, kind="ExternalInput")
    b2_h = nc.dram_tensor("bias2r", [128, C2], DT.float32, kind="ExternalInput")
    sent1_h = nc.dram_tensor("sent1", [1, 256], DT.bfloat16, kind="ExternalInput")
    sent2_h = nc.dram_tensor("sent2", [1, 256], DT.bfloat16, kind="ExternalInput")
    tc01 = cfg.total_cols[0] + cfg.total_cols[1]
    gidx1_h = nc.dram_tensor("gidx1", [128, tc01 * 8], DT.int16, kind="ExternalInput")
    gidx2_h = nc.dram_tensor("gidx2", [128, tc01 * 8], DT.int16, kind="ExternalInput")
    adst1p_h = nc.dram_tensor("adst1p", [2 * NRP, H], DT.bfloat16, kind="ExternalInput")
    cross1_h = nc.dram_tensor("cross1", [128, NRP // 16], DT.int16, kind="ExternalInput")
    a2ai_h = nc.dram_tensor("a2ai", [128, NRP // 16], DT.int16, kind="ExternalInput")
    out_h = nc.dram_tensor("out", [NRP, C2], DT.float32, kind="ExternalOutput")

    # ---- internal DRAM ----
    t1 = [nc.dram_tensor(f"t1_{s}", [half + 1, 256], DT.bfloat16) for s in range(2)]
    t2 = [nc.dram_tensor(f"t2_{s}", [half + 1, 256], DT.bfloat16) for s in range(2)]
    part1 = nc.dram_tensor("part1", [NRP, 128], DT.float32)
    part2 = nc.dram_tensor("part2", [NRP, 128], DT.float32)
    adst2sc = nc.dram_tensor("adst2sc", [NRP, 64], DT.float32)
    hT_shA = nc.dram_tensor("hT_shA", [d1, ca_cols], DT.bfloat16)
    hT_shB = nc.dram_tensor("hT_shB", [d1, cb_cols], DT.bfloat16)
    hT_fullA = nc.dram_tensor("hT_fullA", [cfg.ncores * d1, ca_cols], DT.bfloat16)
    hT_fullB = nc.dram_tensor("hT_fullB", [cfg.ncores * d1, cb_cols], DT.bfloat16)

    try:
      with tile.TileContext(nc) as tc:
        with tc.tile_pool(name="const", bufs=1) as cpool, \
             tc.tile_pool(name="p0", bufs=4) as p0, \
             tc.tile_pool(name="p0ps", bufs=2, space="PSUM") as p0ps, \
             tc.tile_pool(name="pass", bufs=3) as pp, \
             tc.tile_pool(name="px", bufs=2) as pxp, \
             tc.tile_pool(name="gi", bufs=4) as gip, \
             tc.tile_pool(name="cmb", bufs=2) as cb, \
             tc.tile_pool(name="cps", bufs=1, space="PSUM") as cps, \
             tc.tile_pool(name="p3", bufs=4) as p3, \
             tc.tile_pool(name="p3ps", bufs=2, space="PSUM") as p3ps:
            w1s = cpool.tile([F, 128], DT.float32)
            nc.sync.dma_start(w1s[:], w1aug_h[:])
            tgr = cpool.tile([d1, 128], DT.bfloat16)
            nc.sync.dma_start(tgr[:], tgr_h[:])
            w2t = cpool.tile([d1, C2], DT.bfloat16)
            nc.sync.dma_start(w2t[:], w2t_h[:])
            b1s = cpool.tile([128, d1], DT.float32)
            nc.sync.dma_start(b1s[:], b1_h[:])
            b2s = cpool.tile([128, C2], DT.float32)
            nc.sync.dma_start(b2s[:], b2_h[:])
            ident = cpool.tile([128, 128], DT.float32)
            make_identity(nc, ident[:])
            cr1 = cpool.tile([128, NRP // 16], DT.int16)
            nc.sync.dma_start(cr1[:], cross1_h[:])
            a2ai = cpool.tile([128, NRP // 16], DT.int16)
            nc.sync.dma_start(a2ai[:], a2ai_h[:])
            adst1 = []
            for s in range(2):
                a = cpool.tile([128, ntiles, H], DT.bfloat16, tag=f"adst1_{s}")
                nc.sync.dma_start(
                    a[:],
                    adst1p_h[s * NRP:(s + 1) * NRP, :]
                    .rearrange("(t p) h -> p t h", p=128))
                adst1.append(a)
            accB1 = cpool.tile([128, ntiles, 72], DT.float32)
            accB2 = cpool.tile([128, ntiles, 65], DT.float32)
            adst2B = cpool.tile([128, ntiles, 1], DT.bfloat16)
            adst2A = cpool.tile([128, ntiles, 1], DT.bfloat16)

            # sentinel rows
            sc1 = cpool.tile([1, 256], DT.bfloat16, tag="sent1")
            nc.sync.dma_start(sc1[:], sent1_h[:])
            sc2 = cpool.tile([1, 256], DT.bfloat16, tag="sent2")
            nc.sync.dma_start(sc2[:], sent2_h[:])
            for s in range(2):
                nc.sync.dma_start(t1[s][half:half + 1, :], sc1[:])
                nc.sync.dma_start(t2[s][half:half + 1, :], sc2[:])

            # ================= P0: layer-1 table =================
            if stop_after < 1:
                raise _StopBuild()

            def p0_block(k0, nk):
                n0 = k0 * 128
                cnt = min(nk * 128, cfg.N - n0)
                xt_t = p0.tile([F, 4 * 128], DT.float32, tag="xt")
                nc.scalar.dma_start(xt_t[:, :cnt], xT[:, n0:n0 + cnt])
                row = p0.tile([128, 4, 72], DT.bfloat16, tag="row")
                for i in range(nk):
                    c = min(128, cfg.N - n0 - i * 128)
                    if c <= 0:
                        break
                    ps = p0ps.tile([128, 128], DT.float32, tag="ps",
                                   space="PSUM")
                    nc.tensor.matmul(ps[:c, :],
                                     lhsT=xt_t[:, i * 128:i * 128 + c],
                                     rhs=w1s[:], start=True, stop=True)
                    nc.vector.tensor_copy(out=row[:c, i, :],
                                          in_=ps[:c, 0:72])
                if True:
                    for i in range(nk):
                        m0 = n0 + i * 128
                        c = min(128, cfg.N - m0)
                        if c <= 0:
                            break
                        for s in range(2):
                            lo, hi = max(m0, s * half), min(m0 + c,
                                                           (s + 1) * half)
                            if lo < hi:
                                nc.scalar.dma_start(
                                    t1[s][lo - s * half:hi - s * half, 0:72],
                                    row[lo - m0:hi - m0, i, :])

            half0_tiles = half // 128 + 1          # tiles covering src half 0
            p0_blocks = [(k, min(4, nfull_tiles - k))
                         for k in range(0, nfull_tiles, 4)]
            nb_half0 = (half0_tiles + 3) // 4
            for (k0, nk) in p0_blocks[:nb_half0]:
                p0_block(k0, nk)

            # ================= pass machinery =================
            def grid_pass(layer, s, tbl, gidx_h, adst_tile, nheads, dfeat,
                          accum, part, per_tile_post, pre_group=None):
                """One slot-grid pass.  accum=None: write partial rows to
                `part` (pass 0).  accum=tile: reduce into SBUF (pass 1) and
                call per_tile_post(t, rows) after each tile."""
                base8 = (cfg.total_cols[0] if s == 1 else 0) * 8
                for gidx_i, (t0, t1_, c0, ncols) in enumerate(cfg.groups[s]):
                    if pre_group is not None:
                        pre_group(gidx_i)
                    gi = gip.tile([128, GC * 8], DT.int16, tag="gi")
                    nc.sync.dma_start(
                        gi[:, :ncols * 8],
                        gidx_h[:, base8 + c0 * 8: base8 + (c0 + ncols) * 8])
                    G = pp.tile([128, GC, 256], DT.bfloat16, tag="G")
                    nc.gpsimd.dma_gather(G[:, :ncols, :], tbl[:],
                                         gi[:, :ncols * 8], ncols * 128,
                                         ncols * 128, 256, single_packet=False)
                    pex = pxp.tile([128, GC, 64], DT.bfloat16, tag="px")
                    for t in range(t0, t1_):
                        D = int(cfg.D[s][t])
                        o = int(cfg.col_off[s][t]) - c0
                        Gt = G[:, o:o + D, :]
                        asrc = Gt[:, :, 64:64 + nheads]
                        if accum is None:
                            res = pp.tile([128, 128], DT.float32, tag="res")
                            nc.any.memset(res[:, 64 + nheads:], 0.0)
                            dn = res[:, 64:64 + nheads]
                            nm = res[:, 0:64]
                        else:
                            dn = accum[:, t, 64:64 + nheads]
                            nm = accum[:, t, 0:64]
                        pext = pex[:, o:o + D, :]
                        if nheads == 1:
                            # alpha+lrelu+exp+den all on the Act engine
                            p2 = pp.tile([128, GC, 1], DT.bfloat16, tag="p2")
                            nc.scalar.activation(
                                out=p2[:, :D, :], in_=asrc, func=AF.Lrelu,
                                bias=adst_tile[:, t, :], alpha=NEG_SLOPE)
                            nc.scalar.activation(
                                out=p2[:, :D, :], in_=p2[:, :D, :],
                                func=AF.Exp, accum_out=dn)
                            nc.vector.tensor_tensor(
                                out=pext, in0=Gt[:, :, 0:64],
                                in1=p2[:, :D, :].to_broadcast([128, D, 64]),
                                op=ALU.mult)
                        else:
                            al = pp.tile([128, GC, 8], DT.bfloat16, tag="al")
                            alt = al[:, :D, :nheads]
                            nc.vector.tensor_tensor(
                                out=alt, in0=asrc,
                                in1=adst_tile[:, t:t + 1, :]
                                    .to_broadcast([128, D, nheads]),
                                op=ALU.add)
                            nc.vector.scalar_tensor_tensor(
                                out=alt, in0=alt, scalar=NEG_SLOPE, in1=alt,
                                op0=ALU.mult, op1=ALU.max)
                            nc.scalar.activation(
                                out=pext,
                                in_=alt.rearrange("p j (h c) -> p j h c", c=1)
                                       .to_broadcast([128, D, nheads,
                                                      64 // nheads]),
                                func=AF.Exp)
                            nc.vector.tensor_reduce(
                                out=dn,
                                in_=pext.rearrange("p j (h c) -> p h c j",
                                                   h=nheads)[:, :, 0, :],
                                axis=AX.X, op=ALU.add)
                            nc.vector.tensor_tensor(
                                out=pext, in0=Gt[:, :, 0:64],
                                in1=pext, op=ALU.mult)
                        nc.vector.tensor_reduce(
                            out=nm, in_=pext.rearrange("p j f -> p f j"),
                            axis=AX.X, op=ALU.add)
                        if accum is None:
                            nc.sync.dma_start(
                                part[t * 128:(t + 1) * 128, :], res[:])
                        else:
                            per_tile_post(t, 128 if t < ntiles - 1 else tail)

            # ================= layer 1 =================
            if stop_after < 2:
                raise _StopBuild()
            p0_rest = p0_blocks[nb_half0:]
            ngr0 = len(cfg.groups[0])
            per_g = cdiv(len(p0_rest), max(ngr0 - 1, 1))

            def preA(gi_i):
                for (k0, nk) in p0_rest[gi_i * per_g:(gi_i + 1) * per_g]:
                    p0_block(k0, nk)

            grid_pass(1, 0, t1[0], gidx1_h, adst1[0], H, 64, None, part1,
                      None, pre_group=preA)

            if stop_after < 3:
                raise _StopBuild()

            def post1(t, rows):
                cg = cb.tile([128, 1, 128], DT.float32, tag="cg1")
                nc.gpsimd.dma_gather(cg[:], part1[:],
                                     cr1[:, t * 8:(t + 1) * 8], 128, 128, 128,
                                     single_packet=False)
                comb = cb.tile([128, 72], DT.float32, tag="comb1")
                nc.vector.tensor_tensor(out=comb[:], in0=cg[:, 0, 0:72],
                                        in1=accB1[:, t, :], op=ALU.add)
                rec = cb.tile([128, H], DT.float32, tag="rec1")
                nc.vector.tensor_scalar_add(rec[:], comb[:, 64:72], EPS)
                nc.vector.reciprocal(rec[:], rec[:])
                hf = cb.tile([128, 64], DT.float32, tag="hf")
                nc.vector.tensor_tensor(
                    out=hf[:].rearrange("p (h c) -> p h c", h=H),
                    in0=comb[:, 0:64].rearrange("p (h c) -> p h c", h=H),
                    in1=rec[:].rearrange("p (h c) -> p h c", c=1)
                              .to_broadcast([128, H, 8]),
                    op=ALU.mult)
                nc.vector.tensor_tensor(out=hf[:], in0=hf[:], in1=b1s[:],
                                        op=ALU.add)
                # elu(x) = relu(x) + exp(-relu(-x)) - 1
                r = cb.tile([128, 64], DT.float32, tag="r")
                nc.scalar.activation(out=r[:], in_=hf[:], func=AF.Relu)
                m = cb.tile([128, 64], DT.float32, tag="m")
                nc.scalar.activation(out=m[:], in_=hf[:], func=AF.Relu,
                                     scale=-1.0)
                e = cb.tile([128, 64], DT.float32, tag="e")
                nc.scalar.activation(out=e[:], in_=m[:], func=AF.Exp,
                                     scale=-1.0)
                nc.vector.tensor_tensor(out=r[:], in0=r[:], in1=e[:], op=ALU.add)
                nc.vector.tensor_scalar_add(r[:], r[:], -1.0)
                psT = cps.tile([64, 128], DT.float32, tag="psT", space="PSUM")
                nc.tensor.transpose(out=psT[:], in_=r[:, :], identity=ident[:])
                htb = cb.tile([64, 128], DT.bfloat16, tag="htb")
                nc.any.tensor_copy(out=htb[:], in_=psT[:])
                if t < ca_tiles:
                    nc.sync.dma_start(hT_shA[:, t * 128:t * 128 + rows],
                                      htb[:, :rows])
                else:
                    o = (t - ca_tiles) * 128
                    nc.sync.dma_start(hT_shB[:, o:o + rows], htb[:, :rows])
                psA = cps.tile([128, 1], DT.float32, tag="psA", space="PSUM")
                nc.tensor.matmul(psA[:], lhsT=htb[:], rhs=tgr[:, 65:66],
                                 start=True, stop=True)
                nc.any.tensor_copy(out=adst2B[:, t, :], in_=psA[:])
                adrow = cb.tile([128, 64], DT.float32, tag="adrow")
                nc.any.tensor_copy(out=adrow[:],
                                   in_=psA[:, 0:1].to_broadcast([128, 64]))
                nc.sync.dma_start(adst2sc[t * 128:(t + 1) * 128, :], adrow[:])
                if t == ca_tiles - 1:
                    nc.gpsimd.collective_compute(
                        "AllGather", ALU.bypass, ins=[hT_shA[:]],
                        outs=[hT_fullA[:]],
                        replica_groups=[list(range(cfg.ncores))])

            def t2_block(s2, kc4, u0, nu):
                kc = s2 * 4 + kc4
                u_end = u0 + nu
                cols = (min(u_end * 128, nloc)) - u0 * 128
                hts = p3.tile([d1, 4 * 128], DT.bfloat16, tag="hts")
                if u_end <= ca_tiles:
                    nc.scalar.dma_start(
                        hts[:, :cols],
                        hT_fullA[kc * d1:(kc + 1) * d1,
                                 u0 * 128:u0 * 128 + cols])
                else:
                    o = (u0 - ca_tiles) * 128
                    nc.scalar.dma_start(
                        hts[:, :cols],
                        hT_fullB[kc * d1:(kc + 1) * d1, o:o + cols])
                row = p3.tile([128, 4, 66], DT.bfloat16, tag="row2")
                for i in range(nu):
                    c = min(128, nloc - (u0 + i) * 128)
                    ps = p3ps.tile([128, 128], DT.float32, tag="ps2",
                                   space="PSUM")
                    nc.tensor.matmul(ps[:c, :],
                                     lhsT=hts[:, i * 128:i * 128 + c],
                                     rhs=tgr[:], start=True, stop=True)
                    nc.vector.tensor_copy(out=row[:c, i, :],
                                          in_=ps[:c, 0:66])
                ro = kc4 * nloc + u0 * 128
                if True:
                    for i in range(nu):
                        c = min(128, nloc - (u0 + i) * 128)
                        nc.scalar.dma_start(
                            t2[s2][ro + i * 128:ro + i * 128 + c, 0:66],
                            row[:c, i, :])

            # chunk-A u-blocks (never straddle the ca_tiles boundary)
            t2a_blocks, t2b_blocks = [], []
            u = 0
            while u < ca_tiles:
                nu = min(4, ca_tiles - u)
                t2a_blocks += [(s2, kc4, u, nu)
                               for s2 in range(2) for kc4 in range(4)]
                u += nu
            while u < ntiles:
                nu = min(4, ntiles - u)
                t2b_blocks += [(s2, kc4, u, nu)
                               for s2 in range(2) for kc4 in range(4)]
                u += nu

            # sprinkle t2 chunk-A builds into the tail groups of pass B
            ngr1 = len(cfg.groups[1])
            nspr = 4
            spr_start = ngr1 - nspr
            per_g2 = cdiv(len(t2a_blocks), nspr)

            def preB(gi_i):
                if gi_i >= spr_start:
                    i = gi_i - spr_start
                    for (s2, kc4, u0, nu) in \
                            t2a_blocks[i * per_g2:(i + 1) * per_g2]:
                        t2_block(s2, kc4, u0, nu)

            grid_pass(1, 1, t1[1], gidx1_h, adst1[1], H, 64, accB1, None,
                      post1, pre_group=preB)

            nc.gpsimd.collective_compute(
                "AllGather", ALU.bypass, ins=[hT_shB[:]], outs=[hT_fullB[:]],
                replica_groups=[list(range(cfg.ncores))])

            # adst2 in pass-0 order (gpsimd idle during AllGather B)
            ga = cpool.tile([128, ntiles, 64], DT.float32, tag="ga")
            nc.gpsimd.dma_gather(ga[:], adst2sc[:], a2ai[:], NRP, NRP, 64,
                                 single_packet=False)
            nc.vector.tensor_copy(out=adst2A[:, :, 0], in_=ga[:, :, 0])

            # ================= t2 build (chunk B; A was sprinkled) =========
            if stop_after < 4:
                raise _StopBuild()
            for (s2, kc4, u0, nu) in t2b_blocks:
                t2_block(s2, kc4, u0, nu)

            # ================= layer 2 =================
            if stop_after < 5:
                raise _StopBuild()
            grid_pass(2, 0, t2[0], gidx2_h, adst2A[:], 1, 64,
                      None, part2, None)

            if stop_after < 6:
                raise _StopBuild()

            def post2(t, rows):
                cg = cb.tile([128, 1, 128], DT.float32, tag="cg2")
                nc.gpsimd.dma_gather(cg[:], part2[:],
                                     cr1[:, t * 8:(t + 1) * 8], 128, 128, 128,
                                     single_packet=False)
                comb = cb.tile([128, 65], DT.float32, tag="comb2")
                nc.vector.tensor_tensor(out=comb[:], in0=cg[:, 0, 0:65],
                                        in1=accB2[:, t, :], op=ALU.add)
                rec = cb.tile([128, 1], DT.float32, tag="rec2")
                nc.vector.tensor_scalar_add(rec[:], comb[:, 64:65], EPS)
                nc.vector.reciprocal(rec[:], rec[:])
                gg = cb.tile([128, 64], DT.float32, tag="gg")
                nc.vector.tensor_tensor(
                    out=gg[:], in0=comb[:, 0:64],
                    in1=rec[:].to_broadcast([128, 64]), op=ALU.mult)
                psT = cps.tile([64, 128], DT.float32, tag="psT2", space="PSUM")
                nc.tensor.transpose(out=psT[:], in_=gg[:, :], identity=ident[:])
                gtb = cb.tile([64, 128], DT.bfloat16, tag="gtb")
                nc.any.tensor_copy(out=gtb[:], in_=psT[:])
                ps2 = cps.tile([128, C2], DT.float32, tag="ps22", space="PSUM")
                nc.tensor.matmul(ps2[:], lhsT=gtb[:], rhs=w2t[:],
                                 start=True, stop=True)
                o2 = cb.tile([128, C2], DT.float32, tag="o2")
                nc.vector.tensor_tensor(out=o2[:], in0=ps2[:], in1=b2s[:],
                                        op=ALU.add)
                nc.sync.dma_start(out_h[t * 128:t * 128 + 128, :], o2[:])

            grid_pass(2, 1, t2[1], gidx2_h, adst2B[:], 1, 64,
                      accB2, None, post2)

    except _StopBuild:
        pass
    nc.compile()
    return nc


# ----------------------------------------------------------------------------
# Host entry
# ----------------------------------------------------------------------------
def host_inputs(cfg, x, W1, att_src1, att_dst1, bias1, W2, att_src2,
                att_dst2, bias2):
    import ml_dtypes
    H, C1, C2, d1 = cfg.H, cfg.C1, cfg.C2, cfg.d1
    x = np.asarray(x, np.float32)
    W1 = np.asarray(W1, np.float32)
    A_src = np.zeros((d1, H), np.float32)
    A_dst = np.zeros((d1, H), np.float32)
    for h in range(H):
        A_src[h * C1:(h + 1) * C1, h] = np.asarray(att_src1, np.float32)[h]
        A_dst[h * C1:(h + 1) * C1, h] = np.asarray(att_dst1, np.float32)[h]
    w1aug = np.zeros((cfg.F, 128), np.float32)
    w1aug[:, :d1] = W1
    w1aug[:, d1:d1 + H] = W1 @ A_src
    cfg.adst1_full = x @ (W1 @ A_dst)

    W2 = np.asarray(W2, np.float32)
    tgr = np.zeros((d1, 128), np.float32)
    tgr[:, :d1] = np.eye(d1)
    tgr[:, d1:d1 + 1] = W2 @ np.asarray(att_src2, np.float32).T
    tgr[:, d1 + 1:d1 + 2] = W2 @ np.asarray(att_dst2, np.float32).T

    sent1 = np.zeros((1, 256), np.float32)
    sent1[0, d1:d1 + 2 * H] = SENT_VAL
    sent2 = np.zeros((1, 256), np.float32)
    sent2[0, d1:d1 + 2] = SENT_VAL

    common = {
        "xT": np.ascontiguousarray(x.T),
        "w1aug": w1aug,
        "tgr": tgr.astype(ml_dtypes.bfloat16),
        "w2t": W2.astype(ml_dtypes.bfloat16),
        "bias1r": np.tile(np.asarray(bias1, np.float32)[None, :], (128, 1)),
        "bias2r": np.tile(np.asarray(bias2, np.float32)[None, :], (128, 1)),
        "sent1": sent1.astype(ml_dtypes.bfloat16),
        "sent2": sent2.astype(ml_dtypes.bfloat16),
    }
    in_maps = []
    for c in range(cfg.ncores):
        r = build_routing(cfg, c)
        r["adst1p"] = r["adst1p"].astype(ml_dtypes.bfloat16)
        in_maps.append({**common, **r})
    return in_maps


_CACHE = {}


def _run(x, edge_index, W1, att_src1, att_dst1, bias1, W2, att_src2,
         att_dst2, bias2, **run_kwargs):
    x = np.asarray(x, dtype=np.float32)
    N, F = x.shape
    ei = np.asarray(edge_index).astype(np.int64)
    E = ei.shape[1]
    loops = np.arange(N, dtype=np.int64)
    src = np.concatenate([ei[0], loops])
    dst = np.concatenate([ei[1], loops])
    cfg = Cfg(N, F, E, src, dst)
    key = (N, F, E, tuple(cfg.D[0]), tuple(cfg.D[1]))
    if key not in _CACHE:
        _CACHE[key] = build_program(cfg)
    nc = _CACHE[key]
    in_maps = host_inputs(cfg, x, W1, att_src1, att_dst1, bias1,
                          W2, att_src2, att_dst2, bias2)
    res = run_bass_kernel_spmd(nc, in_maps, list(range(cfg.ncores)),
                               **run_kwargs)
    out = np.empty((N, cfg.C2), dtype=np.float32)
    for c in range(cfg.ncores):
        r = np.asarray(res.results[c]["out"], dtype=np.float32)
        out[c * cfg.nloc + cfg.cores[c]["order"][1]] = r[:cfg.nloc]
    return out, res


def kernel(x, edge_index, W1, att_src1, att_dst1, bias1, W2, att_src2,
           att_dst2, bias2):
    out, _ = _run(x, edge_index, W1, att_src1, att_dst1, bias1, W2,
                  att_src2, att_dst2, bias2)
    return out


# revision 20
# speedup vs baseline: 1.6049x; 1.0008x over previous
"""2-layer GAT kernel for Trainium2 (8 NeuronCores), Bass/Tile.  v3.

Sharding: nodes by dst across 8 cores; edges routed to the dst owner.
Per core, edges split into two passes by src half (dma_gather idx is int16
-> gather tables limited to <=32768 rows).  Per pass, dst nodes are sorted
by per-pass degree and packed into 128-partition tiles with DATA-DEPENDENT
exact slot budgets D[s][t] (max over cores; program compiled per budget
vector).  Edge j of dst node d sits at (partition rank%128, tile rank//128,
slot j).  Pad slots point at a sentinel table row (a_src = -1e4 => p = 0).

Both layers share the SAME slot grids (same edges, same orders):
  gidx1[slot] = src id within its half (layer-1 table row)
  gidx2[slot] = global pass-order position of src (layer-2 table row)

Layer tables:
  t1[s] = [x @ W1 | x @ W1 @ Asrc] rows (f32, 512B), built on-device from
    host-pre-transposed x; a_dst1 comes host-computed+permuted (adst1p).
  t2[s] = [g | g@(W2 a_src2^T) | g@(W2 a_dst2^T)] rows (f32, 512B) where
    g = elu(out1 + b1).  h^T (bf16, pass-1-order) is AllGathered in two
    column chunks; t2 rows are one matmul per tile vs [I | w2a] rhs.
    Layer-2 aggregates 64-wide g; W2 is applied after normalization
    (out2 = (gagg/den) @ W2 + b2), valid because W2 is linear.

Per pass: pass 0 writes per-tile partial rows [num|den] to HBM; pass 1
reduces directly into an SBUF accumulator and immediately combines each
tile: gather the pass-0 partial rows for the tile's nodes (cross-rank
permutation), add, normalize.  Layer-1 combine also emits h^T columns and
a_dst2; layer-2 combine applies W2+bias and stores output rows in pass-1
order; the host un-permutes rows at the end.
"""

import numpy as np


class _StopBuild(Exception):
    pass


import concourse.bacc as bacc
import concourse.bass as bass
import concourse.mybir as mybir
import concourse.tile as tile
from concourse._compat import cdiv
from concourse.bass_utils import run_bass_kernel_spmd

AF = mybir.ActivationFunctionType
ALU = mybir.AluOpType
AX = mybir.AxisListType
DT = mybir.dt

NEG_SLOPE = 0.2
EPS = 1e-16
SENT_VAL = -1e4
GC_TARGET = 64


def _wrap_idx(idx):
    """[n] -> [128, n/16] int16: position j -> (partition j%16, col j//16),
    replicated across the 8 groups of 16 partitions."""
    idx = np.asarray(idx, dtype=np.int16)
    assert len(idx) % 16 == 0
    return np.tile(idx.reshape(-1, 16).T, (8, 1))


# ----------------------------------------------------------------------------
# Configuration + host routing (data-dependent)
# ----------------------------------------------------------------------------
class Cfg:
    def __init__(self, N, F, E, src, dst):
        ncores = 8
        self.N, self.F, self.E, self.ncores = N, F, E, ncores
        self.H, self.C1, self.C2 = 8, 8, 128
        self.d1 = 64
        self.nloc = N // ncores          # 6250
        self.half = N // 2               # 25000
        self.ntiles = cdiv(self.nloc, 128)   # 49
        self.nrp = self.ntiles * 128         # 6272
        nloc, half, ntiles = self.nloc, self.half, self.ntiles

        # ---- per-core routing part 1: degrees / orders ----
        self.cores = []
        for c in range(ncores):
            base = c * nloc
            m = (dst >= base) & (dst < base + nloc)
            s_c = src[m]
            d_c = (dst[m] - base).astype(np.int64)
            info = {"s": s_c, "d": d_c, "deg": [], "order": [], "rank": []}
            for s in (0, 1):
                m2 = (s_c // half) == s
                deg = np.bincount(d_c[m2], minlength=nloc)
                order = np.argsort(-deg, kind="stable")
                rank = np.empty(nloc, dtype=np.int64)
                rank[order] = np.arange(nloc)
                info["deg"].append(deg)
                info["order"].append(order)
                info["rank"].append(rank)
            self.cores.append(info)

        # global pass-1-order position of every node (for gidx2 / hT layout)
        self.rank1_global = np.empty(N, dtype=np.int64)
        for c in range(ncores):
            self.rank1_global[c * nloc:(c + 1) * nloc] = self.cores[c]["rank"][1]

        # ---- shared exact budgets D[s][t] = max over cores of tile max ----
        self.D = []
        for s in (0, 1):
            mx = np.zeros(ntiles, dtype=np.int64)
            for c in range(ncores):
                sd = np.sort(self.cores[c]["deg"][s])[::-1]
                pad = np.zeros(ntiles * 128, dtype=np.int64)
                pad[:nloc] = sd
                mx = np.maximum(mx, pad.reshape(ntiles, 128).max(axis=1))
            self.D.append(np.maximum(mx, 1))
        self.col_off = [np.concatenate([[0], np.cumsum(D)]).astype(int)
                        for D in self.D]
        self.total_cols = [int(D.sum()) for D in self.D]

        # ---- group packing (cap GC_TARGET cols per gather) ----
        self.groups = []
        for s in (0, 1):
            gs, t = [], 0
            while t < ntiles:
                t0, c0 = t, int(self.col_off[s][t])
                cols = 0
                while t < ntiles and (cols == 0
                                      or cols + self.D[s][t] <= GC_TARGET):
                    cols += int(self.D[s][t])
                    t += 1
                gs.append((t0, t, c0, cols))
            self.groups.append(gs)
        self.GC = max(g[3] for gs in self.groups for g in gs)


def build_routing(cfg, core):
    """Per-core runtime index arrays."""
    nloc, half, ntiles, nrp = cfg.nloc, cfg.half, cfg.ntiles, cfg.nrp
    info = cfg.cores[core]
    s_c, d_c = info["s"], info["d"]
    g1, g2, adst1p = [], [], []
    for s in (0, 1):
        m2 = (s_c // half) == s
        ss = s_c[m2]                       # global src ids
        dd = d_c[m2]
        deg = info["deg"][s]
        rank = info["rank"][s]
        eo = np.lexsort((ss, dd))
        ss_o, dd_o = ss[eo], dd[eo]
        starts = np.concatenate([[0], np.cumsum(deg)])
        j = np.arange(len(dd_o)) - starts[dd_o]
        r = rank[dd_o]
        tile_e, row_e = r // 128, r % 128
        Dv = cfg.D[s]
        assert (j < Dv[tile_e]).all(), "slot budget overflow (exact budgets)"
        flat1 = np.full(cfg.total_cols[s] * 128, half, dtype=np.int64)
        flat1[(cfg.col_off[s][tile_e] + j) * 128 + row_e] = ss_o - s * half
        g1.append(flat1)
        # layer-2 positions: owner-core pass-1 rank, table offset by half
        pos = (ss_o // nloc - 4 * s) * nloc + cfg.rank1_global[ss_o]
        flat2 = np.full(cfg.total_cols[s] * 128, half, dtype=np.int64)
        flat2[(cfg.col_off[s][tile_e] + j) * 128 + row_e] = pos
        g2.append(flat2)
        a = np.full((nrp, cfg.H), SENT_VAL, dtype=np.float32)
        a[:nloc] = cfg.adst1_full[core * nloc + info["order"][s]]
        adst1p.append(a)
    # cross: pass-1-order position j -> pass-0 partial row
    cross = np.zeros(nrp, dtype=np.int64)
    cross[:nloc] = info["rank"][0][info["order"][1]]
    # adst2 A-order: pass-0 position i -> pass-1 staged row
    a2ai = np.zeros(nrp, dtype=np.int64)
    a2ai[:nloc] = info["rank"][1][info["order"][0]]
    return {
        "gidx1": _wrap_idx(np.concatenate(g1)),
        "gidx2": _wrap_idx(np.concatenate(g2)),
        "adst1p": np.concatenate(adst1p, axis=0),
        "cross1": _wrap_idx(cross),
        "a2ai": _wrap_idx(a2ai),
    }


# ----------------------------------------------------------------------------
# Device program
# ----------------------------------------------------------------------------
def build_program(cfg, stop_after=99):
    from concourse.masks import make_identity

    nc = bacc.Bacc(None, target_bir_lowering=False, debug=True)
    H, d1, C2, F = cfg.H, cfg.d1, cfg.C2, cfg.F
    nloc, half, ntiles = cfg.nloc, cfg.half, cfg.ntiles
    nfull_tiles = cdiv(cfg.N, 128)
    NRP, GC = cfg.nrp, cfg.GC
    tail = nloc - (ntiles - 1) * 128
    ca_tiles = 33                       # hT AllGather chunk A: tiles 0..32
    ca_cols = ca_tiles * 128            # 3200
    cb_cols = nloc - ca_cols            # 3050

    # ---- external IO ----
    xT = nc.dram_tensor("xT", [F, cfg.N], DT.float32, kind="ExternalInput")
    w1aug_h = nc.dram_tensor("w1aug", [F, 128], DT.float32, kind="ExternalInput")
    tgr_h = nc.dram_tensor("tgr", [d1, 128], DT.bfloat16, kind="ExternalInput")
    w2t_h = nc.dram_tensor("w2t", [d1, C2], DT.bfloat16, kind="ExternalInput")
    b1_h = nc.dram_tensor("bias1r", [128, d1], DT.float32, kind="ExternalInput")
    b2_h = nc.dram_tensor("bias2r", [128, C2], DT.float32, kind="ExternalInput")
    sent1_h = nc.dram_tensor("sent1", [1, 256], DT.bfloat16, kind="ExternalInput")
    sent2_h = nc.dram_tensor("sent2", [1, 256], DT.bfloat16, kind="ExternalInput")
    tc01 = cfg.total_cols[0] + cfg.total_cols[1]
    gidx1_h = nc.dram_tensor("gidx1", [128, tc01 * 8], DT.int16, kind="ExternalInput")
    gidx2_h = nc.dram_tensor("gidx2", [128, tc01 * 8], DT.int16, kind="ExternalInput")
    adst1p_h = nc.dram_tensor("adst1p", [2 * NRP, H], DT.bfloat16, kind="ExternalInput")
    cross1_h = nc.dram_tensor("cross1", [128, NRP // 16], DT.int16, kind="ExternalInput")
    a2ai_h = nc.dram_tensor("a2ai", [128, NRP // 16], DT.int16, kind="ExternalInput")
    out_h = nc.dram_tensor("out", [NRP, C2], DT.float32, kind="ExternalOutput")

    # ---- internal DRAM ----
    t1 = [nc.dram_tensor(f"t1_{s}", [half + 1, 256], DT.bfloat16) for s in range(2)]
    t2 = [nc.dram_tensor(f"t2_{s}", [half + 1, 256], DT.bfloat16) for s in range(2)]
    part1 = nc.dram_tensor("part1", [NRP, 128], DT.float32)
    part2 = nc.dram_tensor("part2", [NRP, 128], DT.float32)
    adst2sc = nc.dram_tensor("adst2sc", [NRP, 64], DT.float32)
    hT_shA = nc.dram_tensor("hT_shA", [d1, ca_cols], DT.bfloat16)
    hT_shB = nc.dram_tensor("hT_shB", [d1, cb_cols], DT.bfloat16)
    hT_fullA = nc.dram_tensor("hT_fullA", [cfg.ncores * d1, ca_cols], DT.bfloat16)
    hT_fullB = nc.dram_tensor("hT_fullB", [cfg.ncores * d1, cb_cols], DT.bfloat16)

    try:
      with tile.TileContext(nc) as tc:
        with tc.tile_pool(name="const", bufs=1) as cpool, \
             tc.tile_pool(name="p0", bufs=4) as p0, \
             tc.tile_pool(name="p0ps", bufs=2, space="PSUM") as p0ps, \
             tc.tile_pool(name="pass", bufs=3) as pp, \
             tc.tile_pool(name="px", bufs=2) as pxp, \
             tc.tile_pool(name="gi", bufs=4) as gip, \
             tc.tile_pool(name="cmb", bufs=2) as cb, \
             tc.tile_pool(name="cps", bufs=1, space="PSUM") as cps, \
             tc.tile_pool(name="p3", bufs=4) as p3, \
             tc.tile_pool(name="p3ps", bufs=2, space="PSUM") as p3ps:
            w1s = cpool.tile([F, 128], DT.float32)
            nc.sync.dma_start(w1s[:], w1aug_h[:])
            tgr = cpool.tile([d1, 128], DT.bfloat16)
            nc.sync.dma_start(tgr[:], tgr_h[:])
            w2t = cpool.tile([d1, C2], DT.bfloat16)
            nc.sync.dma_start(w2t[:], w2t_h[:])
            b1s = cpool.tile([128, d1], DT.float32)
            nc.sync.dma_start(b1s[:], b1_h[:])
            b2s = cpool.tile([128, C2], DT.float32)
            nc.sync.dma_start(b2s[:], b2_h[:])
            ident = cpool.tile([128, 128], DT.float32)
            make_identity(nc, ident[:])
            cr1 = cpool.tile([128, NRP // 16], DT.int16)
            nc.sync.dma_start(cr1[:], cross1_h[:])
            a2ai = cpool.tile([128, NRP // 16], DT.int16)
            nc.sync.dma_start(a2ai[:], a2ai_h[:])
            adst1 = []
            for s in range(2):
                a = cpool.tile([128, ntiles, H], DT.bfloat16, tag=f"adst1_{s}")
                nc.sync.dma_start(
                    a[:],
                    adst1p_h[s * NRP:(s + 1) * NRP, :]
                    .rearrange("(t p) h -> p t h", p=128))
                adst1.append(a)
            accB1 = cpool.tile([128, ntiles, 72], DT.float32)
            accB2 = cpool.tile([128, ntiles, 65], DT.float32)
            adst2B = cpool.tile([128, ntiles, 1], DT.bfloat16)
            adst2A = cpool.tile([128, ntiles, 1], DT.bfloat16)

            # sentinel rows
            sc1 = cpool.tile([1, 256], DT.bfloat16, tag="sent1")
            nc.sync.dma_start(sc1[:], sent1_h[:])
            sc2 = cpool.tile([1, 256], DT.bfloat16, tag="sent2")
            nc.sync.dma_start(sc2[:], sent2_h[:])
            for s in range(2):
                nc.sync.dma_start(t1[s][half:half + 1, :], sc1[:])
                nc.sync.dma_start(t2[s][half:half + 1, :], sc2[:])

            # ================= P0: layer-1 table =================
            if stop_after < 1:
                raise _StopBuild()

            def p0_block(k0, nk):
                n0 = k0 * 128
                cnt = min(nk * 128, cfg.N - n0)
                xt_t = p0.tile([F, 4 * 128], DT.float32, tag="xt")
                nc.scalar.dma_start(xt_t[:, :cnt], xT[:, n0:n0 + cnt])
                row = p0.tile([128, 4, 72], DT.bfloat16, tag="row")
                for i in range(nk):
                    c = min(128, cfg.N - n0 - i * 128)
                    if c <= 0:
                        break
                    ps = p0ps.tile([128, 128], DT.float32, tag="ps",
                                   space="PSUM")
                    nc.tensor.matmul(ps[:c, :],
                                     lhsT=xt_t[:, i * 128:i * 128 + c],
                                     rhs=w1s[:], start=True, stop=True)
                    nc.vector.tensor_copy(out=row[:c, i, :],
                                          in_=ps[:c, 0:72])
                if True:
                    for i in range(nk):
                        m0 = n0 + i * 128
                        c = min(128, cfg.N - m0)
                        if c <= 0:
                            break
                        for s in range(2):
                            lo, hi = max(m0, s * half), min(m0 + c,
                                                           (s + 1) * half)
                            if lo < hi:
                                nc.sync.dma_start(
                                    t1[s][lo - s * half:hi - s * half, 0:72],
                                    row[lo - m0:hi - m0, i, :])

            half0_tiles = half // 128 + 1          # tiles covering src half 0
            p0_blocks = [(k, min(4, nfull_tiles - k))
                         for k in range(0, nfull_tiles, 4)]
            nb_half0 = (half0_tiles + 3) // 4
            for (k0, nk) in p0_blocks[:nb_half0]:
                p0_block(k0, nk)

            # ================= pass machinery =================
            def grid_pass(layer, s, tbl, gidx_h, adst_tile, nheads, dfeat,
                          accum, part, per_tile_post, pre_group=None):
                """One slot-grid pass.  accum=None: write partial rows to
                `part` (pass 0).  accum=tile: reduce into SBUF (pass 1) and
                call per_tile_post(t, rows) after each tile."""
                base8 = (cfg.total_cols[0] if s == 1 else 0) * 8
                for gidx_i, (t0, t1_, c0, ncols) in enumerate(cfg.groups[s]):
                    if pre_group is not None:
                        pre_group(gidx_i)
                    gi = gip.tile([128, GC * 8], DT.int16, tag="gi")
                    nc.sync.dma_start(
                        gi[:, :ncols * 8],
                        gidx_h[:, base8 + c0 * 8: base8 + (c0 + ncols) * 8])
                    G = pp.tile([128, GC, 256], DT.bfloat16, tag="G")
                    nc.gpsimd.dma_gather(G[:, :ncols, :], tbl[:],
                                         gi[:, :ncols * 8], ncols * 128,
                                         ncols * 128, 256, single_packet=False)
                    pex = pxp.tile([128, GC, 64], DT.bfloat16, tag="px")
                    for t in range(t0, t1_):
                        D = int(cfg.D[s][t])
                        o = int(cfg.col_off[s][t]) - c0
                        Gt = G[:, o:o + D, :]
                        asrc = Gt[:, :, 64:64 + nheads]
                        if accum is None:
                            res = pp.tile([128, 128], DT.float32, tag="res")
                            nc.any.memset(res[:, 64 + nheads:], 0.0)
                            dn = res[:, 64:64 + nheads]
                            nm = res[:, 0:64]
                        else:
                            dn = accum[:, t, 64:64 + nheads]
                            nm = accum[:, t, 0:64]
                        pext = pex[:, o:o + D, :]
                        if nheads == 1:
                            # alpha+lrelu+exp+den all on the Act engine
                            p2 = pp.tile([128, GC, 1], DT.bfloat16, tag="p2")
                            nc.scalar.activation(
                                out=p2[:, :D, :], in_=asrc, func=AF.Lrelu,
                                bias=adst_tile[:, t, :], alpha=NEG_SLOPE)
                            nc.scalar.activation(
                                out=p2[:, :D, :], in_=p2[:, :D, :],
                                func=AF.Exp, accum_out=dn)
                            nc.vector.tensor_tensor(
                                out=pext, in0=Gt[:, :, 0:64],
                                in1=p2[:, :D, :].to_broadcast([128, D, 64]),
                                op=ALU.mult)
                        else:
                            al = pp.tile([128, GC, 8], DT.bfloat16, tag="al")
                            alt = al[:, :D, :nheads]
                            nc.vector.tensor_tensor(
                                out=alt, in0=asrc,
                                in1=adst_tile[:, t:t + 1, :]
                                    .to_broadcast([128, D, nheads]),
                                op=ALU.add)
                            nc.vector.scalar_tensor_tensor(
                                out=alt, in0=alt, scalar=NEG_SLOPE, in1=alt,
                                op0=ALU.mult, op1=ALU.max)
                            nc.scalar.activation(
                                out=pext,
                                in_=alt.rearrange("p j (h c) -> p j h c", c=1)
                                       .to_broadcast([128, D, nheads,
                                                      64 // nheads]),
                                func=AF.Exp)
                            nc.vector.tensor_reduce(
                                out=dn,
                                in_=pext.rearrange("p j (h c) -> p h c j",
                                                   h=nheads)[:, :, 0, :],
                                axis=AX.X, op=ALU.add)
                            nc.vector.tensor_tensor(
                                out=pext, in0=Gt[:, :, 0:64],
                                in1=pext, op=ALU.mult)
                        nc.vector.tensor_reduce(
                            out=nm, in_=pext.rearrange("p j f -> p f j"),
                            axis=AX.X, op=ALU.add)
                        if accum is None:
                            nc.sync.dma_start(
                                part[t * 128:(t + 1) * 128, :], res[:])
                        else:
                            per_tile_post(t, 128 if t < ntiles - 1 else tail)

            # ================= layer 1 =================
            if stop_after < 2:
                raise _StopBuild()
            p0_rest = p0_blocks[nb_half0:]
            ngr0 = len(cfg.groups[0])
            per_g = cdiv(len(p0_rest), max(ngr0 - 1, 1))

            def preA(gi_i):
                for (k0, nk) in p0_rest[gi_i * per_g:(gi_i + 1) * per_g]:
                    p0_block(k0, nk)

            grid_pass(1, 0, t1[0], gidx1_h, adst1[0], H, 64, None, part1,
                      None, pre_group=preA)

            if stop_after < 3:
                raise _StopBuild()

            def post1(t, rows):
                cg = cb.tile([128, 1, 128], DT.float32, tag="cg1")
                nc.gpsimd.dma_gather(cg[:], part1[:],
                                     cr1[:, t * 8:(t + 1) * 8], 128, 128, 128,
                                     single_packet=False)
                comb = cb.tile([128, 72], DT.float32, tag="comb1")
                nc.vector.tensor_tensor(out=comb[:], in0=cg[:, 0, 0:72],
                                        in1=accB1[:, t, :], op=ALU.add)
                rec = cb.tile([128, H], DT.float32, tag="rec1")
                nc.vector.tensor_scalar_add(rec[:], comb[:, 64:72], EPS)
                nc.vector.reciprocal(rec[:], rec[:])
                hf = cb.tile([128, 64], DT.float32, tag="hf")
                nc.vector.tensor_tensor(
                    out=hf[:].rearrange("p (h c) -> p h c", h=H),
                    in0=comb[:, 0:64].rearrange("p (h c) -> p h c", h=H),
                    in1=rec[:].rearrange("p (h c) -> p h c", c=1)
                              .to_broadcast([128, H, 8]),
                    op=ALU.mult)
                nc.vector.tensor_tensor(out=hf[:], in0=hf[:], in1=b1s[:],
                                        op=ALU.add)
                # elu(x) = relu(x) + exp(-relu(-x)) - 1
                r = cb.tile([128, 64], DT.float32, tag="r")
                nc.scalar.activation(out=r[:], in_=hf[:], func=AF.Relu)
                m = cb.tile([128, 64], DT.float32, tag="m")
                nc.scalar.activation(out=m[:], in_=hf[:], func=AF.Relu,
                                     scale=-1.0)
                e = cb.tile([128, 64], DT.float32, tag="e")
                nc.scalar.activation(out=e[:], in_=m[:], func=AF.Exp,
                                     scale=-1.0)
                nc.vector.tensor_tensor(out=r[:], in0=r[:], in1=e[:], op=ALU.add)
                nc.vector.tensor_scalar_add(r[:], r[:], -1.0)
                psT = cps.tile([64, 128], DT.float32, tag="psT", space="PSUM")
                nc.tensor.transpose(out=psT[:], in_=r[:, :], identity=ident[:])
                htb = cb.tile([64, 128], DT.bfloat16, tag="htb")
                nc.any.tensor_copy(out=htb[:], in_=psT[:])
                if t < ca_tiles:
                    nc.sync.dma_start(hT_shA[:, t * 128:t * 128 + rows],
                                      htb[:, :rows])
                else:
                    o = (t - ca_tiles) * 128
                    nc.sync.dma_start(hT_shB[:, o:o + rows], htb[:, :rows])
                psA = cps.tile([128, 1], DT.float32, tag="psA", space="PSUM")
                nc.tensor.matmul(psA[:], lhsT=htb[:], rhs=tgr[:, 65:66],
                                 start=True, stop=True)
                nc.any.tensor_copy(out=adst2B[:, t, :], in_=psA[:])
                adrow = cb.tile([128, 64], DT.float32, tag="adrow")
                nc.any.tensor_copy(out=adrow[:],
                                   in_=psA[:, 0:1].to_broadcast([128, 64]))
                nc.sync.dma_start(adst2sc[t * 128:(t + 1) * 128, :], adrow[:])
                if t == ca_tiles - 1:
                    nc.gpsimd.collective_compute(
                        "AllGather", ALU.bypass, ins=[hT_shA[:]],
                        outs=[hT_fullA[:]],
                        replica_groups=[list(range(cfg.ncores))])

            def t2_block(s2, kc4, u0, nu):
                kc = s2 * 4 + kc4
                u_end = u0 + nu
                cols = (min(u_end * 128, nloc)) - u0 * 128
                hts = p3.tile([d1, 4 * 128], DT.bfloat16, tag="hts")
                if u_end <= ca_tiles:
                    nc.scalar.dma_start(
                        hts[:, :cols],
                        hT_fullA[kc * d1:(kc + 1) * d1,
                                 u0 * 128:u0 * 128 + cols])
                else:
                    o = (u0 - ca_tiles) * 128
                    nc.scalar.dma_start(
                        hts[:, :cols],
                        hT_fullB[kc * d1:(kc + 1) * d1, o:o + cols])
                row = p3.tile([128, 4, 66], DT.bfloat16, tag="row2")
                for i in range(nu):
                    c = min(128, nloc - (u0 + i) * 128)
                    ps = p3ps.tile([128, 128], DT.float32, tag="ps2",
                                   space="PSUM")
                    nc.tensor.matmul(ps[:c, :],
                                     lhsT=hts[:, i * 128:i * 128 + c],
                                     rhs=tgr[:], start=True, stop=True)
                    nc.vector.tensor_copy(out=row[:c, i, :],
                                          in_=ps[:c, 0:66])
                ro = kc4 * nloc + u0 * 128
                if True:
                    for i in range(nu):
                        c = min(128, nloc - (u0 + i) * 128)
                        nc.sync.dma_start(
                            t2[s2][ro + i * 128:ro + i * 128 + c, 0:66],
                            row[:c, i, :])

            # chunk-A u-blocks (never straddle the ca_tiles boundary)
            t2a_blocks, t2b_blocks = [], []
            u = 0
            while u < ca_tiles:
                nu = min(4, ca_tiles - u)
                t2a_blocks += [(s2, kc4, u, nu)
                               for s2 in range(2) for kc4 in range(4)]
                u += nu
            while u < ntiles:
                nu = min(4, ntiles - u)
                t2b_blocks += [(s2, kc4, u, nu)
                               for s2 in range(2) for kc4 in range(4)]
                u += nu

            # sprinkle t2 chunk-A builds into the tail groups of pass B
            ngr1 = len(cfg.groups[1])
            nspr = 3
            spr_start = ngr1 - nspr
            per_g2 = cdiv(len(t2a_blocks), nspr)

            def preB(gi_i):
                if gi_i >= spr_start:
                    i = gi_i - spr_start
                    for (s2, kc4, u0, nu) in \
                            t2a_blocks[i * per_g2:(i + 1) * per_g2]:
                        t2_block(s2, kc4, u0, nu)

            grid_pass(1, 1, t1[1], gidx1_h, adst1[1], H, 64, accB1, None,
                      post1, pre_group=preB)

            nc.gpsimd.collective_compute(
                "AllGather", ALU.bypass, ins=[hT_shB[:]], outs=[hT_fullB[:]],
                replica_groups=[list(range(cfg.ncores))])

            # adst2 in pass-0 order (gpsimd idle during AllGather B)
            ga = cpool.tile([128, ntiles, 64], DT.float32, tag="ga")
            nc.gpsimd.dma_gather(ga[:], adst2sc[:], a2ai[:], NRP, NRP, 64,
                                 single_packet=False)
            nc.vector.tensor_copy(out=adst2A[:, :, 0], in_=ga[:, :, 0])

            # ================= t2 build (chunk B; A was sprinkled) =========
            if stop_after < 4:
                raise _StopBuild()
            for (s2, kc4, u0, nu) in t2b_blocks:
                t2_block(s2, kc4, u0, nu)

            # ================= layer 2 =================
            if stop_after < 5:
                raise _StopBuild()
            grid_pass(2, 0, t2[0], gidx2_h, adst2A[:], 1, 64,
                      None, part2, None)

            if stop_after < 6:
                raise _StopBuild()

            def post2(t, rows):
                cg = cb.tile([128, 1, 128], DT.float32, tag="cg2")
                nc.gpsimd.dma_gather(cg[:], part2[:],
                                     cr1[:, t * 8:(t + 1) * 8], 128, 128, 128,
                                     single_packet=False)
                comb = cb.tile([128, 65], DT.float32, tag="comb2")
                nc.vector.tensor_tensor(out=comb[:], in0=cg[:, 0, 0:65],
                                        in1=accB2[:, t, :], op=ALU.add)
                rec = cb.tile([128, 1], DT.float32, tag="rec2")
                nc.vector.tensor_scalar_add(rec[:], comb[:, 64:65], EPS)
                nc.vector.reciprocal(rec[:], rec[:])
                gg = cb.tile([128, 64], DT.float32, tag="gg")
                nc.vector.tensor_tensor(
                    out=gg[:], in0=comb[:, 0:64],
                    in1=rec[:].to_broadcast([128, 64]), op=ALU.mult)
                psT = cps.tile([64, 128], DT.float32, tag="psT2", space="PSUM")
                nc.tensor.transpose(out=psT[:], in_=gg[:, :], identity=ident[:])
                gtb = cb.tile([64, 128], DT.bfloat16, tag="gtb")
                nc.any.tensor_copy(out=gtb[:], in_=psT[:])
                ps2 = cps.tile([128, C2], DT.float32, tag="ps22", space="PSUM")
                nc.tensor.matmul(ps2[:], lhsT=gtb[:], rhs=w2t[:],
                                 start=True, stop=True)
                o2 = cb.tile([128, C2], DT.float32, tag="o2")
                nc.vector.tensor_tensor(out=o2[:], in0=ps2[:], in1=b2s[:],
                                        op=ALU.add)
                nc.sync.dma_start(out_h[t * 128:t * 128 + 128, :], o2[:])

            grid_pass(2, 1, t2[1], gidx2_h, adst2B[:], 1, 64,
                      accB2, None, post2)

    except _StopBuild:
        pass
    nc.compile()
    return nc


# ----------------------------------------------------------------------------
# Host entry
# ----------------------------------------------------------------------------
def host_inputs(cfg, x, W1, att_src1, att_dst1, bias1, W2, att_src2,
                att_dst2, bias2):
    import ml_dtypes
    H, C1, C2, d1 = cfg.H, cfg.C1, cfg.C2, cfg.d1
    x = np.asarray(x, np.float32)
    W1 = np.asarray(W1, np.float32)
    A_src = np.zeros((d1, H), np.float32)
    A_dst = np.zeros((d1, H), np.float32)
    for h in range(H):
        A_src[h * C1:(h + 1) * C1, h] = np.asarray(att_src1, np.float32)[h]
        A_dst[h * C1:(h + 1) * C1, h] = np.asarray(att_dst1, np.float32)[h]
    w1aug = np.zeros((cfg.F, 128), np.float32)
    w1aug[:, :d1] = W1
    w1aug[:, d1:d1 + H] = W1 @ A_src
    cfg.adst1_full = x @ (W1 @ A_dst)

    W2 = np.asarray(W2, np.float32)
    tgr = np.zeros((d1, 128), np.float32)
    tgr[:, :d1] = np.eye(d1)
    tgr[:, d1:d1 + 1] = W2 @ np.asarray(att_src2, np.float32).T
    tgr[:, d1 + 1:d1 + 2] = W2 @ np.asarray(att_dst2, np.float32).T

    sent1 = np.zeros((1, 256), np.float32)
    sent1[0, d1:d1 + 2 * H] = SENT_VAL
    sent2 = np.zeros((1, 256), np.float32)
    sent2[0, d1:d1 + 2] = SENT_VAL

    common = {
        "xT": np.ascontiguousarray(x.T),
        "w1aug": w1aug,
        "tgr": tgr.astype(ml_dtypes.bfloat16),
        "w2t": W2.astype(ml_dtypes.bfloat16),
        "bias1r": np.tile(np.asarray(bias1, np.float32)[None, :], (128, 1)),
        "bias2r": np.tile(np.asarray(bias2, np.float32)[None, :], (128, 1)),
        "sent1": sent1.astype(ml_dtypes.bfloat16),
        "sent2": sent2.astype(ml_dtypes.bfloat16),
    }
    in_maps = []
    for c in range(cfg.ncores):
        r = build_routing(cfg, c)
        r["adst1p"] = r["adst1p"].astype(ml_dtypes.bfloat16)
        in_maps.append({**common, **r})
    return in_maps


_CACHE = {}


def _run(x, edge_index, W1, att_src1, att_dst1, bias1, W2, att_src2,
         att_dst2, bias2, **run_kwargs):
    x = np.asarray(x, dtype=np.float32)
    N, F = x.shape
    ei = np.asarray(edge_index).astype(np.int64)
    E = ei.shape[1]
    loops = np.arange(N, dtype=np.int64)
    src = np.concatenate([ei[0], loops])
    dst = np.concatenate([ei[1], loops])
    cfg = Cfg(N, F, E, src, dst)
    key = (N, F, E, tuple(cfg.D[0]), tuple(cfg.D[1]))
    if key not in _CACHE:
        _CACHE[key] = build_program(cfg)
    nc = _CACHE[key]
    in_maps = host_inputs(cfg, x, W1, att_src1, att_dst1, bias1,
                          W2, att_src2, att_dst2, bias2)
    res = run_bass_kernel_spmd(nc, in_maps, list(range(cfg.ncores)),
                               **run_kwargs)
    out = np.empty((N, cfg.C2), dtype=np.float32)
    for c in range(cfg.ncores):
        r = np.asarray(res.results[c]["out"], dtype=np.float32)
        out[c * cfg.nloc + cfg.cores[c]["order"][1]] = r[:cfg.nloc]
    return out, res


def kernel(x, edge_index, W1, att_src1, att_dst1, bias1, W2, att_src2,
           att_dst2, bias2):
    out, _ = _run(x, edge_index, W1, att_src1, att_dst1, bias1, W2,
                  att_src2, att_dst2, bias2)
    return out


# revision 21
# speedup vs baseline: 1.6430x; 1.0237x over previous
"""2-layer GAT kernel for Trainium2 (8 NeuronCores), Bass/Tile.  v3.

Sharding: nodes by dst across 8 cores; edges routed to the dst owner.
Per core, edges split into two passes by src half (dma_gather idx is int16
-> gather tables limited to <=32768 rows).  Per pass, dst nodes are sorted
by per-pass degree and packed into 128-partition tiles with DATA-DEPENDENT
exact slot budgets D[s][t] (max over cores; program compiled per budget
vector).  Edge j of dst node d sits at (partition rank%128, tile rank//128,
slot j).  Pad slots point at a sentinel table row (a_src = -1e4 => p = 0).

Both layers share the SAME slot grids (same edges, same orders):
  gidx1[slot] = src id within its half (layer-1 table row)
  gidx2[slot] = global pass-order position of src (layer-2 table row)

Layer tables:
  t1[s] = [x @ W1 | x @ W1 @ Asrc] rows (f32, 512B), built on-device from
    host-pre-transposed x; a_dst1 comes host-computed+permuted (adst1p).
  t2[s] = [g | g@(W2 a_src2^T) | g@(W2 a_dst2^T)] rows (f32, 512B) where
    g = elu(out1 + b1).  h^T (bf16, pass-1-order) is AllGathered in two
    column chunks; t2 rows are one matmul per tile vs [I | w2a] rhs.
    Layer-2 aggregates 64-wide g; W2 is applied after normalization
    (out2 = (gagg/den) @ W2 + b2), valid because W2 is linear.

Per pass: pass 0 writes per-tile partial rows [num|den] to HBM; pass 1
reduces directly into an SBUF accumulator and immediately combines each
tile: gather the pass-0 partial rows for the tile's nodes (cross-rank
permutation), add, normalize.  Layer-1 combine also emits h^T columns and
a_dst2; layer-2 combine applies W2+bias and stores output rows in pass-1
order; the host un-permutes rows at the end.
"""

import numpy as np


class _StopBuild(Exception):
    pass


import concourse.bacc as bacc
import concourse.bass as bass
import concourse.mybir as mybir
import concourse.tile as tile
from concourse._compat import cdiv
from concourse.bass_utils import run_bass_kernel_spmd

AF = mybir.ActivationFunctionType
ALU = mybir.AluOpType
AX = mybir.AxisListType
DT = mybir.dt

NEG_SLOPE = 0.2
EPS = 1e-16
SENT_VAL = -1e4
GC_TARGET = 64


def _wrap_idx(idx):
    """[n] -> [128, n/16] int16: position j -> (partition j%16, col j//16),
    replicated across the 8 groups of 16 partitions."""
    idx = np.asarray(idx, dtype=np.int16)
    assert len(idx) % 16 == 0
    return np.tile(idx.reshape(-1, 16).T, (8, 1))


# ----------------------------------------------------------------------------
# Configuration + host routing (data-dependent)
# ----------------------------------------------------------------------------
class Cfg:
    def __init__(self, N, F, E, src, dst):
        ncores = 8
        self.N, self.F, self.E, self.ncores = N, F, E, ncores
        self.H, self.C1, self.C2 = 8, 8, 128
        self.d1 = 64
        self.nloc = N // ncores          # 6250
        self.half = N // 2               # 25000
        self.ntiles = cdiv(self.nloc, 128)   # 49
        self.nrp = self.ntiles * 128         # 6272
        nloc, half, ntiles = self.nloc, self.half, self.ntiles

        # ---- per-core routing part 1: degrees / orders ----
        self.cores = []
        for c in range(ncores):
            base = c * nloc
            m = (dst >= base) & (dst < base + nloc)
            s_c = src[m]
            d_c = (dst[m] - base).astype(np.int64)
            info = {"s": s_c, "d": d_c, "deg": [], "order": [], "rank": []}
            for s in (0, 1):
                m2 = (s_c // half) == s
                deg = np.bincount(d_c[m2], minlength=nloc)
                order = np.argsort(-deg, kind="stable")
                rank = np.empty(nloc, dtype=np.int64)
                rank[order] = np.arange(nloc)
                info["deg"].append(deg)
                info["order"].append(order)
                info["rank"].append(rank)
            self.cores.append(info)

        # global pass-1-order position of every node (for gidx2 / hT layout)
        self.rank1_global = np.empty(N, dtype=np.int64)
        for c in range(ncores):
            self.rank1_global[c * nloc:(c + 1) * nloc] = self.cores[c]["rank"][1]

        # ---- shared exact budgets D[s][t] = max over cores of tile max ----
        self.D = []
        for s in (0, 1):
            mx = np.zeros(ntiles, dtype=np.int64)
            for c in range(ncores):
                sd = np.sort(self.cores[c]["deg"][s])[::-1]
                pad = np.zeros(ntiles * 128, dtype=np.int64)
                pad[:nloc] = sd
                mx = np.maximum(mx, pad.reshape(ntiles, 128).max(axis=1))
            self.D.append(np.maximum(mx, 1))
        self.col_off = [np.concatenate([[0], np.cumsum(D)]).astype(int)
                        for D in self.D]
        self.total_cols = [int(D.sum()) for D in self.D]

        # ---- group packing (cap GC_TARGET cols per gather) ----
        self.groups = []
        for s in (0, 1):
            gs, t = [], 0
            while t < ntiles:
                t0, c0 = t, int(self.col_off[s][t])
                cols = 0
                while t < ntiles and (cols == 0
                                      or cols + self.D[s][t] <= GC_TARGET):
                    cols += int(self.D[s][t])
                    t += 1
                gs.append((t0, t, c0, cols))
            self.groups.append(gs)
        self.GC = max(g[3] for gs in self.groups for g in gs)


def build_routing(cfg, core):
    """Per-core runtime index arrays."""
    nloc, half, ntiles, nrp = cfg.nloc, cfg.half, cfg.ntiles, cfg.nrp
    info = cfg.cores[core]
    s_c, d_c = info["s"], info["d"]
    g1, g2, adst1p = [], [], []
    for s in (0, 1):
        m2 = (s_c // half) == s
        ss = s_c[m2]                       # global src ids
        dd = d_c[m2]
        deg = info["deg"][s]
        rank = info["rank"][s]
        eo = np.lexsort((ss, dd))
        ss_o, dd_o = ss[eo], dd[eo]
        starts = np.concatenate([[0], np.cumsum(deg)])
        j = np.arange(len(dd_o)) - starts[dd_o]
        r = rank[dd_o]
        tile_e, row_e = r // 128, r % 128
        Dv = cfg.D[s]
        assert (j < Dv[tile_e]).all(), "slot budget overflow (exact budgets)"
        flat1 = np.full(cfg.total_cols[s] * 128, half, dtype=np.int64)
        flat1[(cfg.col_off[s][tile_e] + j) * 128 + row_e] = ss_o - s * half
        g1.append(flat1)
        # layer-2 positions: owner-core pass-1 rank, table offset by half
        pos = (ss_o // nloc - 4 * s) * nloc + cfg.rank1_global[ss_o]
        flat2 = np.full(cfg.total_cols[s] * 128, half, dtype=np.int64)
        flat2[(cfg.col_off[s][tile_e] + j) * 128 + row_e] = pos
        g2.append(flat2)
        a = np.full((nrp, cfg.H), SENT_VAL, dtype=np.float32)
        a[:nloc] = cfg.adst1_full[core * nloc + info["order"][s]]
        adst1p.append(a)
    # cross: pass-1-order position j -> pass-0 partial row
    cross = np.zeros(nrp, dtype=np.int64)
    cross[:nloc] = info["rank"][0][info["order"][1]]
    # adst2 A-order: pass-0 position i -> pass-1 staged row
    a2ai = np.zeros(nrp, dtype=np.int64)
    a2ai[:nloc] = info["rank"][1][info["order"][0]]
    return {
        "gidx1": _wrap_idx(np.concatenate(g1)),
        "gidx2": _wrap_idx(np.concatenate(g2)),
        "adst1p": np.concatenate(adst1p, axis=0),
        "cross1": _wrap_idx(cross),
        "a2ai": _wrap_idx(a2ai),
    }


# ----------------------------------------------------------------------------
# Device program
# ----------------------------------------------------------------------------
def build_program(cfg, stop_after=99):
    from concourse.masks import make_identity

    nc = bacc.Bacc(None, target_bir_lowering=False, debug=True)
    H, d1, C2, F = cfg.H, cfg.d1, cfg.C2, cfg.F
    nloc, half, ntiles = cfg.nloc, cfg.half, cfg.ntiles
    nfull_tiles = cdiv(cfg.N, 128)
    NRP, GC = cfg.nrp, cfg.GC
    tail = nloc - (ntiles - 1) * 128
    ca_tiles = 33                       # hT AllGather chunk A: tiles 0..32
    ca_cols = ca_tiles * 128            # 3200
    cb_cols = nloc - ca_cols            # 3050

    # ---- external IO ----
    xT = nc.dram_tensor("xT", [F, cfg.N], DT.float32, kind="ExternalInput")
    w1aug_h = nc.dram_tensor("w1aug", [F, 128], DT.float32, kind="ExternalInput")
    tgr_h = nc.dram_tensor("tgr", [d1, 128], DT.bfloat16, kind="ExternalInput")
    w2t_h = nc.dram_tensor("w2t", [d1, C2], DT.bfloat16, kind="ExternalInput")
    b1_h = nc.dram_tensor("bias1r", [128, d1], DT.float32, kind="ExternalInput")
    b2_h = nc.dram_tensor("bias2r", [128, C2], DT.float32, kind="ExternalInput")
    sent1_h = nc.dram_tensor("sent1", [1, 256], DT.bfloat16, kind="ExternalInput")
    sent2_h = nc.dram_tensor("sent2", [1, 256], DT.bfloat16, kind="ExternalInput")
    tc01 = cfg.total_cols[0] + cfg.total_cols[1]
    gidx1_h = nc.dram_tensor("gidx1", [128, tc01 * 8], DT.int16, kind="ExternalInput")
    gidx2_h = nc.dram_tensor("gidx2", [128, tc01 * 8], DT.int16, kind="ExternalInput")
    adst1p_h = nc.dram_tensor("adst1p", [2 * NRP, H], DT.bfloat16, kind="ExternalInput")
    cross1_h = nc.dram_tensor("cross1", [128, NRP // 16], DT.int16, kind="ExternalInput")
    a2ai_h = nc.dram_tensor("a2ai", [128, NRP // 16], DT.int16, kind="ExternalInput")
    out_h = nc.dram_tensor("out", [NRP, C2], DT.float32, kind="ExternalOutput")

    # ---- internal DRAM ----
    t1 = [nc.dram_tensor(f"t1_{s}", [half + 1, 256], DT.bfloat16) for s in range(2)]
    t2 = [nc.dram_tensor(f"t2_{s}", [half + 1, 256], DT.bfloat16) for s in range(2)]
    part1 = nc.dram_tensor("part1", [NRP, 128], DT.float32)
    part2 = nc.dram_tensor("part2", [NRP, 128], DT.float32)
    adst2sc = nc.dram_tensor("adst2sc", [NRP, 64], DT.float32)
    hT_shA = nc.dram_tensor("hT_shA", [d1, ca_cols], DT.bfloat16)
    hT_shB = nc.dram_tensor("hT_shB", [d1, cb_cols], DT.bfloat16)
    hT_fullA = nc.dram_tensor("hT_fullA", [cfg.ncores * d1, ca_cols], DT.bfloat16)
    hT_fullB = nc.dram_tensor("hT_fullB", [cfg.ncores * d1, cb_cols], DT.bfloat16)

    try:
      with tile.TileContext(nc) as tc:
        with tc.tile_pool(name="const", bufs=1) as cpool, \
             tc.tile_pool(name="p0", bufs=4) as p0, \
             tc.tile_pool(name="p0ps", bufs=2, space="PSUM") as p0ps, \
             tc.tile_pool(name="pass", bufs=3) as pp, \
             tc.tile_pool(name="px", bufs=2) as pxp, \
             tc.tile_pool(name="gi", bufs=4) as gip, \
             tc.tile_pool(name="cmb", bufs=2) as cb, \
             tc.tile_pool(name="cps", bufs=1, space="PSUM") as cps, \
             tc.tile_pool(name="p3", bufs=4) as p3, \
             tc.tile_pool(name="p3ps", bufs=2, space="PSUM") as p3ps:
            w1s = cpool.tile([F, 128], DT.float32)
            nc.sync.dma_start(w1s[:], w1aug_h[:])
            tgr = cpool.tile([d1, 128], DT.bfloat16)
            nc.sync.dma_start(tgr[:], tgr_h[:])
            w2t = cpool.tile([d1, C2], DT.bfloat16)
            nc.sync.dma_start(w2t[:], w2t_h[:])
            b1s = cpool.tile([128, d1], DT.float32)
            nc.sync.dma_start(b1s[:], b1_h[:])
            b2s = cpool.tile([128, C2], DT.float32)
            nc.sync.dma_start(b2s[:], b2_h[:])
            ident = cpool.tile([128, 128], DT.float32)
            make_identity(nc, ident[:])
            cr1 = cpool.tile([128, NRP // 16], DT.int16)
            nc.sync.dma_start(cr1[:], cross1_h[:])
            a2ai = cpool.tile([128, NRP // 16], DT.int16)
            nc.sync.dma_start(a2ai[:], a2ai_h[:])
            adst1 = []
            for s in range(2):
                a = cpool.tile([128, ntiles, H], DT.bfloat16, tag=f"adst1_{s}")
                nc.sync.dma_start(
                    a[:],
                    adst1p_h[s * NRP:(s + 1) * NRP, :]
                    .rearrange("(t p) h -> p t h", p=128))
                adst1.append(a)
            accB1 = cpool.tile([128, ntiles, 72], DT.float32)
            accB2 = cpool.tile([128, ntiles, 65], DT.float32)
            adst2B = cpool.tile([128, ntiles, 1], DT.bfloat16)
            adst2A = cpool.tile([128, ntiles, 1], DT.bfloat16)

            # sentinel rows
            sc1 = cpool.tile([1, 256], DT.bfloat16, tag="sent1")
            nc.sync.dma_start(sc1[:], sent1_h[:])
            sc2 = cpool.tile([1, 256], DT.bfloat16, tag="sent2")
            nc.sync.dma_start(sc2[:], sent2_h[:])
            for s in range(2):
                nc.sync.dma_start(t1[s][half:half + 1, :], sc1[:])
                nc.sync.dma_start(t2[s][half:half + 1, :], sc2[:])

            # ================= P0: layer-1 table =================
            if stop_after < 1:
                raise _StopBuild()

            def p0_block(k0, nk):
                n0 = k0 * 128
                cnt = min(nk * 128, cfg.N - n0)
                xt_t = p0.tile([F, 4 * 128], DT.float32, tag="xt")
                nc.scalar.dma_start(xt_t[:, :cnt], xT[:, n0:n0 + cnt])
                row = p0.tile([128, 4, 72], DT.bfloat16, tag="row")
                for i in range(nk):
                    c = min(128, cfg.N - n0 - i * 128)
                    if c <= 0:
                        break
                    ps = p0ps.tile([128, 128], DT.float32, tag="ps",
                                   space="PSUM")
                    nc.tensor.matmul(ps[:c, :],
                                     lhsT=xt_t[:, i * 128:i * 128 + c],
                                     rhs=w1s[:], start=True, stop=True)
                    nc.vector.tensor_copy(out=row[:c, i, :],
                                          in_=ps[:c, 0:72])
                if True:
                    for i in range(nk):
                        m0 = n0 + i * 128
                        c = min(128, cfg.N - m0)
                        if c <= 0:
                            break
                        eng = nc.sync if i % 2 == 0 else nc.scalar
                        for s in range(2):
                            lo, hi = max(m0, s * half), min(m0 + c,
                                                           (s + 1) * half)
                            if lo < hi:
                                eng.dma_start(
                                    t1[s][lo - s * half:hi - s * half, 0:72],
                                    row[lo - m0:hi - m0, i, :])

            half0_tiles = half // 128 + 1          # tiles covering src half 0
            p0_blocks = [(k, min(4, nfull_tiles - k))
                         for k in range(0, nfull_tiles, 4)]
            nb_half0 = (half0_tiles + 3) // 4
            for (k0, nk) in p0_blocks[:nb_half0]:
                p0_block(k0, nk)

            # ================= pass machinery =================
            def grid_pass(layer, s, tbl, gidx_h, adst_tile, nheads, dfeat,
                          accum, part, per_tile_post, pre_group=None):
                """One slot-grid pass.  accum=None: write partial rows to
                `part` (pass 0).  accum=tile: reduce into SBUF (pass 1) and
                call per_tile_post(t, rows) after each tile."""
                base8 = (cfg.total_cols[0] if s == 1 else 0) * 8
                for gidx_i, (t0, t1_, c0, ncols) in enumerate(cfg.groups[s]):
                    if pre_group is not None:
                        pre_group(gidx_i)
                    gi = gip.tile([128, GC * 8], DT.int16, tag="gi")
                    nc.sync.dma_start(
                        gi[:, :ncols * 8],
                        gidx_h[:, base8 + c0 * 8: base8 + (c0 + ncols) * 8])
                    G = pp.tile([128, GC, 256], DT.bfloat16, tag="G")
                    nc.gpsimd.dma_gather(G[:, :ncols, :], tbl[:],
                                         gi[:, :ncols * 8], ncols * 128,
                                         ncols * 128, 256, single_packet=False)
                    pex = pxp.tile([128, GC, 64], DT.bfloat16, tag="px")
                    for t in range(t0, t1_):
                        D = int(cfg.D[s][t])
                        o = int(cfg.col_off[s][t]) - c0
                        Gt = G[:, o:o + D, :]
                        asrc = Gt[:, :, 64:64 + nheads]
                        if accum is None:
                            res = pp.tile([128, 128], DT.float32, tag="res")
                            nc.any.memset(res[:, 64 + nheads:], 0.0)
                            dn = res[:, 64:64 + nheads]
                            nm = res[:, 0:64]
                        else:
                            dn = accum[:, t, 64:64 + nheads]
                            nm = accum[:, t, 0:64]
                        pext = pex[:, o:o + D, :]
                        if nheads == 1:
                            # alpha+lrelu+exp+den all on the Act engine
                            p2 = pp.tile([128, GC, 1], DT.bfloat16, tag="p2")
                            nc.scalar.activation(
                                out=p2[:, :D, :], in_=asrc, func=AF.Lrelu,
                                bias=adst_tile[:, t, :], alpha=NEG_SLOPE)
                            nc.scalar.activation(
                                out=p2[:, :D, :], in_=p2[:, :D, :],
                                func=AF.Exp, accum_out=dn)
                            nc.vector.tensor_tensor(
                                out=pext, in0=Gt[:, :, 0:64],
                                in1=p2[:, :D, :].to_broadcast([128, D, 64]),
                                op=ALU.mult)
                        else:
                            al = pp.tile([128, GC, 8], DT.bfloat16, tag="al")
                            alt = al[:, :D, :nheads]
                            nc.vector.tensor_tensor(
                                out=alt, in0=asrc,
                                in1=adst_tile[:, t:t + 1, :]
                                    .to_broadcast([128, D, nheads]),
                                op=ALU.add)
                            nc.vector.scalar_tensor_tensor(
                                out=alt, in0=alt, scalar=NEG_SLOPE, in1=alt,
                                op0=ALU.mult, op1=ALU.max)
                            nc.scalar.activation(
                                out=pext,
                                in_=alt.rearrange("p j (h c) -> p j h c", c=1)
                                       .to_broadcast([128, D, nheads,
                                                      64 // nheads]),
                                func=AF.Exp)
                            nc.vector.tensor_reduce(
                                out=dn,
                                in_=pext.rearrange("p j (h c) -> p h c j",
                                                   h=nheads)[:, :, 0, :],
                                axis=AX.X, op=ALU.add)
                            nc.vector.tensor_tensor(
                                out=pext, in0=Gt[:, :, 0:64],
                                in1=pext, op=ALU.mult)
                        nc.vector.tensor_reduce(
                            out=nm, in_=pext.rearrange("p j f -> p f j"),
                            axis=AX.X, op=ALU.add)
                        if accum is None:
                            nc.sync.dma_start(
                                part[t * 128:(t + 1) * 128, :], res[:])
                        else:
                            per_tile_post(t, 128 if t < ntiles - 1 else tail)

            # ================= layer 1 =================
            if stop_after < 2:
                raise _StopBuild()
            p0_rest = p0_blocks[nb_half0:]
            ngr0 = len(cfg.groups[0])
            per_g = cdiv(len(p0_rest), max(ngr0 - 1, 1))

            def preA(gi_i):
                for (k0, nk) in p0_rest[gi_i * per_g:(gi_i + 1) * per_g]:
                    p0_block(k0, nk)

            grid_pass(1, 0, t1[0], gidx1_h, adst1[0], H, 64, None, part1,
                      None, pre_group=preA)

            if stop_after < 3:
                raise _StopBuild()

            def post1(t, rows):
                cg = cb.tile([128, 1, 128], DT.float32, tag="cg1")
                nc.gpsimd.dma_gather(cg[:], part1[:],
                                     cr1[:, t * 8:(t + 1) * 8], 128, 128, 128,
                                     single_packet=False)
                comb = cb.tile([128, 72], DT.float32, tag="comb1")
                nc.vector.tensor_tensor(out=comb[:], in0=cg[:, 0, 0:72],
                                        in1=accB1[:, t, :], op=ALU.add)
                rec = cb.tile([128, H], DT.float32, tag="rec1")
                nc.vector.tensor_scalar_add(rec[:], comb[:, 64:72], EPS)
                nc.vector.reciprocal(rec[:], rec[:])
                hf = cb.tile([128, 64], DT.float32, tag="hf")
                nc.vector.tensor_tensor(
                    out=hf[:].rearrange("p (h c) -> p h c", h=H),
                    in0=comb[:, 0:64].rearrange("p (h c) -> p h c", h=H),
                    in1=rec[:].rearrange("p (h c) -> p h c", c=1)
                              .to_broadcast([128, H, 8]),
                    op=ALU.mult)
                nc.vector.tensor_tensor(out=hf[:], in0=hf[:], in1=b1s[:],
                                        op=ALU.add)
                # elu(x) = relu(x) + exp(-relu(-x)) - 1
                r = cb.tile([128, 64], DT.float32, tag="r")
                nc.scalar.activation(out=r[:], in_=hf[:], func=AF.Relu)
                m = cb.tile([128, 64], DT.float32, tag="m")
                nc.scalar.activation(out=m[:], in_=hf[:], func=AF.Relu,
                                     scale=-1.0)
                e = cb.tile([128, 64], DT.float32, tag="e")
                nc.scalar.activation(out=e[:], in_=m[:], func=AF.Exp,
                                     scale=-1.0)
                nc.vector.tensor_tensor(out=r[:], in0=r[:], in1=e[:], op=ALU.add)
                nc.vector.tensor_scalar_add(r[:], r[:], -1.0)
                psT = cps.tile([64, 128], DT.float32, tag="psT", space="PSUM")
                nc.tensor.transpose(out=psT[:], in_=r[:, :], identity=ident[:])
                htb = cb.tile([64, 128], DT.bfloat16, tag="htb")
                nc.any.tensor_copy(out=htb[:], in_=psT[:])
                if t < ca_tiles:
                    nc.sync.dma_start(hT_shA[:, t * 128:t * 128 + rows],
                                      htb[:, :rows])
                else:
                    o = (t - ca_tiles) * 128
                    nc.sync.dma_start(hT_shB[:, o:o + rows], htb[:, :rows])
                psA = cps.tile([128, 1], DT.float32, tag="psA", space="PSUM")
                nc.tensor.matmul(psA[:], lhsT=htb[:], rhs=tgr[:, 65:66],
                                 start=True, stop=True)
                nc.any.tensor_copy(out=adst2B[:, t, :], in_=psA[:])
                adrow = cb.tile([128, 64], DT.float32, tag="adrow")
                nc.any.tensor_copy(out=adrow[:],
                                   in_=psA[:, 0:1].to_broadcast([128, 64]))
                nc.sync.dma_start(adst2sc[t * 128:(t + 1) * 128, :], adrow[:])
                if t == ca_tiles - 1:
                    nc.gpsimd.collective_compute(
                        "AllGather", ALU.bypass, ins=[hT_shA[:]],
                        outs=[hT_fullA[:]],
                        replica_groups=[list(range(cfg.ncores))])

            def t2_block(s2, kc4, u0, nu):
                kc = s2 * 4 + kc4
                u_end = u0 + nu
                cols = (min(u_end * 128, nloc)) - u0 * 128
                hts = p3.tile([d1, 4 * 128], DT.bfloat16, tag="hts")
                if u_end <= ca_tiles:
                    nc.scalar.dma_start(
                        hts[:, :cols],
                        hT_fullA[kc * d1:(kc + 1) * d1,
                                 u0 * 128:u0 * 128 + cols])
                else:
                    o = (u0 - ca_tiles) * 128
                    nc.scalar.dma_start(
                        hts[:, :cols],
                        hT_fullB[kc * d1:(kc + 1) * d1, o:o + cols])
                row = p3.tile([128, 4, 66], DT.bfloat16, tag="row2")
                for i in range(nu):
                    c = min(128, nloc - (u0 + i) * 128)
                    ps = p3ps.tile([128, 128], DT.float32, tag="ps2",
                                   space="PSUM")
                    nc.tensor.matmul(ps[:c, :],
                                     lhsT=hts[:, i * 128:i * 128 + c],
                                     rhs=tgr[:], start=True, stop=True)
                    nc.vector.tensor_copy(out=row[:c, i, :],
                                          in_=ps[:c, 0:66])
                ro = kc4 * nloc + u0 * 128
                if True:
                    for i in range(nu):
                        c = min(128, nloc - (u0 + i) * 128)
                        eng = nc.sync if i % 2 == 0 else nc.scalar
                        eng.dma_start(
                            t2[s2][ro + i * 128:ro + i * 128 + c, 0:66],
                            row[:c, i, :])

            # chunk-A u-blocks (never straddle the ca_tiles boundary)
            t2a_blocks, t2b_blocks = [], []
            u = 0
            while u < ca_tiles:
                nu = min(4, ca_tiles - u)
                t2a_blocks += [(s2, kc4, u, nu)
                               for s2 in range(2) for kc4 in range(4)]
                u += nu
            while u < ntiles:
                nu = min(4, ntiles - u)
                t2b_blocks += [(s2, kc4, u, nu)
                               for s2 in range(2) for kc4 in range(4)]
                u += nu

            # sprinkle t2 chunk-A builds into the tail groups of pass B
            ngr1 = len(cfg.groups[1])
            nspr = 3
            spr_start = ngr1 - nspr
            per_g2 = cdiv(len(t2a_blocks), nspr)

            def preB(gi_i):
                if gi_i >= spr_start:
                    i = gi_i - spr_start
                    for (s2, kc4, u0, nu) in \
                            t2a_blocks[i * per_g2:(i + 1) * per_g2]:
                        t2_block(s2, kc4, u0, nu)

            grid_pass(1, 1, t1[1], gidx1_h, adst1[1], H, 64, accB1, None,
                      post1, pre_group=preB)

            # adst2 in pass-0 order (before the blocking AG2 collective)
            ga = cpool.tile([128, ntiles, 64], DT.float32, tag="ga")
            nc.gpsimd.dma_gather(ga[:], adst2sc[:], a2ai[:], NRP, NRP, 64,
                                 single_packet=False)
            nc.vector.tensor_copy(out=adst2A[:, :, 0], in_=ga[:, :, 0])

            nc.gpsimd.collective_compute(
                "AllGather", ALU.bypass, ins=[hT_shB[:]], outs=[hT_fullB[:]],
                replica_groups=[list(range(cfg.ncores))])

            # ================= t2 build (chunk B; A was sprinkled) =========
            if stop_after < 4:
                raise _StopBuild()
            for (s2, kc4, u0, nu) in t2b_blocks:
                t2_block(s2, kc4, u0, nu)

            # ================= layer 2 =================
            if stop_after < 5:
                raise _StopBuild()
            grid_pass(2, 0, t2[0], gidx2_h, adst2A[:], 1, 64,
                      None, part2, None)

            if stop_after < 6:
                raise _StopBuild()

            def post2(t, rows):
                cg = cb.tile([128, 1, 128], DT.float32, tag="cg2")
                nc.gpsimd.dma_gather(cg[:], part2[:],
                                     cr1[:, t * 8:(t + 1) * 8], 128, 128, 128,
                                     single_packet=False)
                comb = cb.tile([128, 65], DT.float32, tag="comb2")
                nc.vector.tensor_tensor(out=comb[:], in0=cg[:, 0, 0:65],
                                        in1=accB2[:, t, :], op=ALU.add)
                rec = cb.tile([128, 1], DT.float32, tag="rec2")
                nc.vector.tensor_scalar_add(rec[:], comb[:, 64:65], EPS)
                nc.vector.reciprocal(rec[:], rec[:])
                gg = cb.tile([128, 64], DT.float32, tag="gg")
                nc.vector.tensor_tensor(
                    out=gg[:], in0=comb[:, 0:64],
                    in1=rec[:].to_broadcast([128, 64]), op=ALU.mult)
                psT = cps.tile([64, 128], DT.float32, tag="psT2", space="PSUM")
                nc.tensor.transpose(out=psT[:], in_=gg[:, :], identity=ident[:])
                gtb = cb.tile([64, 128], DT.bfloat16, tag="gtb")
                nc.any.tensor_copy(out=gtb[:], in_=psT[:])
                ps2 = cps.tile([128, C2], DT.float32, tag="ps22", space="PSUM")
                nc.tensor.matmul(ps2[:], lhsT=gtb[:], rhs=w2t[:],
                                 start=True, stop=True)
                o2 = cb.tile([128, C2], DT.float32, tag="o2")
                nc.vector.tensor_tensor(out=o2[:], in0=ps2[:], in1=b2s[:],
                                        op=ALU.add)
                nc.sync.dma_start(out_h[t * 128:t * 128 + 128, :], o2[:])

            grid_pass(2, 1, t2[1], gidx2_h, adst2B[:], 1, 64,
                      accB2, None, post2)

    except _StopBuild:
        pass
    nc.compile()
    return nc


# ----------------------------------------------------------------------------
# Host entry
# ----------------------------------------------------------------------------
def host_inputs(cfg, x, W1, att_src1, att_dst1, bias1, W2, att_src2,
                att_dst2, bias2):
    import ml_dtypes
    H, C1, C2, d1 = cfg.H, cfg.C1, cfg.C2, cfg.d1
    x = np.asarray(x, np.float32)
    W1 = np.asarray(W1, np.float32)
    A_src = np.zeros((d1, H), np.float32)
    A_dst = np.zeros((d1, H), np.float32)
    for h in range(H):
        A_src[h * C1:(h + 1) * C1, h] = np.asarray(att_src1, np.float32)[h]
        A_dst[h * C1:(h + 1) * C1, h] = np.asarray(att_dst1, np.float32)[h]
    w1aug = np.zeros((cfg.F, 128), np.float32)
    w1aug[:, :d1] = W1
    w1aug[:, d1:d1 + H] = W1 @ A_src
    cfg.adst1_full = x @ (W1 @ A_dst)

    W2 = np.asarray(W2, np.float32)
    tgr = np.zeros((d1, 128), np.float32)
    tgr[:, :d1] = np.eye(d1)
    tgr[:, d1:d1 + 1] = W2 @ np.asarray(att_src2, np.float32).T
    tgr[:, d1 + 1:d1 + 2] = W2 @ np.asarray(att_dst2, np.float32).T

    sent1 = np.zeros((1, 256), np.float32)
    sent1[0, d1:d1 + 2 * H] = SENT_VAL
    sent2 = np.zeros((1, 256), np.float32)
    sent2[0, d1:d1 + 2] = SENT_VAL

    common = {
        "xT": np.ascontiguousarray(x.T),
        "w1aug": w1aug,
        "tgr": tgr.astype(ml_dtypes.bfloat16),
        "w2t": W2.astype(ml_dtypes.bfloat16),
        "bias1r": np.tile(np.asarray(bias1, np.float32)[None, :], (128, 1)),
        "bias2r": np.tile(np.asarray(bias2, np.float32)[None, :], (128, 1)),
        "sent1": sent1.astype(ml_dtypes.bfloat16),
        "sent2": sent2.astype(ml_dtypes.bfloat16),
    }
    in_maps = []
    for c in range(cfg.ncores):
        r = build_routing(cfg, c)
        r["adst1p"] = r["adst1p"].astype(ml_dtypes.bfloat16)
        in_maps.append({**common, **r})
    return in_maps


_CACHE = {}


def _run(x, edge_index, W1, att_src1, att_dst1, bias1, W2, att_src2,
         att_dst2, bias2, **run_kwargs):
    x = np.asarray(x, dtype=np.float32)
    N, F = x.shape
    ei = np.asarray(edge_index).astype(np.int64)
    E = ei.shape[1]
    loops = np.arange(N, dtype=np.int64)
    src = np.concatenate([ei[0], loops])
    dst = np.concatenate([ei[1], loops])
    cfg = Cfg(N, F, E, src, dst)
    key = (N, F, E, tuple(cfg.D[0]), tuple(cfg.D[1]))
    if key not in _CACHE:
        _CACHE[key] = build_program(cfg)
    nc = _CACHE[key]
    in_maps = host_inputs(cfg, x, W1, att_src1, att_dst1, bias1,
                          W2, att_src2, att_dst2, bias2)
    res = run_bass_kernel_spmd(nc, in_maps, list(range(cfg.ncores)),
                               **run_kwargs)
    out = np.empty((N, cfg.C2), dtype=np.float32)
    for c in range(cfg.ncores):
        r = np.asarray(res.results[c]["out"], dtype=np.float32)
        out[c * cfg.nloc + cfg.cores[c]["order"][1]] = r[:cfg.nloc]
    return out, res


def kernel(x, edge_index, W1, att_src1, att_dst1, bias1, W2, att_src2,
           att_dst2, bias2):
    out, _ = _run(x, edge_index, W1, att_src1, att_dst1, bias1, W2,
                  att_src2, att_dst2, bias2)
    return out
